# revision 32
# baseline (speedup 1.0000x reference)
"""Trainium2 Bass kernel for nn_BiMaTrLayer (dual-path filter + bidirectional
Mamba/attention stack + GLU).  Data-parallel over 8 NeuronCores (4 samples per
core, processed as 2 passes of 2 samples).

v2: bf16 matmul datapath (weights host-cast, activations evacuated bf16),
selective-scan restructured: tensor_tensor_scan runs on the (otherwise idle)
Pool engine in n-pairs over channel-fused [128, 2*DIC*F] tiles, DVE keeps only
the dtu*B and h*C muls, and the sum over states accumulates in PSUM via PE
identity matmuls (D*u enters via a host-precomputed diagonal matmul).
dA = exp(a_n * dt) uses a literal scalar scale per state (A is d-independent
in this model; baked values are part of the program cache key).
"""

import sys
import hashlib
import numpy as np

sys.path.append("/opt/trn_rl_repo")

import concourse.bass as bass
from concourse import bacc


class _Bacc(bacc.Bacc):
    """Bacc with act-table steering: resolve Exp and Ln to the combined
    natural_log_exp_and_others set so softplus/LN chains don't ping-pong
    table loads (2.7us each)."""

    def insert_act_table_loads(self):
        import concourse.mybir as _mb
        from concourse.hw_specs import get_activation_tables
        from concourse import bacc as _bacc
        has_activation = any(
            isinstance(i, _mb.InstActivation)
            for b in self.main_func.blocks
            for i in b.instructions
        )
        if not has_activation:
            return
        tables = list(get_activation_tables(self.m.arch).items())
        AFT = _mb.ActivationFunctionType
        steer = {"exp_and_others": {AFT.Exp}, "exp_and_friends": {AFT.Exp},
                 "natural_log": {AFT.Ln}}
        tables = [(nm, fn - steer.get(nm, set())) for nm, fn in tables]
        _bacc._bass_rust.insert_act_table_loads(self, tables)

import concourse.mybir as mybir
import concourse.tile as tile
from concourse.masks import make_identity
from contextlib import ExitStack

AF = mybir.ActivationFunctionType
OP = mybir.AluOpType
F32 = mybir.dt.float32
BF16 = mybir.dt.bfloat16
P = 128

B, S, D = 32, 256, 256
NCORES = 8
BC = B // NCORES            # samples per core
PB = 2                      # samples per pass
NPASS = BC // PB
F = PB * S                  # 512: free dim (sample, time) per pass
DI, DS, DTR, NL, H, HD = 512, 16, 16, 2, 4, 64
DIC = DI // P
NPAIR = DS // 2
L2 = 69
NF = S // 2 + 1
DC = 4

DEC_LO = np.array([-0.010597401784997278, 0.032883011666982945,
                   0.030841381835986965, -0.18703481171888114,
                   -0.02798376941698385, 0.6308807679295904,
                   0.7148465705525415, 0.23037781330885523], np.float64)


def _bf16(a):
    import ml_dtypes
    return np.ascontiguousarray(np.asarray(a, np.float32).astype(ml_dtypes.bfloat16))


def _f32(a):
    return np.ascontiguousarray(np.asarray(a), np.float32)


# ----------------------------------------------------------------- host consts
def _dwt1_mat(L):
    out_full = L + 14 - 8 + 1
    idx = np.arange(1, out_full, 2)
    M = np.zeros((len(idx), L))
    for s in range(L):
        x = np.zeros(L)
        x[s] = 1.0
        y = np.correlate(np.pad(x, 7), DEC_LO[::-1], 'valid')
        M[:, s] = y[idx]
    return M


def _interp_mat(Lin, out_len):
    pos = (np.arange(out_len) + 0.5) * (Lin / out_len) - 0.5
    pos = np.clip(pos, 0.0, Lin - 1.0)
    lo = np.floor(pos).astype(int)
    hi = np.minimum(lo + 1, Lin - 1)
    t = pos - lo
    M = np.zeros((out_len, Lin))
    M[np.arange(out_len), lo] += 1.0 - t
    M[np.arange(out_len), hi] += t
    return M


def _fft_mats():
    s = np.arange(S)
    f = np.arange(NF)
    ang = 2 * np.pi * np.outer(f, s) / S
    Fr = np.cos(ang) / np.sqrt(S)
    Fi = -np.sin(ang) / np.sqrt(S)
    c = np.full(NF, 2.0)
    c[0] = 1.0
    c[-1] = 1.0
    angT = 2 * np.pi * np.outer(s, f) / S
    Gr = c * np.cos(angT) / np.sqrt(S)
    Gi = -c * np.sin(angT) / np.sqrt(S)
    Gi[:, 0] = 0.0
    Gi[:, -1] = 0.0
    return Fr, Fi, Gr, Gi


def _host_consts():
    Fr, Fi, Gr, Gi = _fft_mats()
    D1 = _dwt1_mat(S)
    D2 = _dwt1_mat(D1.shape[0])
    T = D2 @ D1
    I = _interp_mat(T.shape[0], S)
    return dict(frT=_bf16(Fr.T), fiT=_bf16(Fi.T), grT=_bf16(Gr.T),
                giT=_bf16(Gi.T), tdT=_bf16(T.T), iiT=_bf16(I.T))


def _prep_weights(inp):
    w = dict(_host_consts())
    w["fftWa"] = _bf16(np.concatenate([_f32(inp["fft_W"]).T,
                                       _f32(inp["fft_b"])[None, :]], 0))
    for nm in ("wl1", "wl2"):
        w[nm + "T"] = _bf16(_f32(inp[nm + "_W"]).transpose(2, 1, 0))
        w[nm + "b"] = _f32(np.asarray(inp[nm + "_b"])[:, None])
    qkv = _f32(inp["ca_Wqkv"])
    bqkv = _f32(inp["ca_bqkv"])
    wo = _f32(inp["ca_Wo"])
    w["caWqT"] = _bf16(qkv[0:D].T)
    w["caWkT"] = _bf16(qkv[D:2 * D].T)
    w["caWvT"] = _bf16(qkv[2 * D:].T)
    w["caWoT"] = _bf16(wo.T)
    w["caBq"] = _f32(bqkv[0:D][:, None])
    w["caBk"] = _f32(bqkv[D:2 * D][:, None])
    w["caBo"] = _f32((_f32(inp["ca_bo"]) + wo @ bqkv[2 * D:])[:, None])
    w["gateWT"] = _bf16(_f32(inp["gate_W"]).T)
    w["gateB"] = _f32(np.asarray(inp["gate_b"])[:, None])
    pidx = np.arange(P)
    for pre in ("mf", "mb"):
        w[pre + "inWT"] = _bf16(_f32(inp[pre + "_in_W"]).transpose(0, 2, 1))
        cw = _f32(inp[pre + "_conv_W"])          # [NL, DI, DC]
        cd = np.zeros((NL, P, DIC, DC, P), np.float32)
        cd[:, pidx, :, :, pidx] = cw.reshape(NL, DIC, P, DC).transpose(
            2, 0, 1, 3)                           # -> [P, NL, DIC, DC]
        w[pre + "convD"] = _bf16(cd)
        dv = _f32(inp[pre + "_D"])                # [NL, DI]
        dd = np.zeros((NL, P, DIC, P), np.float32)
        dd[:, pidx, :, pidx] = dv.reshape(NL, DIC, P).transpose(2, 0, 1)
        w[pre + "diagD"] = _bf16(dd)
        w[pre + "cols"] = _f32(np.stack([_f32(inp[pre + "_conv_b"]),
                                         _f32(inp[pre + "_dt_b"]),
                                         dv], -1))
        xp = _f32(inp[pre + "_xproj_W"]).transpose(0, 2, 1)   # [NL, DI, 48]
        perm = list(range(DTR, DTR + 2 * DS)) + list(range(DTR))  # [B;C;dt]
        w[pre + "xpT"] = _bf16(xp[:, :, perm])
        w[pre + "dtWT"] = _bf16(_f32(inp[pre + "_dt_W"]).transpose(0, 2, 1))
        w[pre + "outWT"] = _bf16(_f32(inp[pre + "_out_W"]).transpose(0, 2, 1))
    for pre in ("af", "ab"):
        qkv = _f32(inp[pre + "_Wqkv"])
        bqkv = _f32(inp[pre + "_bqkv"])
        wo = _f32(inp[pre + "_Wo"])
        w[pre + "WqT"] = _bf16(qkv[:, 0:D].transpose(0, 2, 1))
        w[pre + "WkT"] = _bf16(qkv[:, D:2 * D].transpose(0, 2, 1))
        w[pre + "WvT"] = _bf16(qkv[:, 2 * D:].transpose(0, 2, 1))
        w[pre + "WoT"] = _bf16(wo.transpose(0, 2, 1))
        w[pre + "Bq"] = _f32(bqkv[:, 0:D][:, :, None])
        w[pre + "Bk"] = _f32(bqkv[:, D:2 * D][:, :, None])
        w[pre + "Bo"] = _f32((_f32(inp[pre + "_bo"])
                              + np.einsum('lod,ld->lo', wo, bqkv[:, 2 * D:]))[:, :, None])
    fgrows = []
    for g, b in (("fl_ln_g", "fl_ln_b"), ("glu_ln_g", "glu_ln_b")):
        fgrows.append(np.stack([_f32(inp[g]), _f32(inp[b])], 0)[None])
    w["lnFG"] = _bf16(np.concatenate(fgrows, 0))         # [2, 2, D]
    rows = []
    for nm in ("anf", "anb", "nf", "nb"):
        rows += [_f32(inp[nm + "_g"])[:, None, :],
                 _f32(inp[nm + "_b"])[:, None, :]]
    w["lnAll"] = _bf16(np.concatenate(rows, 1))          # [NL, 8, D]
    w["glu1WT"] = _bf16(_f32(inp["glu1_W"]).T)
    w["glu1B"] = _f32(np.asarray(inp["glu1_b"])[:, None])
    w["glu2WT"] = _bf16(_f32(inp["glu2_W"]).T)
    w["glu2B"] = _f32(np.asarray(inp["glu2_b"])[:, None])
    return w


def _scan_consts(inp):
    """Per-(dir, layer, state) decay scales a_n = -exp(Alog); the model's Alog
    is d-independent, verified here; baked into the emitted program (cache-
    keyed on the values)."""
    av = {}
    for pre in ("mf", "mb"):
        al = _f32(inp[pre + "_Alog"])            # [NL, DI, DS]
        a = -np.exp(al.astype(np.float64))
        med = np.median(a, axis=1)               # [NL, DS]
        assert np.abs(a - med[:, None, :]).max() < 1e-5 * np.abs(med).max(), \
            "Alog is d-dependent; scalar-scale dA path invalid"
        av[pre] = med
    return av


# ----------------------------------------------------------------- emit helpers
class Emit:
    def __init__(self, nc, tc, ctx):
        self.nc, self.tc = nc, tc
        self.sb = ctx.enter_context(tc.tile_pool(name="sb", bufs=1))
        self.s2p = ctx.enter_context(tc.tile_pool(name="s2p", bufs=2))
        self.s3p = ctx.enter_context(tc.tile_pool(name="s3p", bufs=2))
        self.pp = ctx.enter_context(tc.tile_pool(name="pp", bufs=2, space="PSUM"))
        self.pn = ctx.enter_context(tc.tile_pool(name="pn", bufs=2, space="PSUM"))
        self.pyac = ctx.enter_context(tc.tile_pool(name="pyac", bufs=1, space="PSUM"))

    def load_wT(self, drh, K, M, tag):
        nc = self.nc
        if not isinstance(drh, bass.AP):
            drh = drh[:, :]
        kc_n = (K + P - 1) // P
        t = self.sb.tile([min(K, P), kc_n, M], BF16, tag=tag, name="wT")
        if K % P == 0:
            st = drh.ap[-1][0]
            src = bass.AP(tensor=drh.tensor, offset=drh.offset,
                          ap=[[M * st, P], [P * M * st, kc_n], [st, M]])
            nc.sync.dma_start(out=t, in_=src)
        else:
            for kc in range(kc_n):
                kp = min(P, K - kc * P)
                nc.sync.dma_start(out=t[:kp, kc, :], in_=drh[kc * P:kc * P + kp, :])
        return t

    def load_col(self, drh, M, tag):
        nc = self.nc
        if not isinstance(drh, bass.AP):
            drh = drh[:, :]
        mc_n = (M + P - 1) // P
        t = self.sb.tile([P, mc_n], F32, tag=tag, name="col")
        if M % P == 0:
            src = bass.AP(tensor=drh.tensor, offset=drh.offset,
                          ap=[[1, P], [P, mc_n]])
            nc.sync.dma_start(out=t, in_=src)
        else:
            for mc in range(mc_n):
                mp = min(P, M - mc * P)
                nc.sync.dma_start(out=t[:mp, mc:mc + 1],
                                  in_=drh[mc * P:mc * P + mp, :])
        return t

    def dense(self, x, wT, Mout, bias=None, act=None, out=None, out_pool=None,
              out_tag=None, Fw=None, out_dt=BF16):
        nc = self.nc
        Fw = Fw or F
        kc_n = x.shape[1]
        mc_n = (Mout + P - 1) // P
        if out is None:
            out = (out_pool or self.s3p).tile([P, mc_n, Fw], out_dt,
                                              tag=out_tag, name="dn")
        for mc in range(mc_n):
            mp = min(P, Mout - mc * P)
            ps = self.pp.tile([P, 512], F32, tag="mm", name="ps")
            for kc in range(kc_n):
                nc.tensor.matmul(ps[:mp, :Fw],
                                 wT[:, kc, mc * P:mc * P + mp],
                                 x[:, kc, 0:Fw],
                                 start=(kc == 0), stop=(kc == kc_n - 1))
            bap = bias[:mp, mc:mc + 1] if bias is not None else None
            if act is None and bias is None:
                nc.scalar.copy(out[:mp, mc, 0:Fw], ps[:mp, :Fw])
            else:
                nc.scalar.activation(out[:mp, mc, 0:Fw], ps[:mp, :Fw],
                                     act or AF.Identity,
                                     bias=bap if bap is not None else 0.0,
                                     scale=1.0)
        return out

    def add(self, out, a, b):
        self.nc.vector.tensor_add(out, a, b)

    def mul(self, out, a, b):
        self.nc.vector.tensor_mul(out, a, b)

    def act(self, out, in_, func, bias=0.0, scale=1.0):
        self.nc.scalar.activation(out=out, in_=in_, func=func, bias=bias, scale=scale)


def rev_view(ap2, n_blk, blk):
    st = ap2.ap[-1][0]
    off = ap2.offset + (blk - 1) * st
    if n_blk == 1:
        return bass.AP(tensor=ap2.tensor, offset=off, ap=[ap2.ap[0], [-st, blk]])
    return bass.AP(tensor=ap2.tensor, offset=off,
                   ap=[ap2.ap[0], [blk * st, n_blk], [-st, blk]])


def _g_layer_norm(E, x, gR, bR, eps, out, x_is_f32=False, tag=""):
    """x, out: [128, 2, F] feature-major (D=256 on partitions). gR/bR bf16
    rows [1, D].  Generator: yields at chunk boundaries."""
    nc = E.nc
    fw = F
    stat = E.sb.tile([1, 2, 512], F32, tag="lnstat" + tag, name="stat")
    A = stat[0:1, 0, :fw]          # m, later m*r
    Bv = stat[0:1, 1, :fw]         # q, later var, later r
    mrb = E.sb.tile([1, 2, 512], BF16, tag="lnthinb" + tag, name="mrb")
    xsq = E.sb.tile([P, 2, 512], BF16, tag="xsq", name="xsq")
    E.act(xsq, x, AF.Square)
    ones = E.ones128f if x_is_f32 else E.ones128
    for which, dst in ((0, A), (1, Bv)):
        ps = E.pn.tile([P, 512], F32, tag="th", name="ps")
        for kc in range(2):
            if which == 0:
                nc.tensor.matmul(ps[0:1, :fw], ones, x[:, kc, 0:fw],
                                 start=(kc == 0), stop=(kc == 1))
            else:
                nc.tensor.matmul(ps[0:1, :fw], E.ones128, xsq[:, kc, 0:fw],
                                 start=(kc == 0), stop=(kc == 1))
        nc.vector.tensor_scalar_mul(dst, ps[0:1, :fw], 1.0 / D)
        yield
    E.act(mrb[0:1, 0, :fw], A, AF.Square)          # m^2 (bf16 scratch)
    nc.vector.tensor_tensor(Bv, Bv, mrb[0:1, 0, :fw], OP.subtract)
    E.act(Bv, Bv, AF.Ln, bias=E.eps[eps][0:1, 0:1])
    E.act(Bv, Bv, AF.Exp, scale=-0.5)              # r (f32)
    nc.vector.tensor_copy(mrb[0:1, 0, :fw], Bv)    # r (bf16)
    E.mul(A, A, Bv)                                # m*r (f32, in place)
    E.act(mrb[0:1, 1, :fw], A, AF.Identity, scale=-1.0)   # -m*r (bf16)
    yield
    for mc in range(2):
        gRc = gR[0:1, mc * P:(mc + 1) * P]
        bRc = bR[0:1, mc * P:(mc + 1) * P]
        ps_s = E.pn.tile([P, 512], F32, tag="th", name="ps_s")
        nc.tensor.matmul(ps_s[:, :fw], gRc, mrb[0:1, 0, :fw], start=True, stop=True)
        ps_o = E.pn.tile([P, 512], F32, tag="th", name="ps_o")
        nc.tensor.matmul(ps_o[:, :fw], bRc, E.onesF[0:1, :fw],
                         start=True, stop=False)
        nc.tensor.matmul(ps_o[:, :fw], gRc, mrb[0:1, 1, :fw], start=False, stop=True)
        tmp = E.s2p.tile([P, 512], BF16, tag="lntmp", name="tmp", bufs=1)
        E.mul(tmp[:, :fw], x[:, mc, 0:fw], ps_s[:, :fw])
        E.add(out[:, mc, 0:fw], tmp[:, :fw], ps_o[:, :fw])
        yield


def _g_attention(E, q_src, kv_src, wq, wk, wv, wo, bq, bk, bo, out_tag, ob, okey):
    """MHA over PB samples; q_src/kv_src [128, 2, F] fm bf16.  Generator;
    result tile into ob[okey]."""
    nc = E.nc
    ofm = E.s3p.tile([P, 2, F], BF16, tag="t8", name="ofm")
    se = E.sb.tile([1, H, PB, S], BF16, tag="thin8", name="se")
    for b in range(PB):
        qf = E.s2p.tile([P, 2, S], BF16, tag="qfb", name="qf", bufs=1)
        kf = E.s2p.tile([P, 2, S], BF16, tag="kfb", name="kf", bufs=1)
        vtm = E.s2p.tile([P, 2, D], BF16, tag="vtmb", name="vtm", bufs=1)
        for mc in range(2):
            for dst, wT, bias in ((qf, wq, bq), (kf, wk, bk)):
                ps = E.pp.tile([P, 512], F32, tag="mm", name="ps")
                for kc in range(2):
                    nc.tensor.matmul(ps[:, :S], wT[:, kc, mc * P:(mc + 1) * P],
                                     q_src[:, kc, b * S:(b + 1) * S] if dst is qf
                                     else kv_src[:, kc, b * S:(b + 1) * S],
                                     start=(kc == 0), stop=(kc == 1))
                nc.scalar.activation(dst[:, mc, :], ps[:, :S], AF.Identity,
                                     bias=bias[:, mc:mc + 1], scale=1.0)
                yield
        for tcn in range(2):
            ps = E.pp.tile([P, 512], F32, tag="mm", name="ps")
            for kc in range(2):
                nc.tensor.matmul(ps[:, :D],
                                 kv_src[:, kc, b * S + tcn * P: b * S + (tcn + 1) * P],
                                 wv[:, kc, :], start=(kc == 0), stop=(kc == 1))
            nc.scalar.copy(vtm[:, tcn, :], ps[:, :D])
        yield
        pse = None
        for h in range(H):
            hc, off = h // 2, (h % 2) * 64
            expT = E.s2p.tile([P, 2, S], BF16, tag="expT", name="expT", bufs=1)
            ps = E.pp.tile([P, 512], F32, tag="mm", name="ps")
            for kc in range(2):
                nc.tensor.matmul(ps[:, kc * S:(kc + 1) * S],
                                 kf[off:off + 64, hc, kc * P:(kc + 1) * P],
                                 qf[off:off + 64, hc, :],
                                 start=True, stop=True)
            E.act(expT, ps, AF.Exp, scale=1.0 / np.sqrt(HD))
            if h % 2 == 0:
                pse = E.pn.tile([P, 512], F32, tag="th", name="pse")
            for kc in range(2):
                nc.tensor.matmul(pse[0:1, (h % 2) * S:(h % 2) * S + S],
                                 E.ones128, expT[:, kc, :],
                                 start=(kc == 0), stop=(kc == 1))
            if h % 2 == 1:
                E.act(se[0:1, h - 1:h + 1, b, :],
                      pse[0:1, :].rearrange("p (h s) -> p h s", h=2), AF.Ln)
            if h % 2 == 0:
                psav = E.pp.tile([P, 512], F32, tag="mm", name="psav")
            for kc in range(2):
                nc.tensor.matmul(psav[off:off + 64, :S],
                                 vtm[:, kc, h * 64:(h + 1) * 64],
                                 expT[:, kc, :], start=(kc == 0), stop=(kc == 1))
            if h % 2 == 1:
                nc.scalar.copy(ofm[:, hc, b * S:(b + 1) * S], psav[:, :S])
            yield
    E.act(se, se, AF.Exp, scale=-1.0)              # 1/sumexp, in place
    yield
    for h in range(H):
        dc, off = h // 2, (h % 2) * 64
        ps = E.pn.tile([P, 512], F32, tag="th", name="ps")
        nc.tensor.matmul(ps[0:64, :F], E.ones1x64,
                         se[0:1, h].rearrange("p b s -> p (b s)"),
                         start=True, stop=True)
        E.mul(ofm[off:off + 64, dc, :], ofm[off:off + 64, dc, :], ps[0:64, :F])
        yield
    ob[okey] = E.dense(ofm, wo, D, bias=bo, out_tag=out_tag)


def _g_mamba_prep_a(E, io, x, pre, l, flip, pr):
    """Silu-table phase: weights, in-proj xi, conv via host diag mats, z."""
    nc = E.nc
    d = pre
    inW = E.load_wT(io[pre + "inWT"][l], D, 2 * DI, "inW")
    cols = E.sb.tile([P, DIC, 3], F32, tag="mcols" + d, name="cols")
    cd = io[pre + "cols"][l]
    nc.sync.dma_start(out=cols, in_=bass.AP(
        tensor=cd.tensor, offset=cd.offset, ap=[[3, P], [P * 3, DIC], [1, 3]]))
    convD = E.sb.tile([P, DIC, DC, P], BF16, tag="convD", name="convD")
    nc.sync.dma_start(out=convD, in_=io[pre + "convD"][l])
    diagD = E.sb.tile([P, DIC, P], BF16, tag="diagD" + d, name="diagD")
    nc.sync.dma_start(out=diagD, in_=io[pre + "diagD"][l])
    xpw = E.load_wT(io[pre + "xpT"][l], DI, DTR + 2 * DS, "xpw" + d)
    dtw = E.sb.tile([2 * DS + DTR, DI], BF16, tag="dtw" + d, name="dtw")
    nc.sync.dma_start(out=dtw[2 * DS:, :], in_=io[pre + "dtWT"][l])
    ow = E.load_wT(io[pre + "outWT"][l], DI, D, "outW" + d)
    yield

    def inproj(c0, dst_tag, silu):
        dst = E.sb.tile([P, DIC, F], BF16, tag=dst_tag, name="xi")
        for c in range(DIC):
            ps = E.pp.tile([P, 512], F32, tag="mm", name="ps")
            for b in range(PB):
                for kc in range(2):
                    rhs = x[:, kc, b * S:(b + 1) * S]
                    if flip:
                        rhs = rev_view(rhs, 1, S)
                    nc.tensor.matmul(ps[:, b * S:(b + 1) * S],
                                     inW[:, kc, (c0 + c) * P:(c0 + c + 1) * P], rhs,
                                     start=(kc == 0), stop=(kc == 1))
            if silu:
                E.act(dst[:, c, :], ps, AF.Silu)
            else:
                nc.scalar.copy(dst[:, c, :], ps)
        return dst

    xi = inproj(0, "xi", False)
    yield
    xc = E.s2p.tile([P, DIC, F], BF16, tag="xc", name="xc")
    for c in range(DIC):
        ps = E.pp.tile([P, 512], F32, tag="mm", name="ps")
        for b in range(PB):
            nc.tensor.matmul(ps[:, b * S:(b + 1) * S], convD[:, c, DC - 1, :],
                             xi[:, c, b * S:(b + 1) * S], start=True, stop=False)
            for j in range(DC - 1):
                sh = DC - 1 - j
                nc.tensor.matmul(ps[:, b * S + sh:(b + 1) * S], convD[:, c, j, :],
                                 xi[:, c, b * S:(b + 1) * S - sh],
                                 start=False, stop=(j == DC - 2))
        E.act(xc[:, c, :], ps, AF.Silu, bias=cols[:, c, 0:1])
        yield
    z = inproj(DIC, "z" + d, True)
    yield
    pr.update(xc=xc, z=z, diagD=diagD, ow=ow, xpw=xpw, dtw=dtw, cols=cols)


def _g_mamba_prep_b(E, io, pr, pre, l, bcd):
    """NLE-table phase: xproj -> dbl (+DRAM bounce), dt softplus, dtu."""
    nc = E.nc
    d = pre
    xc, xpw, dtw, cols = pr["xc"], pr["xpw"], pr["dtw"], pr["cols"]
    dbl = E.sb.tile([DTR + 2 * DS, F], BF16, tag="dbl" + d, name="dbl")
    ps = E.pp.tile([P, 512], F32, tag="mm", name="ps")
    for kc in range(DIC):
        nc.tensor.matmul(ps[:DTR + 2 * DS, :F], xpw[:, kc, :], xc[:, kc, :],
                         start=(kc == 0), stop=(kc == DIC - 1))
    nc.scalar.copy(dbl, ps[:DTR + 2 * DS, :F])
    nc.sync.dma_start(out=bcd[:, :], in_=dbl[0:2 * DS, :])
    yield
    dt = E.sb.tile([P, DIC, F], BF16, tag="dt" + d, name="dt")
    for mc in range(DIC):
        ps = E.pp.tile([P, 512], F32, tag="mm", name="ps")
        nc.tensor.matmul(ps[:, :F], dtw[2 * DS:, mc * P:(mc + 1) * P],
                         dbl[2 * DS:2 * DS + DTR, :], start=True, stop=True)
        dtx = E.sb.tile([P, F], BF16, tag="dtx", name="dtx")
        E.act(dtx, ps[:, :F], AF.Exp, bias=cols[:, mc, 1:2])
        E.act(dt[:, mc, :], dtx, AF.Ln, bias=1.0)
        yield
    dtu = E.sb.tile([P, DIC, F], BF16, tag="dtu" + d, name="dtu")
    E.mul(dtu, dt, xc)
    nc.vector.memset(dt[:, :, 0:F:S], 1.0e30)
    pr.update(dt=dt, dtu=dtu, bcd=bcd)


def _mamba_scan(E, pr, avl, out_tag, bg, pump):
    """n-pair scan loop: dA on ACT (literal scalar scale), dBu / h*C muls on
    DVE (bf16), scans on the Pool engine, Sum_n C*h_n (+ D*u) accumulated in
    PSUM by PE identity matmuls; y*silu(z) reads PSUM directly on DVE."""
    nc = E.nc
    dt, dtu, z = pr["dt"], pr["dtu"], pr["z"]
    yac = [E.pyac.tile([P, 512], F32, tag=f"yac{c}", name="yac") for c in range(DIC)]
    for c in range(DIC):
        nc.tensor.matmul(yac[c], pr["diagD"][:, c, :], pr["xc"][:, c, :],
                         start=True, stop=False, skip_group_check=True)
    dt2 = dt[:, :, :].rearrange("p c f -> p (c f)")
    bcd = pr["bcd"][:, :]
    for n in range(DS):
        bg.pump(pump)
        dA = E.s2p.tile([P, DIC * F], BF16, tag="dA", name="dA")
        E.act(dA, dt2, AF.Exp, scale=float(avl[n]))
        bc = E.s2p.tile([P, 2, F], BF16, tag="bc", name="bc")
        src = bass.AP(tensor=bcd.tensor, offset=bcd.offset + n * F,
                      ap=[[0, P], [DS * F, 2], [1, F]])
        nc.sync.dma_start(out=bc, in_=src)
        dBu = E.s2p.tile([P, DIC, F], BF16, tag="dBu", name="dBu")
        bview = bass.AP(tensor=bc.tensor, offset=bc.offset,
                        ap=[bc.ap[0], [0, DIC], [1, F]])
        # the dtu*B broadcast mul rides the otherwise-idle Pool engine for
        # most states (Multiply has a real gpsimd impl; the scan does not)
        meng = nc.vector if n in (0, 8) else nc.gpsimd
        meng.tensor_mul(dBu, dtu, bview)
        hn = E.s2p.tile([P, DIC, F], BF16, tag="hn", name="hn")
        nc.vector.tensor_tensor_scan(
            out=hn[:, :, :].rearrange("p c f -> p (c f)"),
            data0=dA[:, :],
            data1=dBu[:, :, :].rearrange("p c f -> p (c f)"),
            initial=0.0, op0=OP.mult, op1=OP.add)
        hnC = E.sb.tile([P, DIC, F], BF16, tag="hnC", name="hnC")
        cview = bass.AP(tensor=bc.tensor, offset=bc.offset + F,
                        ap=[bc.ap[0], [0, DIC], [1, F]])
        E.mul(hnC, hn, cview)
        last = (n == DS - 1)
        for c in range(DIC):
            nc.tensor.matmul(yac[c], E.identb, hnC[:, c, :],
                             start=False, stop=last,
                             skip_group_check=True)
    bg.pump(2)
    y = E.sb.tile([P, DIC, F], BF16, tag="yb", name="y")
    for c in range(DIC):
        E.mul(y[:, c, :], z[:, c, :], yac[c])
    return E.dense(y, pr["ow"], D, out_pool=E.s2p, out_tag=out_tag)


# ------------------------------------------------------------------- program
def build_program(wshapes, av):
    nc = _Bacc()
    io = {}
    io["input"] = nc.declare_dram_parameter("input", [BC, S, D], F32, isOutput=False)
    for k, shp, dt in wshapes:
        io[k] = nc.declare_dram_parameter(k, list(shp), dt, isOutput=False)
    io["out"] = nc.declare_dram_parameter("out", [BC, S, D], F32, isOutput=True)
    for pss in range(NPASS):
        for l in range(NL):
            for pre in ("mf", "mb"):
                io[f"bcrows_{pss}_{l}_{pre}"] = nc.dram_tensor(
                    f"bcrows_{pss}_{l}_{pre}", [2 * DS, F], BF16)
    with tile.TileContext(nc) as tc:
        with ExitStack() as ctx:
            E = Emit(nc, tc, ctx)
            identb = E.sb.tile([P, P], BF16, tag="identb", name="identb")
            make_identity(nc, identb)
            E.identb = identb
            identf = E.sb.tile([P, P], F32, tag="identf", name="identf")
            make_identity(nc, identf)
            E.identf = identf
            E.ones128 = E.sb.tile([P, 1], BF16, tag="ones128", name="ones128")
            nc.vector.memset(E.ones128, 1.0)
            E.ones128f = E.sb.tile([P, 1], F32, tag="ones128f", name="ones128f")
            nc.vector.memset(E.ones128f, 1.0)
            E.ones1x64 = E.sb.tile([1, 64], BF16, tag="ones64", name="ones64")
            nc.vector.memset(E.ones1x64, 1.0)
            E.ones1xP = E.sb.tile([1, P], BF16, tag="ones1p", name="ones1p")
            nc.vector.memset(E.ones1xP, 1.0)
            E.onesF = E.sb.tile([1, 512], BF16, tag="onesF", name="onesF")
            nc.vector.memset(E.onesF, 1.0)
            E.eps = {}
            for ev in (1e-5, 1e-12):
                t = E.sb.tile([1, 1], F32, tag=f"eps{ev}", name="eps")
                nc.vector.memset(t, ev)
                E.eps[ev] = t
            # software-pipelined pass interleave: pass-1's FFT/wavelet/gate
            # stage and layer preps are emitted inside pass-0's scan windows
            # so the Pool engine (scans) never drains.
            bg = _BG()
            box = {}
            c00, c10, c01, c11 = {}, {}, {}, {}
            _run(_g_stage03(E, io, 0, box, "x1a"))
            _run(_g_layer_preps(E, io, 0, 0, lambda: box["x1a"], av, c00))
            bg.add(_chain(
                _g_stage03(E, io, 1, box, "x1b"),
                _g_layer_preps(E, io, 1, 0, lambda: box["x1b"], av, c10)))
            _emit_layer_scans(E, c00, av, bg)
            bg.drain()
            bg.add(_chain(
                _g_layer_post(E, c00, box, "x1a"),
                _g_layer_preps(E, io, 0, 1, lambda: box["x1a"], av, c01)))
            _emit_layer_scans(E, c10, av, bg)
            bg.drain()
            bg.add(_chain(
                _g_layer_post(E, c10, box, "x1b"),
                _g_layer_preps(E, io, 1, 1, lambda: box["x1b"], av, c11)))
            _emit_layer_scans(E, c01, av, bg)
            bg.drain()
            bg.add(_chain(
                _g_layer_post(E, c01, box, "x1a"),
                _g_glu(E, io, 0, lambda: box["x1a"])))
            s2d11 = {}
            _emit_layer_scans(E, c11, av, bg,
                              mid_add=_g_post_dir(E, c11, "mf", "af", s2d11))
            bg.drain()
            _run(_g_layer_post(E, c11, box, "x1b", s2d=s2d11))
            _run(_g_glu(E, io, 1, lambda: box["x1b"]))
    nc.finalize()
    return nc


class _BG:
    def __init__(self):
        from collections import deque
        self.q = deque()

    def add(self, gen):
        self.q.append(gen)

    def pump(self, n=1):
        while n > 0 and self.q:
            try:
                next(self.q[0])
                n -= 1
            except StopIteration:
                self.q.popleft()

    def drain(self):
        while self.q:
            self.pump(64)


def _run(gen):
    for _ in gen:
        pass


def _chain(*gens):
    for g in gens:
        yield from g


def _g_stage03(E, io, pss, box, key):
    nc = E.nc
    # ---------------- stage 0: load x + cast + transpose to feature-major
    x_tm = E.sb.tile([P, PB * 2, D], BF16, tag="xtm", name="x_tm")
    for b in range(PB):
        for sc in range(2):
            xch = E.s2p.tile([P, D], F32, tag="xt32", name="xch")
            nc.sync.dma_start(out=xch,
                              in_=io["input"][pss * PB + b, sc * P:(sc + 1) * P, :])
            nc.vector.tensor_copy(x_tm[:, b * 2 + sc, :], xch)
    yield
    x_fm = E.sb.tile([P, 2, F], BF16, tag="xfm", name="x_fm")
    for b in range(PB):
        for sc in range(2):
            for dc in range(2):
                pst = E.pn.tile([P, P], BF16, tag="th", name="pst")
                nc.tensor.transpose(pst, x_tm[:, b * 2 + sc, dc * P:(dc + 1) * P],
                                    E.identb)
                nc.scalar.copy(x_fm[:, dc, b * S + sc * P: b * S + (sc + 1) * P], pst)
            yield

    # ---------------- stage 1: FFT path
    frT = E.load_wT(io["frT"], S, NF, "frT")
    fiT = E.load_wT(io["fiT"], S, NF, "fiT")
    fftWa = E.load_wT(io["fftWa"], 513, 2 * D, "fftWa")
    grT = E.load_wT(io["grT"], NF, S, "grT")
    giT = E.load_wT(io["giT"], NF, S, "giT")
    yield
    x_fft = E.sb.tile([P, 2, F], BF16, tag="xfft", name="x_fft")
    for b in range(PB):
        comb = E.s3p.tile([P, 4, NF], BF16, tag="t8", name="comb")
        for ri, mat in ((0, frT), (1, fiT)):
            for mc in range(2):
                ps = E.pp.tile([P, 512], F32, tag="mm", name="ps")
                for kc in range(2):
                    nc.tensor.matmul(ps[:, :NF], x_tm[:, b * 2 + kc, mc * P:(mc + 1) * P],
                                     mat[:, kc, :], start=(kc == 0), stop=(kc == 1))
                nc.scalar.copy(comb[:, ri * 2 + mc, :], ps[:, :NF])
                yield
        filt = E.s3p.tile([P, 2 * D], BF16, tag="t8", name="filt")
        filtN = E.sb.tile([1, 2 * D], BF16, tag="filtN", name="filtN")
        for mt, mp, f0 in ((filt, P, 0), (filtN, 1, P)):
            ps = E.pp.tile([P, 512], F32, tag="mm", name="ps")
            for kc in range(4):
                nc.tensor.matmul(ps[:mp, :], comb[:, kc, f0:f0 + mp], fftWa[:, kc, :],
                                 start=(kc == 0), stop=False)
            nc.tensor.matmul(ps[:mp, :], E.ones1xP[0:1, 0:mp], fftWa[0:1, 4, :],
                             start=False, stop=True)
            E.act(mt[0:mp, :] if mt is filtN else mt, ps[:mp, :], AF.Gelu)
            yield
        for mc in range(2):
            ps = E.pp.tile([P, 512], F32, tag="mm", name="ps")
            nc.tensor.matmul(ps[:, :S], filt[:, mc * P:(mc + 1) * P], grT[:, 0, :],
                             start=True, stop=False)
            nc.tensor.matmul(ps[:, :S], filtN[0:1, mc * P:(mc + 1) * P], grT[0:1, 1, :],
                             start=False, stop=False)
            nc.tensor.matmul(ps[:, :S], filt[:, D + mc * P:D + (mc + 1) * P], giT[:, 0, :],
                             start=False, stop=False)
            nc.tensor.matmul(ps[:, :S], filtN[0:1, D + mc * P:D + (mc + 1) * P],
                             giT[0:1, 1, :], start=False, stop=True)
            nc.scalar.copy(x_fft[:, mc, b * S:(b + 1) * S], ps[:, :S])
            yield

    # ---------------- stage 2: wavelet path
    tdT = E.load_wT(io["tdT"], S, L2, "tdT")
    iiT = E.sb.tile([L2, S], BF16, tag="iiT", name="iiT")
    nc.sync.dma_start(out=iiT, in_=io["iiT"][:, :])
    wl1T = [E.load_wT(io["wl1T"][k], D, D, t) for k, t in enumerate(("awq", "awk", "awv"))]
    wl2T = [E.load_wT(io["wl2T"][k], D, D, t) for k, t in enumerate(("awo", "wlo1", "wlo2"))]
    wl1b = E.load_col(io["wl1b"], D, "wl1b")
    wl2b = E.load_col(io["wl2b"], D, "wl2b")
    yield
    x_wl = E.sb.tile([P, 2, F], BF16, tag="xwl", name="x_wl")
    a_fm = E.sb.tile([P, 2, PB, L2], BF16, tag="afm", name="a_fm")
    for b in range(PB):
        for mc in range(2):
            ps = E.pp.tile([P, 512], F32, tag="mm", name="ps")
            for kc in range(2):
                nc.tensor.matmul(ps[:, :L2], x_tm[:, b * 2 + kc, mc * P:(mc + 1) * P],
                                 tdT[:, kc, :], start=(kc == 0), stop=(kc == 1))
            nc.scalar.copy(a_fm[:, mc, b, :], ps[:, :L2])
    yield

    def conv3(src, wT, bcol, actf, dst_tag):
        dst = E.s2p.tile([P, 2, PB, L2], BF16, tag=dst_tag, name="c3")
        for b in range(PB):
            for mc in range(2):
                ps = E.pp.tile([P, 512], F32, tag="mm", name="ps")
                for kc in range(2):
                    nc.tensor.matmul(ps[:, :L2], wT[1][:, kc, mc * P:(mc + 1) * P],
                                     src[:, kc, b, :], start=(kc == 0), stop=False)
                for kc in range(2):
                    nc.tensor.matmul(ps[:, 1:L2], wT[0][:, kc, mc * P:(mc + 1) * P],
                                     src[:, kc, b, 0:L2 - 1], start=False, stop=False)
                for kc in range(2):
                    nc.tensor.matmul(ps[:, 0:L2 - 1], wT[2][:, kc, mc * P:(mc + 1) * P],
                                     src[:, kc, b, 1:L2], start=False, stop=(kc == 1))
                E.act(dst[:, mc, b, :], ps[:, :L2], actf, bias=bcol[:, mc:mc + 1])
        return dst

    c1 = conv3(a_fm, wl1T, wl1b, AF.Gelu, "c1")  # s2p ring
    yield
    c2 = conv3(c1, wl2T, wl2b, AF.Identity, "afm")
    yield
    c2T = E.sb.tile([L2, 2, PB, P], BF16, tag="c2T", name="c2T")
    for b in range(PB):
        for mc in range(2):
            pst = E.pn.tile([P, P], BF16, tag="th", name="pst")
            nc.tensor.transpose(pst[0:L2, :], c2[:, mc, b, :], E.identb)
            nc.scalar.copy(c2T[:, mc, b, :], pst[0:L2, :])
    yield
    for b in range(PB):
        for mc in range(2):
            ps = E.pp.tile([P, 512], F32, tag="mm", name="ps")
            nc.tensor.matmul(ps[:, :S], c2T[:, mc, b, :], iiT, start=True, stop=True)
            nc.scalar.copy(x_wl[:, mc, b * S:(b + 1) * S], ps[:, :S])
    yield

    # ---------------- stage 3: cross-attention + gate + LN
    caWq = E.load_wT(io["caWqT"], D, D, "awq")
    caWk = E.load_wT(io["caWkT"], D, D, "awk")
    caWv = E.load_wT(io["caWvT"], D, D, "awv")
    caWo = E.load_wT(io["caWoT"], D, D, "awo")
    caBq = E.load_col(io["caBq"], D, "abq")
    caBk = E.load_col(io["caBk"], D, "abk")
    caBo = E.load_col(io["caBo"], D, "abo")
    ab = {}
    yield from _g_attention(E, x_fft, x_wl, caWq, caWk, caWv, caWo, caBq, caBk,
                            caBo, "t8", ab, "att")
    fused = E.s3p.tile([P, 2, F], BF16, tag="t8", name="fused")
    E.add(fused, ab["att"], x_fm)
    gateW = E.load_wT(io["gateWT"], 2 * D, 2 * D, "gateW")
    gateB = E.load_col(io["gateB"], 2 * D, "gateB")
    ga = E.s3p.tile([P, 2, F], BF16, tag="t8", name="ga")
    gb = E.s3p.tile([P, 2, F], BF16, tag="t8", name="gb")
    for mc in range(4):
        actf = AF.Identity if mc < 2 else AF.Sigmoid
        gdst = ga if mc < 2 else gb
        ps = E.pp.tile([P, 512], F32, tag="mm", name="ps")
        for kc in range(4):
            gsrc = fused if kc < 2 else x_fm
            nc.tensor.matmul(ps[:, :F], gateW[:, kc, mc * P:(mc + 1) * P],
                             gsrc[:, kc % 2, :], start=(kc == 0), stop=(kc == 3))
        E.act(gdst[:, mc % 2, :], ps[:, :F], actf, bias=gateB[:, mc:mc + 1])
        yield
    gated = ga
    E.mul(gated, ga, gb)
    flt = E.s2p.tile([1, 2, D], BF16, tag="lnFG", name="flt")
    nc.sync.dma_start(out=flt, in_=io["lnFG"][0])
    x1 = E.s2p.tile([P, 2, F], BF16, tag="x1", name="x1", bufs=3)
    yield from _g_layer_norm(E, gated, flt[0:1, 0, :], flt[0:1, 1, :], 1e-5, x1)
    box[key] = x1


_DIRS = (("mf", "af", False, "anf", "nf"),
         ("mb", "ab", True, "anb", "nb"))


def _g_layer_preps(E, io, pss, l, x1f, av, cd):
    x1 = x1f()
    prs = {}
    for (mp, ap_, flip, anG, nG) in _DIRS:
        prs[mp] = {}
        yield from _g_mamba_prep_a(E, io, x1, mp, l, flip, prs[mp])
    for (mp, ap_, flip, anG, nG) in _DIRS:
        bcd = io[f"bcrows_{pss}_{l}_{mp}"]
        yield from _g_mamba_prep_b(E, io, prs[mp], mp, l, bcd)
    cd.update(prs=prs, x1=x1, l=l, io=io, pss=pss)


def _emit_layer_scans(E, cd, av, bg, mid_add=None):
    cd["ms"] = {}
    for di, (mp, ap_, flip, anG, nG) in enumerate(_DIRS):
        cd["ms"][mp] = _mamba_scan(E, cd["prs"][mp], av[mp][cd["l"]],
                                   "ms" + mp, bg, pump=2 if di == 0 else 4)
        if di == 0 and mid_add is not None:
            bg.add(mid_add)


def _g_post_dir(E, cd, mp, ap_, s2d):
    nc = E.nc
    io, l = cd["io"], cd["l"]
    ab = {}
    wq = E.load_wT(io[ap_ + "WqT"][l], D, D, "awq" + mp)
    wk = E.load_wT(io[ap_ + "WkT"][l], D, D, "awk" + mp)
    wv = E.load_wT(io[ap_ + "WvT"][l], D, D, "awv" + mp)
    wo = E.load_wT(io[ap_ + "WoT"][l], D, D, "awo" + mp)
    abq = E.load_col(io[ap_ + "Bq"][l], D, "abq" + mp)
    abk = E.load_col(io[ap_ + "Bk"][l], D, "abk" + mp)
    abo = E.load_col(io[ap_ + "Bo"][l], D, "abo" + mp)
    ms = cd["ms"][mp]
    yield from _g_attention(E, ms, ms, wq, wk, wv, wo, abq, abk, abo,
                            "t8", ab, "att")
    E.add(ms, ms, ab["att"])
    s2d[mp] = ms
    yield


def _g_layer_post(E, cd, box, key, s2d=None):
    nc = E.nc
    io, l = cd["io"], cd["l"]
    x1 = cd["x1"]
    lnt = E.s2p.tile([1, 8, D], BF16, tag="lnAll", name="lnt")
    nc.sync.dma_start(out=lnt, in_=io["lnAll"][l])
    nidx = {"anf": 0, "anb": 1, "nf": 2, "nb": 3}

    def ln_params(name):
        i = nidx[name] * 2
        return (lnt[0:1, i, :], lnt[0:1, i + 1, :])

    if s2d is None:
        s2d = {}
    s5d = {}
    for (mp, ap_, flip, anG, nG) in _DIRS:
        if mp in s2d:
            continue
        yield from _g_post_dir(E, cd, mp, ap_, s2d)
    s3d = {}
    for (mp, ap_, flip, anG, nG) in _DIRS:
        s3 = E.s3p.tile([P, 2, F], BF16, tag="t8", name="s3")
        (ang, anb_) = ln_params(anG)
        yield from _g_layer_norm(E, s2d[mp], ang, anb_, 1e-5, s3)
        s4 = E.s3p.tile([P, 2, F], BF16, tag="t8", name="s4")
        if flip:
            for kc in range(2):
                E.add(s4[:, kc, :].rearrange("p (b s) -> p b s", b=PB),
                      rev_view(s3[:, kc, :], PB, S),
                      x1[:, kc, :].rearrange("p (b s) -> p b s", b=PB))
        else:
            E.add(s4, s3, x1)
        s3d[mp] = s4
        yield
    for (mp, ap_, flip, anG, nG) in _DIRS:
        s5 = E.s2p.tile([P, 2, F], BF16, tag="s5", name="s5")
        (ng, nb_) = ln_params(nG)
        yield from _g_layer_norm(E, s3d[mp], ng, nb_, 1e-5, s5)
        s5d[mp] = s5
    x1n = E.s2p.tile([P, 2, F], BF16, tag="x1", name="x1n", bufs=3)
    E.add(x1n, s5d["mf"], s5d["mb"])
    box[key] = x1n


def _g_glu(E, io, pss, x1f):
    nc = E.nc
    x1 = x1f()
    # ---------------- stage 5: GLU + final LN
    glu1W = E.load_wT(io["glu1WT"], D, 2 * D, "glu1W")
    glu1B = E.load_col(io["glu1B"], 2 * D, "glu1B")
    va = E.s3p.tile([P, 2, F], BF16, tag="t8", name="va")
    vb = E.s3p.tile([P, 2, F], BF16, tag="t8", name="vb")
    for mc in range(4):
        actf = AF.Identity if mc < 2 else AF.Sigmoid
        vdst = va if mc < 2 else vb
        ps = E.pp.tile([P, 512], F32, tag="mm", name="ps")
        for kc in range(2):
            nc.tensor.matmul(ps[:, :F], glu1W[:, kc, mc * P:(mc + 1) * P],
                             x1[:, kc, :], start=(kc == 0), stop=(kc == 1))
        E.act(vdst[:, mc % 2, :], ps[:, :F], actf, bias=glu1B[:, mc:mc + 1])
        yield
    gv = va
    E.mul(gv, va, vb)
    glu2W = E.load_wT(io["glu2WT"], D, D, "glu2W")
    glu2B = E.load_col(io["glu2B"], D, "glu2B")
    gvo = E.dense(gv, glu2W, D, bias=glu2B, out_tag="t8")
    yield
    res = E.sb.tile([P, 2, F], F32, tag="res", name="res")
    E.add(res, gvo, x1)
    glt = E.s2p.tile([1, 2, D], BF16, tag="lnFG", name="glt")
    nc.sync.dma_start(out=glt, in_=io["lnFG"][1])
    out_fm = E.sb.tile([P, 2, F], F32, tag="reso", name="out_fm")
    yield from _g_layer_norm(E, res, glt[0:1, 0, :], glt[0:1, 1, :], 1e-12, out_fm,
                             x_is_f32=True)

    # ---------------- stage 6: transpose + store
    for b in range(PB):
        for sc in range(2):
            ot = E.sb.tile([P, D], F32, tag="otile", name="ot")
            for dc in range(2):
                pst = E.pn.tile([P, P], F32, tag="th", name="pst")
                nc.tensor.transpose(pst, out_fm[:, dc, b * S + sc * P: b * S + (sc + 1) * P],
                                    E.identf)
                nc.scalar.copy(ot[:, dc * P:(dc + 1) * P], pst)
            nc.sync.dma_start(out=io["out"][pss * PB + b, sc * P:(sc + 1) * P, :], in_=ot)
            yield


# ------------------------------------------------------------------- driver
_CACHE = {}


def _get_program(w, av):
    wshapes = []
    for k, v in sorted(w.items()):
        dt = BF16 if v.dtype.itemsize == 2 else F32
        wshapes.append((k, tuple(v.shape), dt))
    avh = hashlib.sha256(
        b"".join(np.ascontiguousarray(av[p]).tobytes() for p in ("mf", "mb"))
    ).hexdigest()
    key = (tuple(wshapes), avh)
    if key not in _CACHE:
        _CACHE[key] = build_program(wshapes, av)
    return _CACHE[key]


def kernel(**inputs):
    from concourse.bass_utils import run_bass_kernel_spmd
    w = _prep_weights(inputs)
    av = _scan_consts(inputs)
    nc = _get_program(w, av)
    x = np.ascontiguousarray(np.asarray(inputs["input_tensor"], np.float32))
    in_maps = []
    for core in range(NCORES):
        m = {"input": np.ascontiguousarray(x[core * BC:(core + 1) * BC])}
        m.update(w)
        in_maps.append(m)
    res = run_bass_kernel_spmd(nc, in_maps, list(range(NCORES)))
    return np.concatenate([res.results[i]["out"] for i in range(NCORES)], axis=0)


# revision 44
# speedup vs baseline: 1.5687x; 1.5687x over previous
"""Trainium2 Bass kernel for nn_BiMaTrLayer (dual-path filter + bidirectional
Mamba/attention stack + GLU).  Data-parallel over 8 NeuronCores (4 samples per
core, processed as 2 passes of 2 samples).

v2: bf16 matmul datapath (weights host-cast, activations evacuated bf16),
selective-scan restructured: tensor_tensor_scan runs on the (otherwise idle)
Pool engine in n-pairs over channel-fused [128, 2*DIC*F] tiles, DVE keeps only
the dtu*B and h*C muls, and the sum over states accumulates in PSUM via PE
identity matmuls (D*u enters via a host-precomputed diagonal matmul).
dA = exp(a_n * dt) uses a literal scalar scale per state (A is d-independent
in this model; baked values are part of the program cache key).
"""

import sys
import hashlib
import numpy as np

sys.path.append("/opt/trn_rl_repo")

import concourse.bass as bass
from concourse import bacc


class _Bacc(bacc.Bacc):
    """Bacc with act-table steering: resolve Exp and Ln to the combined
    natural_log_exp_and_others set so softplus/LN chains don't ping-pong
    table loads (2.7us each)."""

    def insert_act_table_loads(self):
        import concourse.mybir as _mb
        from concourse.hw_specs import get_activation_tables
        from concourse import bacc as _bacc
        has_activation = any(
            isinstance(i, _mb.InstActivation)
            for b in self.main_func.blocks
            for i in b.instructions
        )
        if not has_activation:
            return
        tables = list(get_activation_tables(self.m.arch).items())
        AFT = _mb.ActivationFunctionType
        steer = {"exp_and_others": {AFT.Exp}, "exp_and_friends": {AFT.Exp},
                 "natural_log": {AFT.Ln}}
        tables = [(nm, fn - steer.get(nm, set())) for nm, fn in tables]
        _bacc._bass_rust.insert_act_table_loads(self, tables)

import concourse.mybir as mybir
import concourse.tile as tile
from concourse.masks import make_identity
from contextlib import ExitStack

AF = mybir.ActivationFunctionType
OP = mybir.AluOpType
F32 = mybir.dt.float32
BF16 = mybir.dt.bfloat16
P = 128

B, S, D = 32, 256, 256
NCORES = 8
BC = B // NCORES            # samples per core
PB = 2                      # samples per pass
NPASS = BC // PB
F = PB * S                  # 512: free dim (sample, time) per pass
DI, DS, DTR, NL, H, HD = 512, 16, 16, 2, 4, 64
DIC = DI // P
NPAIR = DS // 2
L2 = 69
NF = S // 2 + 1
DC = 4

DEC_LO = np.array([-0.010597401784997278, 0.032883011666982945,
                   0.030841381835986965, -0.18703481171888114,
                   -0.02798376941698385, 0.6308807679295904,
                   0.7148465705525415, 0.23037781330885523], np.float64)


def _bf16(a):
    import ml_dtypes
    return np.ascontiguousarray(np.asarray(a, np.float32).astype(ml_dtypes.bfloat16))


def _f32(a):
    return np.ascontiguousarray(np.asarray(a), np.float32)


# ----------------------------------------------------------------- host consts
def _dwt1_mat(L):
    out_full = L + 14 - 8 + 1
    idx = np.arange(1, out_full, 2)
    M = np.zeros((len(idx), L))
    for s in range(L):
        x = np.zeros(L)
        x[s] = 1.0
        y = np.correlate(np.pad(x, 7), DEC_LO[::-1], 'valid')
        M[:, s] = y[idx]
    return M


def _interp_mat(Lin, out_len):
    pos = (np.arange(out_len) + 0.5) * (Lin / out_len) - 0.5
    pos = np.clip(pos, 0.0, Lin - 1.0)
    lo = np.floor(pos).astype(int)
    hi = np.minimum(lo + 1, Lin - 1)
    t = pos - lo
    M = np.zeros((out_len, Lin))
    M[np.arange(out_len), lo] += 1.0 - t
    M[np.arange(out_len), hi] += t
    return M


def _fft_mats():
    s = np.arange(S)
    f = np.arange(NF)
    ang = 2 * np.pi * np.outer(f, s) / S
    Fr = np.cos(ang) / np.sqrt(S)
    Fi = -np.sin(ang) / np.sqrt(S)
    c = np.full(NF, 2.0)
    c[0] = 1.0
    c[-1] = 1.0
    angT = 2 * np.pi * np.outer(s, f) / S
    Gr = c * np.cos(angT) / np.sqrt(S)
    Gi = -c * np.sin(angT) / np.sqrt(S)
    Gi[:, 0] = 0.0
    Gi[:, -1] = 0.0
    return Fr, Fi, Gr, Gi


def _host_consts():
    Fr, Fi, Gr, Gi = _fft_mats()
    D1 = _dwt1_mat(S)
    D2 = _dwt1_mat(D1.shape[0])
    T = D2 @ D1
    I = _interp_mat(T.shape[0], S)
    return dict(frT=_bf16(Fr.T), fiT=_bf16(Fi.T), grT=_bf16(Gr.T),
                giT=_bf16(Gi.T), tdT=_bf16(T.T), iiT=_bf16(I.T))


def _prep_weights(inp):
    w = dict(_host_consts())
    w["fftWa"] = _bf16(np.concatenate([_f32(inp["fft_W"]).T,
                                       _f32(inp["fft_b"])[None, :]], 0))
    for nm in ("wl1", "wl2"):
        w[nm + "T"] = _bf16(_f32(inp[nm + "_W"]).transpose(2, 1, 0))
        w[nm + "b"] = _f32(np.asarray(inp[nm + "_b"])[:, None])
    qkv = _f32(inp["ca_Wqkv"])
    bqkv = _f32(inp["ca_bqkv"])
    wo = _f32(inp["ca_Wo"])
    w["caWqT"] = _bf16(qkv[0:D].T)
    w["caWkT"] = _bf16(qkv[D:2 * D].T)
    w["caWvT"] = _bf16(qkv[2 * D:].T)
    w["caWoT"] = _bf16(wo.T)
    w["caBq"] = _f32(bqkv[0:D][:, None])
    w["caBk"] = _f32(bqkv[D:2 * D][:, None])
    w["caBo"] = _f32((_f32(inp["ca_bo"]) + wo @ bqkv[2 * D:])[:, None])
    w["gateWT"] = _bf16(_f32(inp["gate_W"]).T)
    w["gateB"] = _f32(np.asarray(inp["gate_b"])[:, None])
    pidx = np.arange(P)
    for pre in ("mf", "mb"):
        w[pre + "inWT"] = _bf16(_f32(inp[pre + "_in_W"]).transpose(0, 2, 1))
        cw = _f32(inp[pre + "_conv_W"])          # [NL, DI, DC]
        cd = np.zeros((NL, P, DIC, DC, P), np.float32)
        cd[:, pidx, :, :, pidx] = cw.reshape(NL, DIC, P, DC).transpose(
            2, 0, 1, 3)                           # -> [P, NL, DIC, DC]
        w[pre + "convD"] = _bf16(cd)
        dv = _f32(inp[pre + "_D"])                # [NL, DI]
        dd = np.zeros((NL, P, DIC, P), np.float32)
        dd[:, pidx, :, pidx] = dv.reshape(NL, DIC, P).transpose(2, 0, 1)
        w[pre + "diagD"] = _bf16(dd)
        w[pre + "cols"] = _f32(np.stack([_f32(inp[pre + "_conv_b"]),
                                         _f32(inp[pre + "_dt_b"]),
                                         dv], -1))
        xp = _f32(inp[pre + "_xproj_W"]).transpose(0, 2, 1)   # [NL, DI, 48]
        perm = list(range(DTR, DTR + 2 * DS)) + list(range(DTR))  # [B;C;dt]
        w[pre + "xpT"] = _bf16(xp[:, :, perm])
        w[pre + "dtWT"] = _bf16(_f32(inp[pre + "_dt_W"]).transpose(0, 2, 1))
        w[pre + "outWT"] = _bf16(_f32(inp[pre + "_out_W"]).transpose(0, 2, 1))
    for pre in ("af", "ab"):
        qkv = _f32(inp[pre + "_Wqkv"])
        bqkv = _f32(inp[pre + "_bqkv"])
        wo = _f32(inp[pre + "_Wo"])
        w[pre + "WqT"] = _bf16(qkv[:, 0:D].transpose(0, 2, 1))
        w[pre + "WkT"] = _bf16(qkv[:, D:2 * D].transpose(0, 2, 1))
        w[pre + "WvT"] = _bf16(qkv[:, 2 * D:].transpose(0, 2, 1))
        w[pre + "WoT"] = _bf16(wo.transpose(0, 2, 1))
        w[pre + "Bq"] = _f32(bqkv[:, 0:D][:, :, None])
        w[pre + "Bk"] = _f32(bqkv[:, D:2 * D][:, :, None])
        w[pre + "Bo"] = _f32((_f32(inp[pre + "_bo"])
                              + np.einsum('lod,ld->lo', wo, bqkv[:, 2 * D:]))[:, :, None])
    fgrows = []
    for g, b in (("fl_ln_g", "fl_ln_b"), ("glu_ln_g", "glu_ln_b")):
        fgrows.append(np.stack([_f32(inp[g]), _f32(inp[b])], 0)[None])
    w["lnFG"] = _bf16(np.concatenate(fgrows, 0))         # [2, 2, D]
    rows = []
    for nm in ("anf", "anb", "nf", "nb"):
        rows += [_f32(inp[nm + "_g"])[:, None, :],
                 _f32(inp[nm + "_b"])[:, None, :]]
    w["lnAll"] = _bf16(np.concatenate(rows, 1))          # [NL, 8, D]
    w["glu1WT"] = _bf16(_f32(inp["glu1_W"]).T)
    w["glu1B"] = _f32(np.asarray(inp["glu1_b"])[:, None])
    w["glu2WT"] = _bf16(_f32(inp["glu2_W"]).T)
    w["glu2B"] = _f32(np.asarray(inp["glu2_b"])[:, None])
    return w


def _scan_consts(inp):
    """Per-(dir, layer, state) decay scales a_n = -exp(Alog); the model's Alog
    is d-independent, verified here; baked into the emitted program (cache-
    keyed on the values)."""
    av = {}
    for pre in ("mf", "mb"):
        al = _f32(inp[pre + "_Alog"])            # [NL, DI, DS]
        a = -np.exp(al.astype(np.float64))
        med = np.median(a, axis=1)               # [NL, DS]
        assert np.abs(a - med[:, None, :]).max() < 1e-5 * np.abs(med).max(), \
            "Alog is d-dependent; scalar-scale dA path invalid"
        av[pre] = med
    return av


# ----------------------------------------------------------------- emit helpers
class Emit:
    def __init__(self, nc, tc, ctx):
        self.nc, self.tc = nc, tc
        self.sb = ctx.enter_context(tc.tile_pool(name="sb", bufs=1))
        self.s2p = ctx.enter_context(tc.tile_pool(name="s2p", bufs=2))
        self.s3p = ctx.enter_context(tc.tile_pool(name="s3p", bufs=2))
        self.pp = ctx.enter_context(tc.tile_pool(name="pp", bufs=2, space="PSUM"))
        self.pn = ctx.enter_context(tc.tile_pool(name="pn", bufs=2, space="PSUM"))
        self.pyac = ctx.enter_context(tc.tile_pool(name="pyac", bufs=1, space="PSUM"))

    def load_wT(self, drh, K, M, tag):
        nc = self.nc
        if not isinstance(drh, bass.AP):
            drh = drh[:, :]
        kc_n = (K + P - 1) // P
        t = self.sb.tile([min(K, P), kc_n, M], BF16, tag=tag, name="wT")
        if K % P == 0:
            st = drh.ap[-1][0]
            src = bass.AP(tensor=drh.tensor, offset=drh.offset,
                          ap=[[M * st, P], [P * M * st, kc_n], [st, M]])
            nc.sync.dma_start(out=t, in_=src)
        else:
            for kc in range(kc_n):
                kp = min(P, K - kc * P)
                nc.sync.dma_start(out=t[:kp, kc, :], in_=drh[kc * P:kc * P + kp, :])
        return t

    def load_col(self, drh, M, tag):
        nc = self.nc
        if not isinstance(drh, bass.AP):
            drh = drh[:, :]
        mc_n = (M + P - 1) // P
        t = self.sb.tile([P, mc_n], F32, tag=tag, name="col")
        if M % P == 0:
            src = bass.AP(tensor=drh.tensor, offset=drh.offset,
                          ap=[[1, P], [P, mc_n]])
            nc.sync.dma_start(out=t, in_=src)
        else:
            for mc in range(mc_n):
                mp = min(P, M - mc * P)
                nc.sync.dma_start(out=t[:mp, mc:mc + 1],
                                  in_=drh[mc * P:mc * P + mp, :])
        return t

    def dense(self, x, wT, Mout, bias=None, act=None, out=None, out_pool=None,
              out_tag=None, Fw=None, out_dt=BF16):
        nc = self.nc
        Fw = Fw or F
        kc_n = x.shape[1]
        mc_n = (Mout + P - 1) // P
        if out is None:
            out = (out_pool or self.s3p).tile([P, mc_n, Fw], out_dt,
                                              tag=out_tag, name="dn")
        for mc in range(mc_n):
            mp = min(P, Mout - mc * P)
            ps = self.pp.tile([P, 512], F32, tag="mm", name="ps")
            for kc in range(kc_n):
                nc.tensor.matmul(ps[:mp, :Fw],
                                 wT[:, kc, mc * P:mc * P + mp],
                                 x[:, kc, 0:Fw],
                                 start=(kc == 0), stop=(kc == kc_n - 1))
            bap = bias[:mp, mc:mc + 1] if bias is not None else None
            if act is None and bias is None:
                nc.scalar.copy(out[:mp, mc, 0:Fw], ps[:mp, :Fw])
            else:
                nc.scalar.activation(out[:mp, mc, 0:Fw], ps[:mp, :Fw],
                                     act or AF.Identity,
                                     bias=bap if bap is not None else 0.0,
                                     scale=1.0)
        return out

    def add(self, out, a, b):
        self.nc.vector.tensor_add(out, a, b)

    def mul(self, out, a, b):
        self.nc.vector.tensor_mul(out, a, b)

    def act(self, out, in_, func, bias=0.0, scale=1.0):
        self.nc.scalar.activation(out=out, in_=in_, func=func, bias=bias, scale=scale)


def rev_view(ap2, n_blk, blk):
    st = ap2.ap[-1][0]
    off = ap2.offset + (blk - 1) * st
    if n_blk == 1:
        return bass.AP(tensor=ap2.tensor, offset=off, ap=[ap2.ap[0], [-st, blk]])
    return bass.AP(tensor=ap2.tensor, offset=off,
                   ap=[ap2.ap[0], [blk * st, n_blk], [-st, blk]])


def _g_layer_norm(E, x, gR, bR, eps, out, x_is_f32=False, tag=""):
    """x, out: [128, 2, F] feature-major (D=256 on partitions). gR/bR bf16
    rows [1, D].  Generator: yields at chunk boundaries."""
    nc = E.nc
    fw = F
    stat = E.sb.tile([1, 2, 512], F32, tag="lnstat" + tag, name="stat")
    A = stat[0:1, 0, :fw]          # m, later m*r
    Bv = stat[0:1, 1, :fw]         # q, later var, later r
    mrb = E.sb.tile([1, 2, 512], BF16, tag="lnthinb" + tag, name="mrb")
    xsq = E.sb.tile([P, 2, 512], BF16, tag="xsq", name="xsq")
    E.act(xsq, x, AF.Square)
    ones = E.ones128f if x_is_f32 else E.ones128
    for which, dst in ((0, A), (1, Bv)):
        ps = E.pn.tile([P, 512], F32, tag="th", name="ps")
        for kc in range(2):
            if which == 0:
                nc.tensor.matmul(ps[0:1, :fw], ones, x[:, kc, 0:fw],
                                 start=(kc == 0), stop=(kc == 1))
            else:
                nc.tensor.matmul(ps[0:1, :fw], E.ones128, xsq[:, kc, 0:fw],
                                 start=(kc == 0), stop=(kc == 1))
        nc.vector.tensor_scalar_mul(dst, ps[0:1, :fw], 1.0 / D)
        yield
    E.act(mrb[0:1, 0, :fw], A, AF.Square)          # m^2 (bf16 scratch)
    nc.vector.tensor_tensor(Bv, Bv, mrb[0:1, 0, :fw], OP.subtract)
    E.act(Bv, Bv, AF.Ln, bias=E.eps[eps][0:1, 0:1])
    E.act(Bv, Bv, AF.Exp, scale=-0.5)              # r (f32)
    nc.vector.tensor_copy(mrb[0:1, 0, :fw], Bv)    # r (bf16)
    E.mul(A, A, Bv)                                # m*r (f32, in place)
    E.act(mrb[0:1, 1, :fw], A, AF.Identity, scale=-1.0)   # -m*r (bf16)
    yield
    for mc in range(2):
        gRc = gR[0:1, mc * P:(mc + 1) * P]
        bRc = bR[0:1, mc * P:(mc + 1) * P]
        ps_s = E.pn.tile([P, 512], F32, tag="th", name="ps_s")
        nc.tensor.matmul(ps_s[:, :fw], gRc, mrb[0:1, 0, :fw], start=True, stop=True)
        ps_o = E.pn.tile([P, 512], F32, tag="th", name="ps_o")
        nc.tensor.matmul(ps_o[:, :fw], bRc, E.onesF[0:1, :fw],
                         start=True, stop=False)
        nc.tensor.matmul(ps_o[:, :fw], gRc, mrb[0:1, 1, :fw], start=False, stop=True)
        tmp = E.s2p.tile([P, 512], BF16, tag="lntmp", name="tmp", bufs=1)
        E.mul(tmp[:, :fw], x[:, mc, 0:fw], ps_s[:, :fw])
        E.add(out[:, mc, 0:fw], tmp[:, :fw], ps_o[:, :fw])
        yield


def _g_attention(E, q_src, kv_src, wq, wk, wv, wo, bq, bk, bo, out_tag, ob, okey):
    """MHA over PB samples; q_src/kv_src [128, 2, F] fm bf16.  Generator;
    result tile into ob[okey]."""
    nc = E.nc
    ofm = E.s3p.tile([P, 2, F], BF16, tag="t8", name="ofm")
    se = E.sb.tile([1, H, PB, S], BF16, tag="thin8", name="se")
    qf = E.s2p.tile([P, 2, F], BF16, tag="qfb", name="qf", bufs=1)
    kf = E.s2p.tile([P, 2, F], BF16, tag="kfb", name="kf", bufs=1)
    for mc in range(2):
        for dst, wT, bias, srcT in ((qf, wq, bq, q_src), (kf, wk, bk, kv_src)):
            ps = E.pp.tile([P, 512], F32, tag="mm", name="ps")
            for kc in range(2):
                nc.tensor.matmul(ps[:, :F], wT[:, kc, mc * P:(mc + 1) * P],
                                 srcT[:, kc, :], start=(kc == 0), stop=(kc == 1))
            nc.scalar.activation(dst[:, mc, :], ps[:, :F], AF.Identity,
                                 bias=bias[:, mc:mc + 1], scale=1.0)
            yield
    for b in range(PB):
        vtm = E.s2p.tile([P, 2, D], BF16, tag="vtmb", name="vtm", bufs=1)
        ps = E.pp.tile([P, 512], F32, tag="mm", name="ps")
        for tcn in range(2):
            for kc in range(2):
                nc.tensor.matmul(ps[:, tcn * D:(tcn + 1) * D],
                                 kv_src[:, kc, b * S + tcn * P: b * S + (tcn + 1) * P],
                                 wv[:, kc, :], start=(kc == 0), stop=(kc == 1))
        nc.scalar.copy(vtm[:, :, :].rearrange("p a d -> p (a d)"), ps[:, :])
        yield
        pse = None
        for h in range(H):
            hc, off = h // 2, (h % 2) * 64
            expT = E.s2p.tile([P, 2, S], BF16, tag="expT", name="expT", bufs=1)
            ps = E.pp.tile([P, 512], F32, tag="mm", name="ps")
            for kc in range(2):
                nc.tensor.matmul(ps[:, kc * S:(kc + 1) * S],
                                 kf[off:off + 64, hc, b * S + kc * P:b * S + (kc + 1) * P],
                                 qf[off:off + 64, hc, b * S:(b + 1) * S],
                                 start=True, stop=True)
            E.act(expT, ps, AF.Exp, scale=1.0 / np.sqrt(HD))
            if h % 2 == 0:
                pse = E.pn.tile([P, 512], F32, tag="th", name="pse")
            for kc in range(2):
                nc.tensor.matmul(pse[0:1, (h % 2) * S:(h % 2) * S + S],
                                 E.ones128, expT[:, kc, :],
                                 start=(kc == 0), stop=(kc == 1))
            if h % 2 == 1:
                E.act(se[0:1, h - 1:h + 1, b, :],
                      pse[0:1, :].rearrange("p (h s) -> p h s", h=2), AF.Ln)
            if h % 2 == 0:
                psav = E.pp.tile([P, 512], F32, tag="mm", name="psav")
            for kc in range(2):
                nc.tensor.matmul(psav[off:off + 64, :S],
                                 vtm[:, kc, h * 64:(h + 1) * 64],
                                 expT[:, kc, :], start=(kc == 0), stop=(kc == 1))
            if h % 2 == 1:
                nc.scalar.copy(ofm[:, hc, b * S:(b + 1) * S], psav[:, :S])
            yield
    E.act(se, se, AF.Exp, scale=-1.0)              # 1/sumexp, in place
    yield
    for h in range(H):
        dc, off = h // 2, (h % 2) * 64
        ps = E.pn.tile([P, 512], F32, tag="th", name="ps")
        nc.tensor.matmul(ps[0:64, :F], E.ones1x64,
                         se[0:1, h].rearrange("p b s -> p (b s)"),
                         start=True, stop=True)
        E.mul(ofm[off:off + 64, dc, :], ofm[off:off + 64, dc, :], ps[0:64, :F])
        yield
    ob[okey] = E.dense(ofm, wo, D, bias=bo, out_tag=out_tag)


def _g_mamba_prep_a(E, io, x, pre, l, flip, pr):
    """Silu-table phase: weights, in-proj xi, conv via host diag mats, z."""
    nc = E.nc
    d = pre
    inW = E.load_wT(io[pre + "inWT"][l], D, 2 * DI, "inW")
    cols = E.sb.tile([P, DIC, 3], F32, tag="mcols" + d, name="cols")
    cd = io[pre + "cols"][l]
    nc.sync.dma_start(out=cols, in_=bass.AP(
        tensor=cd.tensor, offset=cd.offset, ap=[[3, P], [P * 3, DIC], [1, 3]]))
    convD = E.sb.tile([P, DIC, DC, P], BF16, tag="convD", name="convD")
    nc.sync.dma_start(out=convD, in_=io[pre + "convD"][l])
    diagD = E.sb.tile([P, DIC, P], BF16, tag="diagD" + d, name="diagD")
    nc.sync.dma_start(out=diagD, in_=io[pre + "diagD"][l])
    xpw = E.load_wT(io[pre + "xpT"][l], DI, DTR + 2 * DS, "xpw" + d)
    dtw = E.sb.tile([2 * DS + DTR, DI], BF16, tag="dtw" + d, name="dtw")
    nc.sync.dma_start(out=dtw[2 * DS:, :], in_=io[pre + "dtWT"][l])
    ow = E.load_wT(io[pre + "outWT"][l], DI, D, "outW" + d)
    yield

    def inproj(c0, dst_tag, silu):
        dst = E.sb.tile([P, DIC, F], BF16, tag=dst_tag, name="xi")
        for c in range(DIC):
            ps = E.pp.tile([P, 512], F32, tag="mm", name="ps")
            for b in range(PB):
                for kc in range(2):
                    rhs = x[:, kc, b * S:(b + 1) * S]
                    if flip:
                        rhs = rev_view(rhs, 1, S)
                    nc.tensor.matmul(ps[:, b * S:(b + 1) * S],
                                     inW[:, kc, (c0 + c) * P:(c0 + c + 1) * P], rhs,
                                     start=(kc == 0), stop=(kc == 1))
            if silu:
                E.act(dst[:, c, :], ps, AF.Silu)
            else:
                nc.scalar.copy(dst[:, c, :], ps)
        return dst

    xi = inproj(0, "xi", False)
    yield
    xc = E.s2p.tile([P, DIC, F], BF16, tag="xc", name="xc")
    for c in range(DIC):
        ps = E.pp.tile([P, 512], F32, tag="mm", name="ps")
        for b in range(PB):
            nc.tensor.matmul(ps[:, b * S:(b + 1) * S], convD[:, c, DC - 1, :],
                             xi[:, c, b * S:(b + 1) * S], start=True, stop=False)
            for j in range(DC - 1):
                sh = DC - 1 - j
                nc.tensor.matmul(ps[:, b * S + sh:(b + 1) * S], convD[:, c, j, :],
                                 xi[:, c, b * S:(b + 1) * S - sh],
                                 start=False, stop=(j == DC - 2))
        E.act(xc[:, c, :], ps, AF.Silu, bias=cols[:, c, 0:1])
        yield
    z = inproj(DIC, "z" + d, True)
    yield
    pr.update(xc=xc, z=z, diagD=diagD, ow=ow, xpw=xpw, dtw=dtw, cols=cols)


def _g_mamba_prep_b(E, io, pr, pre, l, bcd):
    """NLE-table phase: xproj -> dbl (+DRAM bounce), dt softplus, dtu."""
    nc = E.nc
    d = pre
    xc, xpw, dtw, cols = pr["xc"], pr["xpw"], pr["dtw"], pr["cols"]
    dbl = E.sb.tile([DTR + 2 * DS, F], BF16, tag="dbl" + d, name="dbl")
    ps = E.pp.tile([P, 512], F32, tag="mm", name="ps")
    for kc in range(DIC):
        nc.tensor.matmul(ps[:DTR + 2 * DS, :F], xpw[:, kc, :], xc[:, kc, :],
                         start=(kc == 0), stop=(kc == DIC - 1))
    nc.scalar.copy(dbl, ps[:DTR + 2 * DS, :F])
    nc.sync.dma_start(out=bcd[:, :], in_=dbl[0:2 * DS, :])
    yield
    dt = E.sb.tile([P, DIC, F], BF16, tag="dt" + d, name="dt")
    for mc in range(DIC):
        ps = E.pp.tile([P, 512], F32, tag="mm", name="ps")
        nc.tensor.matmul(ps[:, :F], dtw[2 * DS:, mc * P:(mc + 1) * P],
                         dbl[2 * DS:2 * DS + DTR, :], start=True, stop=True)
        dtx = E.sb.tile([P, F], BF16, tag="dtx", name="dtx")
        E.act(dtx, ps[:, :F], AF.Exp, bias=cols[:, mc, 1:2])
        E.act(dt[:, mc, :], dtx, AF.Ln, bias=1.0)
        yield
    dtu = E.sb.tile([P, DIC, F], BF16, tag="dtu" + d, name="dtu")
    E.mul(dtu, dt, xc)
    nc.vector.memset(dt[:, :, 0:F:S], 1.0e30)
    pr.update(dt=dt, dtu=dtu, bcd=bcd)


def _mamba_scan(E, pr, avl, out_tag, bg, pump):
    """n-pair scan loop: dA on ACT (literal scalar scale), dBu / h*C muls on
    DVE (bf16), scans on the Pool engine, Sum_n C*h_n (+ D*u) accumulated in
    PSUM by PE identity matmuls; y*silu(z) reads PSUM directly on DVE."""
    nc = E.nc
    dt, dtu, z = pr["dt"], pr["dtu"], pr["z"]
    yac = [E.pyac.tile([P, 512], F32, tag=f"yac{c}", name="yac") for c in range(DIC)]
    for c in range(DIC):
        nc.tensor.matmul(yac[c], pr["diagD"][:, c, :], pr["xc"][:, c, :],
                         start=True, stop=False, skip_group_check=True)
    dt2 = dt[:, :, :].rearrange("p c f -> p (c f)")
    bcd = pr["bcd"][:, :]
    for n in range(DS):
        bg.pump(pump)
        dA = E.s2p.tile([P, DIC * F], BF16, tag="dA", name="dA")
        E.act(dA, dt2, AF.Exp, scale=float(avl[n]))
        bc = E.s2p.tile([P, 2, F], BF16, tag="bc", name="bc", bufs=3)
        src = bass.AP(tensor=bcd.tensor, offset=bcd.offset + n * F,
                      ap=[[0, P], [DS * F, 2], [1, F]])
        nc.sync.dma_start(out=bc, in_=src)
        dBu = E.s2p.tile([P, DIC, F], BF16, tag="dBu", name="dBu", bufs=3)
        bview = bass.AP(tensor=bc.tensor, offset=bc.offset,
                        ap=[bc.ap[0], [0, DIC], [1, F]])
        # the dtu*B broadcast mul rides the otherwise-idle Pool engine for
        # most states (Multiply has a real gpsimd impl; the scan does not)
        meng = nc.vector if n in (0, 5, 10, 15) else nc.gpsimd
        meng.tensor_mul(dBu, dtu, bview)
        hn = E.s2p.tile([P, DIC, F], BF16, tag="hn", name="hn", bufs=3)
        nc.vector.tensor_tensor_scan(
            out=hn[:, :, :].rearrange("p c f -> p (c f)"),
            data0=dA[:, :],
            data1=dBu[:, :, :].rearrange("p c f -> p (c f)"),
            initial=0.0, op0=OP.mult, op1=OP.add)
        hnC = E.sb.tile([P, DIC, F], BF16, tag="hnC", name="hnC")
        cview = bass.AP(tensor=bc.tensor, offset=bc.offset + F,
                        ap=[bc.ap[0], [0, DIC], [1, F]])
        E.mul(hnC, hn, cview)
        last = (n == DS - 1)
        for c in range(DIC):
            nc.tensor.matmul(yac[c], E.identb, hnC[:, c, :],
                             start=False, stop=last,
                             skip_group_check=True)
    bg.pump(2)
    y = dtu            # dtu is dead after the last dBu; reuse its buffer
    for c in range(DIC):
        E.mul(y[:, c, :], z[:, c, :], yac[c])
    return E.dense(y, pr["ow"], D, out_pool=E.s2p, out_tag=out_tag)


# ------------------------------------------------------------------- program
def build_program(wshapes, av):
    nc = _Bacc()
    io = {}
    io["input"] = nc.declare_dram_parameter("input", [BC, S, D], F32, isOutput=False)
    for k, shp, dt in wshapes:
        io[k] = nc.declare_dram_parameter(k, list(shp), dt, isOutput=False)
    io["out"] = nc.declare_dram_parameter("out", [BC, S, D], F32, isOutput=True)
    for pss in range(NPASS):
        for l in range(NL):
            for pre in ("mf", "mb"):
                io[f"bcrows_{pss}_{l}_{pre}"] = nc.dram_tensor(
                    f"bcrows_{pss}_{l}_{pre}", [2 * DS, F], BF16)
    with tile.TileContext(nc) as tc:
        with ExitStack() as ctx:
            E = Emit(nc, tc, ctx)
            identb = E.sb.tile([P, P], BF16, tag="identb", name="identb")
            make_identity(nc, identb)
            E.identb = identb
            identf = E.sb.tile([P, P], F32, tag="identf", name="identf")
            make_identity(nc, identf)
            E.identf = identf
            E.ones128 = E.sb.tile([P, 1], BF16, tag="ones128", name="ones128")
            nc.vector.memset(E.ones128, 1.0)
            E.ones128f = E.sb.tile([P, 1], F32, tag="ones128f", name="ones128f")
            nc.vector.memset(E.ones128f, 1.0)
            E.ones1x64 = E.sb.tile([1, 64], BF16, tag="ones64", name="ones64")
            nc.vector.memset(E.ones1x64, 1.0)
            E.ones1xP = E.sb.tile([1, P], BF16, tag="ones1p", name="ones1p")
            nc.vector.memset(E.ones1xP, 1.0)
            E.onesF = E.sb.tile([1, 512], BF16, tag="onesF", name="onesF")
            nc.vector.memset(E.onesF, 1.0)
            E.eps = {}
            for ev in (1e-5, 1e-12):
                t = E.sb.tile([1, 1], F32, tag=f"eps{ev}", name="eps")
                nc.vector.memset(t, ev)
                E.eps[ev] = t
            # software-pipelined pass interleave: pass-1's FFT/wavelet/gate
            # stage and layer preps are emitted inside pass-0's scan windows
            # so the Pool engine (scans) never drains.
            bg = _BG()
            box = {}
            c00, c10, c01, c11 = {}, {}, {}, {}
            _run(_g_stage03(E, io, 0, box, "x1a"))
            _run(_g_layer_preps(E, io, 0, 0, lambda: box["x1a"], av, c00))
            bg.add(_chain(
                _g_stage03(E, io, 1, box, "x1b"),
                _g_layer_preps(E, io, 1, 0, lambda: box["x1b"], av, c10)))
            _emit_layer_scans(E, c00, av, bg)
            bg.drain()
            bg.add(_chain(
                _g_layer_post(E, c00, box, "x1a"),
                _g_layer_preps(E, io, 0, 1, lambda: box["x1a"], av, c01)))
            _emit_layer_scans(E, c10, av, bg)
            bg.drain()
            bg.add(_chain(
                _g_layer_post(E, c10, box, "x1b"),
                _g_layer_preps(E, io, 1, 1, lambda: box["x1b"], av, c11)))
            _emit_layer_scans(E, c01, av, bg)
            bg.drain()
            bg.add(_chain(
                _g_layer_post(E, c01, box, "x1a"),
                _g_glu(E, io, 0, lambda: box["x1a"])))
            _emit_layer_scans(E, c11, av, bg,
                              mid_add=_chain(_g_post_attn(E, c11, "mf", "af"),
                                             _g_post_lnt(E, c11)))
            bg.drain()
            _run(_g_layer_post(E, c11, box, "x1b"))
            _run(_g_glu(E, io, 1, lambda: box["x1b"]))
    nc.finalize()
    return nc


class _BG:
    def __init__(self):
        from collections import deque
        self.q = deque()

    def add(self, gen):
        self.q.append(gen)

    def pump(self, n=1):
        while n > 0 and self.q:
            try:
                next(self.q[0])
                n -= 1
            except StopIteration:
                self.q.popleft()

    def drain(self):
        while self.q:
            self.pump(64)


def _run(gen):
    for _ in gen:
        pass


def _chain(*gens):
    for g in gens:
        yield from g


def _g_stage03(E, io, pss, box, key):
    nc = E.nc
    # ---------------- stage 0: load x + cast + transpose to feature-major
    x_tm = E.sb.tile([P, PB * 2, D], BF16, tag="xtm", name="x_tm")
    for b in range(PB):
        for sc in range(2):
            xch = E.s2p.tile([P, D], F32, tag="xt32", name="xch")
            nc.sync.dma_start(out=xch,
                              in_=io["input"][pss * PB + b, sc * P:(sc + 1) * P, :])
            nc.vector.tensor_copy(x_tm[:, b * 2 + sc, :], xch)
    yield
    x_fm = E.sb.tile([P, 2, F], BF16, tag="xfm", name="x_fm")
    for b in range(PB):
        for sc in range(2):
            for dc in range(2):
                pst = E.pn.tile([P, P], BF16, tag="th", name="pst")
                nc.tensor.transpose(pst, x_tm[:, b * 2 + sc, dc * P:(dc + 1) * P],
                                    E.identb)
                nc.scalar.copy(x_fm[:, dc, b * S + sc * P: b * S + (sc + 1) * P], pst)
            yield

    # ---------------- stage 1: FFT path
    frT = E.load_wT(io["frT"], S, NF, "frT")
    fiT = E.load_wT(io["fiT"], S, NF, "fiT")
    fftWa = E.load_wT(io["fftWa"], 513, 2 * D, "fftWa")
    grT = E.load_wT(io["grT"], NF, S, "grT")
    giT = E.load_wT(io["giT"], NF, S, "giT")
    yield
    x_fft = E.sb.tile([P, 2, F], BF16, tag="xfft", name="x_fft")
    for b in range(PB):
        comb = E.s3p.tile([P, 4, NF], BF16, tag="t8", name="comb")
        for ri, mat in ((0, frT), (1, fiT)):
            for mc in range(2):
                ps = E.pp.tile([P, 512], F32, tag="mm", name="ps")
                for kc in range(2):
                    nc.tensor.matmul(ps[:, :NF], x_tm[:, b * 2 + kc, mc * P:(mc + 1) * P],
                                     mat[:, kc, :], start=(kc == 0), stop=(kc == 1))
                nc.scalar.copy(comb[:, ri * 2 + mc, :], ps[:, :NF])
                yield
        filt = E.s3p.tile([P, 2 * D], BF16, tag="t8", name="filt")
        filtN = E.sb.tile([1, 2 * D], BF16, tag="filtN", name="filtN")
        for mt, mp, f0 in ((filt, P, 0), (filtN, 1, P)):
            ps = E.pp.tile([P, 512], F32, tag="mm", name="ps")
            for kc in range(4):
                nc.tensor.matmul(ps[:mp, :], comb[:, kc, f0:f0 + mp], fftWa[:, kc, :],
                                 start=(kc == 0), stop=False)
            nc.tensor.matmul(ps[:mp, :], E.ones1xP[0:1, 0:mp], fftWa[0:1, 4, :],
                             start=False, stop=True)
            E.act(mt[0:mp, :] if mt is filtN else mt, ps[:mp, :], AF.Gelu)
            yield
        for mc in range(2):
            ps = E.pp.tile([P, 512], F32, tag="mm", name="ps")
            nc.tensor.matmul(ps[:, :S], filt[:, mc * P:(mc + 1) * P], grT[:, 0, :],
                             start=True, stop=False)
            nc.tensor.matmul(ps[:, :S], filtN[0:1, mc * P:(mc + 1) * P], grT[0:1, 1, :],
                             start=False, stop=False)
            nc.tensor.matmul(ps[:, :S], filt[:, D + mc * P:D + (mc + 1) * P], giT[:, 0, :],
                             start=False, stop=False)
            nc.tensor.matmul(ps[:, :S], filtN[0:1, D + mc * P:D + (mc + 1) * P],
                             giT[0:1, 1, :], start=False, stop=True)
            nc.scalar.copy(x_fft[:, mc, b * S:(b + 1) * S], ps[:, :S])
            yield

    # ---------------- stage 2: wavelet path
    tdT = E.load_wT(io["tdT"], S, L2, "tdT")
    iiT = E.sb.tile([L2, S], BF16, tag="iiT", name="iiT")
    nc.sync.dma_start(out=iiT, in_=io["iiT"][:, :])
    wl1T = [E.load_wT(io["wl1T"][k], D, D, t) for k, t in enumerate(("awq", "awk", "awv"))]
    wl2T = [E.load_wT(io["wl2T"][k], D, D, t) for k, t in enumerate(("awo", "wlo1", "wlo2"))]
    wl1b = E.load_col(io["wl1b"], D, "wl1b")
    wl2b = E.load_col(io["wl2b"], D, "wl2b")
    yield
    x_wl = E.sb.tile([P, 2, F], BF16, tag="xwl", name="x_wl")
    a_fm = E.sb.tile([P, 2, PB, L2], BF16, tag="afm", name="a_fm")
    for b in range(PB):
        for mc in range(2):
            ps = E.pp.tile([P, 512], F32, tag="mm", name="ps")
            for kc in range(2):
                nc.tensor.matmul(ps[:, :L2], x_tm[:, b * 2 + kc, mc * P:(mc + 1) * P],
                                 tdT[:, kc, :], start=(kc == 0), stop=(kc == 1))
            nc.scalar.copy(a_fm[:, mc, b, :], ps[:, :L2])
    yield

    def conv3(src, wT, bcol, actf, dst_tag):
        dst = E.s2p.tile([P, 2, PB, L2], BF16, tag=dst_tag, name="c3")
        for b in range(PB):
            for mc in range(2):
                ps = E.pp.tile([P, 512], F32, tag="mm", name="ps")
                for kc in range(2):
                    nc.tensor.matmul(ps[:, :L2], wT[1][:, kc, mc * P:(mc + 1) * P],
                                     src[:, kc, b, :], start=(kc == 0), stop=False)
                for kc in range(2):
                    nc.tensor.matmul(ps[:, 1:L2], wT[0][:, kc, mc * P:(mc + 1) * P],
                                     src[:, kc, b, 0:L2 - 1], start=False, stop=False)
                for kc in range(2):
                    nc.tensor.matmul(ps[:, 0:L2 - 1], wT[2][:, kc, mc * P:(mc + 1) * P],
                                     src[:, kc, b, 1:L2], start=False, stop=(kc == 1))
                E.act(dst[:, mc, b, :], ps[:, :L2], actf, bias=bcol[:, mc:mc + 1])
        return dst

    c1 = conv3(a_fm, wl1T, wl1b, AF.Gelu, "c1")  # s2p ring
    yield
    c2 = conv3(c1, wl2T, wl2b, AF.Identity, "afm")
    yield
    c2T = E.sb.tile([L2, 2, PB, P], BF16, tag="c2T", name="c2T")
    for b in range(PB):
        for mc in range(2):
            pst = E.pn.tile([P, P], BF16, tag="th", name="pst")
            nc.tensor.transpose(pst[0:L2, :], c2[:, mc, b, :], E.identb)
            nc.scalar.copy(c2T[:, mc, b, :], pst[0:L2, :])
    yield
    for b in range(PB):
        for mc in range(2):
            ps = E.pp.tile([P, 512], F32, tag="mm", name="ps")
            nc.tensor.matmul(ps[:, :S], c2T[:, mc, b, :], iiT, start=True, stop=True)
            nc.scalar.copy(x_wl[:, mc, b * S:(b + 1) * S], ps[:, :S])
    yield

    # ---------------- stage 3: cross-attention + gate + LN
    caWq = E.load_wT(io["caWqT"], D, D, "awq")
    caWk = E.load_wT(io["caWkT"], D, D, "awk")
    caWv = E.load_wT(io["caWvT"], D, D, "awv")
    caWo = E.load_wT(io["caWoT"], D, D, "awo")
    caBq = E.load_col(io["caBq"], D, "abq")
    caBk = E.load_col(io["caBk"], D, "abk")
    caBo = E.load_col(io["caBo"], D, "abo")
    ab = {}
    yield from _g_attention(E, x_fft, x_wl, caWq, caWk, caWv, caWo, caBq, caBk,
                            caBo, "t8", ab, "att")
    fused = E.s3p.tile([P, 2, F], BF16, tag="t8", name="fused")
    E.add(fused, ab["att"], x_fm)
    gateW = E.load_wT(io["gateWT"], 2 * D, 2 * D, "gateW")
    gateB = E.load_col(io["gateB"], 2 * D, "gateB")
    ga = E.s3p.tile([P, 2, F], BF16, tag="t8", name="ga")
    gb = E.s3p.tile([P, 2, F], BF16, tag="t8", name="gb")
    for mc in range(4):
        actf = AF.Identity if mc < 2 else AF.Sigmoid
        gdst = ga if mc < 2 else gb
        ps = E.pp.tile([P, 512], F32, tag="mm", name="ps")
        for kc in range(4):
            gsrc = fused if kc < 2 else x_fm
            nc.tensor.matmul(ps[:, :F], gateW[:, kc, mc * P:(mc + 1) * P],
                             gsrc[:, kc % 2, :], start=(kc == 0), stop=(kc == 3))
        E.act(gdst[:, mc % 2, :], ps[:, :F], actf, bias=gateB[:, mc:mc + 1])
        yield
    gated = ga
    E.mul(gated, ga, gb)
    flt = E.s2p.tile([1, 2, D], BF16, tag="lnFG", name="flt")
    nc.sync.dma_start(out=flt, in_=io["lnFG"][0])
    x1 = E.s2p.tile([P, 2, F], BF16, tag="x1", name="x1", bufs=3)
    yield from _g_layer_norm(E, gated, flt[0:1, 0, :], flt[0:1, 1, :], 1e-5, x1)
    box[key] = x1


_DIRS = (("mf", "af", False, "anf", "nf"),
         ("mb", "ab", True, "anb", "nb"))


def _g_layer_preps(E, io, pss, l, x1f, av, cd):
    x1 = x1f()
    prs = {}
    for (mp, ap_, flip, anG, nG) in _DIRS:
        prs[mp] = {}
        yield from _g_mamba_prep_a(E, io, x1, mp, l, flip, prs[mp])
    for (mp, ap_, flip, anG, nG) in _DIRS:
        bcd = io[f"bcrows_{pss}_{l}_{mp}"]
        yield from _g_mamba_prep_b(E, io, prs[mp], mp, l, bcd)
    cd.update(prs=prs, x1=x1, l=l, io=io, pss=pss)


def _emit_layer_scans(E, cd, av, bg, mid_add=None):
    cd["ms"] = {}
    for di, (mp, ap_, flip, anG, nG) in enumerate(_DIRS):
        cd["ms"][mp] = _mamba_scan(E, cd["prs"][mp], av[mp][cd["l"]],
                                   "ms" + mp, bg, pump=3 if di == 0 else 5)
        if di == 0 and mid_add is not None:
            bg.add(mid_add)


def _g_post_attn(E, cd, mp, ap_):
    nc = E.nc
    io, l = cd["io"], cd["l"]
    ab = {}
    wq = E.load_wT(io[ap_ + "WqT"][l], D, D, "awq" + mp)
    wk = E.load_wT(io[ap_ + "WkT"][l], D, D, "awk" + mp)
    wv = E.load_wT(io[ap_ + "WvT"][l], D, D, "awv" + mp)
    wo = E.load_wT(io[ap_ + "WoT"][l], D, D, "awo" + mp)
    abq = E.load_col(io[ap_ + "Bq"][l], D, "abq" + mp)
    abk = E.load_col(io[ap_ + "Bk"][l], D, "abk" + mp)
    abo = E.load_col(io[ap_ + "Bo"][l], D, "abo" + mp)
    ms = cd["ms"][mp]
    yield from _g_attention(E, ms, ms, wq, wk, wv, wo, abq, abk, abo,
                            "t8", ab, "att")
    E.add(ms, ms, ab["att"])
    cd.setdefault("s2d", {})[mp] = ms
    yield


def _g_post_lnt(E, cd):
    nc = E.nc
    if "lnt" in cd:
        return
    lnt = E.s2p.tile([1, 8, D], BF16, tag="lnAll", name="lnt")
    nc.sync.dma_start(out=lnt, in_=cd["io"]["lnAll"][cd["l"]])
    cd["lnt"] = lnt
    yield


def _ln_params(cd, name):
    nidx = {"anf": 0, "anb": 1, "nf": 2, "nb": 3}
    i = nidx[name] * 2
    lnt = cd["lnt"]
    return (lnt[0:1, i, :], lnt[0:1, i + 1, :])


def _g_post_lns(E, cd, mp, flip, anG, nG):
    x1 = cd["x1"]
    s3 = E.s3p.tile([P, 2, F], BF16, tag="t8", name="s3")
    (ang, anb_) = _ln_params(cd, anG)
    yield from _g_layer_norm(E, cd["s2d"][mp], ang, anb_, 1e-5, s3)
    s4 = E.s3p.tile([P, 2, F], BF16, tag="t8", name="s4")
    if flip:
        for kc in range(2):
            E.add(s4[:, kc, :].rearrange("p (b s) -> p b s", b=PB),
                  rev_view(s3[:, kc, :], PB, S),
                  x1[:, kc, :].rearrange("p (b s) -> p b s", b=PB))
    else:
        E.add(s4, s3, x1)
    yield
    s5 = E.s2p.tile([P, 2, F], BF16, tag="s5", name="s5")
    (ng, nb_) = _ln_params(cd, nG)
    yield from _g_layer_norm(E, s4, ng, nb_, 1e-5, s5)
    cd.setdefault("s5d", {})[mp] = s5


def _g_layer_post(E, cd, box, key):
    done = cd.get("s2d", {})
    for (mp, ap_, flip, anG, nG) in _DIRS:
        if mp not in done:
            yield from _g_post_attn(E, cd, mp, ap_)
    yield from _g_post_lnt(E, cd)
    s5d = cd.get("s5d", {})
    for (mp, ap_, flip, anG, nG) in _DIRS:
        if mp not in s5d:
            yield from _g_post_lns(E, cd, mp, flip, anG, nG)
    x1n = E.s2p.tile([P, 2, F], BF16, tag="x1", name="x1n", bufs=3)
    E.add(x1n, cd["s5d"]["mf"], cd["s5d"]["mb"])
    box[key] = x1n


def _g_glu(E, io, pss, x1f):
    nc = E.nc
    x1 = x1f()
    # ---------------- stage 5: GLU + final LN
    glu1W = E.load_wT(io["glu1WT"], D, 2 * D, "glu1W")
    glu1B = E.load_col(io["glu1B"], 2 * D, "glu1B")
    va = E.s3p.tile([P, 2, F], BF16, tag="t8", name="va")
    vb = E.s3p.tile([P, 2, F], BF16, tag="t8", name="vb")
    for mc in range(4):
        actf = AF.Identity if mc < 2 else AF.Sigmoid
        vdst = va if mc < 2 else vb
        ps = E.pp.tile([P, 512], F32, tag="mm", name="ps")
        for kc in range(2):
            nc.tensor.matmul(ps[:, :F], glu1W[:, kc, mc * P:(mc + 1) * P],
                             x1[:, kc, :], start=(kc == 0), stop=(kc == 1))
        E.act(vdst[:, mc % 2, :], ps[:, :F], actf, bias=glu1B[:, mc:mc + 1])
        yield
    gv = va
    E.mul(gv, va, vb)
    glu2W = E.load_wT(io["glu2WT"], D, D, "glu2W")
    glu2B = E.load_col(io["glu2B"], D, "glu2B")
    gvo = E.dense(gv, glu2W, D, bias=glu2B, out_tag="t8")
    yield
    res = E.sb.tile([P, 2, F], F32, tag="res", name="res")
    E.add(res, gvo, x1)
    glt = E.s2p.tile([1, 2, D], BF16, tag="lnFG", name="glt")
    nc.sync.dma_start(out=glt, in_=io["lnFG"][1])
    out_fm = E.sb.tile([P, 2, F], F32, tag="reso", name="out_fm")
    yield from _g_layer_norm(E, res, glt[0:1, 0, :], glt[0:1, 1, :], 1e-12, out_fm,
                             x_is_f32=True)

    # ---------------- stage 6: transpose + store
    for b in range(PB):
        for sc in range(2):
            ot = E.sb.tile([P, D], F32, tag="otile", name="ot")
            for dc in range(2):
                pst = E.pn.tile([P, P], F32, tag="th", name="pst")
                nc.tensor.transpose(pst, out_fm[:, dc, b * S + sc * P: b * S + (sc + 1) * P],
                                    E.identf)
                nc.scalar.copy(ot[:, dc * P:(dc + 1) * P], pst)
            nc.sync.dma_start(out=io["out"][pss * PB + b, sc * P:(sc + 1) * P, :], in_=ot)
            yield


# ------------------------------------------------------------------- driver
_CACHE = {}


def _get_program(w, av):
    wshapes = []
    for k, v in sorted(w.items()):
        dt = BF16 if v.dtype.itemsize == 2 else F32
        wshapes.append((k, tuple(v.shape), dt))
    avh = hashlib.sha256(
        b"".join(np.ascontiguousarray(av[p]).tobytes() for p in ("mf", "mb"))
    ).hexdigest()
    key = (tuple(wshapes), avh)
    if key not in _CACHE:
        _CACHE[key] = build_program(wshapes, av)
    return _CACHE[key]


def kernel(**inputs):
    from concourse.bass_utils import run_bass_kernel_spmd
    w = _prep_weights(inputs)
    av = _scan_consts(inputs)
    nc = _get_program(w, av)
    x = np.ascontiguousarray(np.asarray(inputs["input_tensor"], np.float32))
    in_maps = []
    for core in range(NCORES):
        m = {"input": np.ascontiguousarray(x[core * BC:(core + 1) * BC])}
        m.update(w)
        in_maps.append(m)
    res = run_bass_kernel_spmd(nc, in_maps, list(range(NCORES)))
    return np.concatenate([res.results[i]["out"] for i in range(NCORES)], axis=0)


# revision 45
# speedup vs baseline: 2.1226x; 1.3531x over previous
"""Trainium2 Bass kernel for nn_BiMaTrLayer (dual-path filter + bidirectional
Mamba/attention stack + GLU).  Data-parallel over 8 NeuronCores (4 samples per
core, processed as 2 passes of 2 samples).

v2: bf16 matmul datapath (weights host-cast, activations evacuated bf16),
selective-scan restructured: tensor_tensor_scan runs on the (otherwise idle)
Pool engine in n-pairs over channel-fused [128, 2*DIC*F] tiles, DVE keeps only
the dtu*B and h*C muls, and the sum over states accumulates in PSUM via PE
identity matmuls (D*u enters via a host-precomputed diagonal matmul).
dA = exp(a_n * dt) uses a literal scalar scale per state (A is d-independent
in this model; baked values are part of the program cache key).
"""

import sys
import hashlib
import numpy as np

sys.path.append("/opt/trn_rl_repo")

import concourse.bass as bass
from concourse import bacc


class _Bacc(bacc.Bacc):
    """Bacc with act-table steering: resolve Exp and Ln to the combined
    natural_log_exp_and_others set so softplus/LN chains don't ping-pong
    table loads (2.7us each)."""

    def insert_act_table_loads(self):
        import concourse.mybir as _mb
        from concourse.hw_specs import get_activation_tables
        from concourse import bacc as _bacc
        has_activation = any(
            isinstance(i, _mb.InstActivation)
            for b in self.main_func.blocks
            for i in b.instructions
        )
        if not has_activation:
            return
        tables = list(get_activation_tables(self.m.arch).items())
        AFT = _mb.ActivationFunctionType
        steer = {"exp_and_others": {AFT.Exp}, "exp_and_friends": {AFT.Exp},
                 "natural_log": {AFT.Ln}}
        tables = [(nm, fn - steer.get(nm, set())) for nm, fn in tables]
        _bacc._bass_rust.insert_act_table_loads(self, tables)

import concourse.mybir as mybir
import concourse.tile as tile
from concourse.masks import make_identity
from contextlib import ExitStack

AF = mybir.ActivationFunctionType
OP = mybir.AluOpType
F32 = mybir.dt.float32
BF16 = mybir.dt.bfloat16
P = 128

B, S, D = 32, 256, 256
NCORES = 8
BC = B // NCORES            # samples per core
PB = 2                      # samples per pass
NPASS = BC // PB
F = PB * S                  # 512: free dim (sample, time) per pass
DI, DS, DTR, NL, H, HD = 512, 16, 16, 2, 4, 64
DIC = DI // P
L2 = 69
NF = S // 2 + 1
DC = 4

DEC_LO = np.array([-0.010597401784997278, 0.032883011666982945,
                   0.030841381835986965, -0.18703481171888114,
                   -0.02798376941698385, 0.6308807679295904,
                   0.7148465705525415, 0.23037781330885523], np.float64)


def _bf16(a):
    import ml_dtypes
    return np.ascontiguousarray(np.asarray(a, np.float32).astype(ml_dtypes.bfloat16))


def _f32(a):
    return np.ascontiguousarray(np.asarray(a), np.float32)


# ----------------------------------------------------------------- host consts
def _dwt1_mat(L):
    out_full = L + 14 - 8 + 1
    idx = np.arange(1, out_full, 2)
    M = np.zeros((len(idx), L))
    for s in range(L):
        x = np.zeros(L)
        x[s] = 1.0
        y = np.correlate(np.pad(x, 7), DEC_LO[::-1], 'valid')
        M[:, s] = y[idx]
    return M


def _interp_mat(Lin, out_len):
    pos = (np.arange(out_len) + 0.5) * (Lin / out_len) - 0.5
    pos = np.clip(pos, 0.0, Lin - 1.0)
    lo = np.floor(pos).astype(int)
    hi = np.minimum(lo + 1, Lin - 1)
    t = pos - lo
    M = np.zeros((out_len, Lin))
    M[np.arange(out_len), lo] += 1.0 - t
    M[np.arange(out_len), hi] += t
    return M


def _fft_mats():
    s = np.arange(S)
    f = np.arange(NF)
    ang = 2 * np.pi * np.outer(f, s) / S
    Fr = np.cos(ang) / np.sqrt(S)
    Fi = -np.sin(ang) / np.sqrt(S)
    c = np.full(NF, 2.0)
    c[0] = 1.0
    c[-1] = 1.0
    angT = 2 * np.pi * np.outer(s, f) / S
    Gr = c * np.cos(angT) / np.sqrt(S)
    Gi = -c * np.sin(angT) / np.sqrt(S)
    Gi[:, 0] = 0.0
    Gi[:, -1] = 0.0
    return Fr, Fi, Gr, Gi


def _host_consts():
    Fr, Fi, Gr, Gi = _fft_mats()
    D1 = _dwt1_mat(S)
    D2 = _dwt1_mat(D1.shape[0])
    T = D2 @ D1
    I = _interp_mat(T.shape[0], S)
    return dict(frT=_bf16(Fr.T), fiT=_bf16(Fi.T), grT=_bf16(Gr.T),
                giT=_bf16(Gi.T), tdT=_bf16(T.T), iiT=_bf16(I.T))


def _prep_weights(inp):
    w = dict(_host_consts())
    w["fftWa"] = _bf16(np.concatenate([_f32(inp["fft_W"]).T,
                                       _f32(inp["fft_b"])[None, :]], 0))
    for nm in ("wl1", "wl2"):
        w[nm + "T"] = _bf16(_f32(inp[nm + "_W"]).transpose(2, 1, 0))
        w[nm + "b"] = _f32(np.asarray(inp[nm + "_b"])[:, None])
    qkv = _f32(inp["ca_Wqkv"])
    bqkv = _f32(inp["ca_bqkv"])
    wo = _f32(inp["ca_Wo"])
    w["caWqT"] = _bf16(qkv[0:D].T)
    w["caWkT"] = _bf16(qkv[D:2 * D].T)
    w["caWvT"] = _bf16(qkv[2 * D:].T)
    w["caWoT"] = _bf16(wo.T)
    w["caBq"] = _f32(bqkv[0:D][:, None])
    w["caBk"] = _f32(bqkv[D:2 * D][:, None])
    w["caBo"] = _f32((_f32(inp["ca_bo"]) + wo @ bqkv[2 * D:])[:, None])
    w["gateWT"] = _bf16(_f32(inp["gate_W"]).T)
    w["gateB"] = _f32(np.asarray(inp["gate_b"])[:, None])
    pidx = np.arange(P)
    for pre in ("mf", "mb"):
        w[pre + "inWT"] = _bf16(_f32(inp[pre + "_in_W"]).transpose(0, 2, 1))
        cw = _f32(inp[pre + "_conv_W"])          # [NL, DI, DC]
        cd = np.zeros((NL, P, DIC, DC, P), np.float32)
        cd[:, pidx, :, :, pidx] = cw.reshape(NL, DIC, P, DC).transpose(
            2, 0, 1, 3)                           # -> [P, NL, DIC, DC]
        w[pre + "convD"] = _bf16(cd)
        dv = _f32(inp[pre + "_D"])                # [NL, DI]
        dd = np.zeros((NL, P, DIC, P), np.float32)
        dd[:, pidx, :, pidx] = dv.reshape(NL, DIC, P).transpose(2, 0, 1)
        w[pre + "diagD"] = _bf16(dd)
        w[pre + "cols"] = _f32(np.stack([_f32(inp[pre + "_conv_b"]),
                                         _f32(inp[pre + "_dt_b"]),
                                         dv], -1))
        xp = _f32(inp[pre + "_xproj_W"]).transpose(0, 2, 1)   # [NL, DI, 48]
        perm = list(range(DTR, DTR + 2 * DS)) + list(range(DTR))  # [B;C;dt]
        w[pre + "xpT"] = _bf16(xp[:, :, perm])
        w[pre + "dtWT"] = _bf16(_f32(inp[pre + "_dt_W"]).transpose(0, 2, 1))
        w[pre + "outWT"] = _bf16(_f32(inp[pre + "_out_W"]).transpose(0, 2, 1))
    for pre in ("af", "ab"):
        qkv = _f32(inp[pre + "_Wqkv"])
        bqkv = _f32(inp[pre + "_bqkv"])
        wo = _f32(inp[pre + "_Wo"])
        w[pre + "WqT"] = _bf16(qkv[:, 0:D].transpose(0, 2, 1))
        w[pre + "WkT"] = _bf16(qkv[:, D:2 * D].transpose(0, 2, 1))
        w[pre + "WvT"] = _bf16(qkv[:, 2 * D:].transpose(0, 2, 1))
        w[pre + "WoT"] = _bf16(wo.transpose(0, 2, 1))
        w[pre + "Bq"] = _f32(bqkv[:, 0:D][:, :, None])
        w[pre + "Bk"] = _f32(bqkv[:, D:2 * D][:, :, None])
        w[pre + "Bo"] = _f32((_f32(inp[pre + "_bo"])
                              + np.einsum('lod,ld->lo', wo, bqkv[:, 2 * D:]))[:, :, None])
    fgrows = []
    for g, b in (("fl_ln_g", "fl_ln_b"), ("glu_ln_g", "glu_ln_b")):
        fgrows.append(np.stack([_f32(inp[g]), _f32(inp[b])], 0)[None])
    w["lnFG"] = _bf16(np.concatenate(fgrows, 0))         # [2, 2, D]
    rows = []
    for nm in ("anf", "anb", "nf", "nb"):
        rows += [_f32(inp[nm + "_g"])[:, None, :],
                 _f32(inp[nm + "_b"])[:, None, :]]
    w["lnAll"] = _bf16(np.concatenate(rows, 1))          # [NL, 8, D]
    w["glu1WT"] = _bf16(_f32(inp["glu1_W"]).T)
    w["glu1B"] = _f32(np.asarray(inp["glu1_b"])[:, None])
    w["glu2WT"] = _bf16(_f32(inp["glu2_W"]).T)
    w["glu2B"] = _f32(np.asarray(inp["glu2_b"])[:, None])
    return w


def _scan_consts(inp):
    """Per-(dir, layer, state) decay scales a_n = -exp(Alog); the model's Alog
    is d-independent, verified here; baked into the emitted program (cache-
    keyed on the values)."""
    av = {}
    for pre in ("mf", "mb"):
        al = _f32(inp[pre + "_Alog"])            # [NL, DI, DS]
        a = -np.exp(al.astype(np.float64))
        med = np.median(a, axis=1)               # [NL, DS]
        assert np.abs(a - med[:, None, :]).max() < 1e-5 * np.abs(med).max(), \
            "Alog is d-dependent; scalar-scale dA path invalid"
        av[pre] = med
    return av


# ----------------------------------------------------------------- emit helpers
class Emit:
    def __init__(self, nc, tc, ctx):
        self.nc, self.tc = nc, tc
        self.sb = ctx.enter_context(tc.tile_pool(name="sb", bufs=1))
        self.s2p = ctx.enter_context(tc.tile_pool(name="s2p", bufs=2))
        self.s3p = ctx.enter_context(tc.tile_pool(name="s3p", bufs=2))
        self.pp = ctx.enter_context(tc.tile_pool(name="pp", bufs=2, space="PSUM"))
        self.pn = ctx.enter_context(tc.tile_pool(name="pn", bufs=2, space="PSUM"))
        self.pyac = ctx.enter_context(tc.tile_pool(name="pyac", bufs=1, space="PSUM"))

    def load_wT(self, drh, K, M, tag):
        nc = self.nc
        if not isinstance(drh, bass.AP):
            drh = drh[:, :]
        kc_n = (K + P - 1) // P
        t = self.sb.tile([min(K, P), kc_n, M], BF16, tag=tag, name="wT")
        if K % P == 0:
            st = drh.ap[-1][0]
            src = bass.AP(tensor=drh.tensor, offset=drh.offset,
                          ap=[[M * st, P], [P * M * st, kc_n], [st, M]])
            nc.sync.dma_start(out=t, in_=src)
        else:
            for kc in range(kc_n):
                kp = min(P, K - kc * P)
                nc.sync.dma_start(out=t[:kp, kc, :], in_=drh[kc * P:kc * P + kp, :])
        return t

    def load_col(self, drh, M, tag):
        nc = self.nc
        if not isinstance(drh, bass.AP):
            drh = drh[:, :]
        mc_n = (M + P - 1) // P
        t = self.sb.tile([P, mc_n], F32, tag=tag, name="col")
        if M % P == 0:
            src = bass.AP(tensor=drh.tensor, offset=drh.offset,
                          ap=[[1, P], [P, mc_n]])
            nc.sync.dma_start(out=t, in_=src)
        else:
            for mc in range(mc_n):
                mp = min(P, M - mc * P)
                nc.sync.dma_start(out=t[:mp, mc:mc + 1],
                                  in_=drh[mc * P:mc * P + mp, :])
        return t

    def dense(self, x, wT, Mout, bias=None, act=None, out=None, out_pool=None,
              out_tag=None, Fw=None, out_dt=BF16):
        nc = self.nc
        Fw = Fw or F
        kc_n = x.shape[1]
        mc_n = (Mout + P - 1) // P
        if out is None:
            out = (out_pool or self.s3p).tile([P, mc_n, Fw], out_dt,
                                              tag=out_tag, name="dn")
        for mc in range(mc_n):
            mp = min(P, Mout - mc * P)
            ps = self.pp.tile([P, 512], F32, tag="mm", name="ps")
            for kc in range(kc_n):
                nc.tensor.matmul(ps[:mp, :Fw],
                                 wT[:, kc, mc * P:mc * P + mp],
                                 x[:, kc, 0:Fw],
                                 start=(kc == 0), stop=(kc == kc_n - 1))
            bap = bias[:mp, mc:mc + 1] if bias is not None else None
            if act is None and bias is None:
                nc.scalar.copy(out[:mp, mc, 0:Fw], ps[:mp, :Fw])
            else:
                nc.scalar.activation(out[:mp, mc, 0:Fw], ps[:mp, :Fw],
                                     act or AF.Identity,
                                     bias=bap if bap is not None else 0.0,
                                     scale=1.0)
        return out

    def add(self, out, a, b):
        self.nc.vector.tensor_add(out, a, b)

    def mul(self, out, a, b):
        self.nc.vector.tensor_mul(out, a, b)

    def act(self, out, in_, func, bias=0.0, scale=1.0):
        self.nc.scalar.activation(out=out, in_=in_, func=func, bias=bias, scale=scale)


def rev_view(ap2, n_blk, blk):
    st = ap2.ap[-1][0]
    off = ap2.offset + (blk - 1) * st
    if n_blk == 1:
        return bass.AP(tensor=ap2.tensor, offset=off, ap=[ap2.ap[0], [-st, blk]])
    return bass.AP(tensor=ap2.tensor, offset=off,
                   ap=[ap2.ap[0], [blk * st, n_blk], [-st, blk]])


def _g_layer_norm(E, x, gR, bR, eps, out, x_is_f32=False, tag=""):
    """x, out: [128, 2, F] feature-major (D=256 on partitions). gR/bR bf16
    rows [1, D].  Generator: yields at chunk boundaries."""
    nc = E.nc
    fw = F
    stat = E.sb.tile([1, 2, 512], F32, tag="lnstat" + tag, name="stat")
    A = stat[0:1, 0, :fw]          # m, later m*r
    Bv = stat[0:1, 1, :fw]         # q, later var, later r
    mrb = E.sb.tile([1, 2, 512], BF16, tag="lnthinb" + tag, name="mrb")
    xsq = E.sb.tile([P, 2, 512], BF16, tag="xsq", name="xsq")
    E.act(xsq, x, AF.Square)
    ones = E.ones128f if x_is_f32 else E.ones128
    for which, dst in ((0, A), (1, Bv)):
        ps = E.pn.tile([P, 512], F32, tag="th", name="ps")
        for kc in range(2):
            if which == 0:
                nc.tensor.matmul(ps[0:1, :fw], ones, x[:, kc, 0:fw],
                                 start=(kc == 0), stop=(kc == 1))
            else:
                nc.tensor.matmul(ps[0:1, :fw], E.ones128, xsq[:, kc, 0:fw],
                                 start=(kc == 0), stop=(kc == 1))
        nc.vector.tensor_scalar_mul(dst, ps[0:1, :fw], 1.0 / D)
        yield
    E.act(mrb[0:1, 0, :fw], A, AF.Square)          # m^2 (bf16 scratch)
    nc.vector.tensor_tensor(Bv, Bv, mrb[0:1, 0, :fw], OP.subtract)
    E.act(Bv, Bv, AF.Ln, bias=E.eps[eps][0:1, 0:1])
    E.act(Bv, Bv, AF.Exp, scale=-0.5)              # r (f32)
    nc.vector.tensor_copy(mrb[0:1, 0, :fw], Bv)    # r (bf16)
    E.mul(A, A, Bv)                                # m*r (f32, in place)
    E.act(mrb[0:1, 1, :fw], A, AF.Identity, scale=-1.0)   # -m*r (bf16)
    yield
    for mc in range(2):
        gRc = gR[0:1, mc * P:(mc + 1) * P]
        bRc = bR[0:1, mc * P:(mc + 1) * P]
        ps_s = E.pn.tile([P, 512], F32, tag="th", name="ps_s")
        nc.tensor.matmul(ps_s[:, :fw], gRc, mrb[0:1, 0, :fw], start=True, stop=True)
        ps_o = E.pn.tile([P, 512], F32, tag="th", name="ps_o")
        nc.tensor.matmul(ps_o[:, :fw], bRc, E.onesF[0:1, :fw],
                         start=True, stop=False)
        nc.tensor.matmul(ps_o[:, :fw], gRc, mrb[0:1, 1, :fw], start=False, stop=True)
        tmp = E.s2p.tile([P, 512], BF16, tag="lntmp", name="tmp", bufs=1)
        E.mul(tmp[:, :fw], x[:, mc, 0:fw], ps_s[:, :fw])
        E.add(out[:, mc, 0:fw], tmp[:, :fw], ps_o[:, :fw])
        yield


def _g_attention(E, q_src, kv_src, wq, wk, wv, wo, bq, bk, bo, out_tag, ob, okey):
    """MHA over PB samples; q_src/kv_src [128, 2, F] fm bf16.  Generator;
    result tile into ob[okey]."""
    nc = E.nc
    ofm = E.s3p.tile([P, 2, F], BF16, tag="t8", name="ofm")
    se = E.sb.tile([1, H, PB, S], BF16, tag="thin8", name="se")
    qf = E.s2p.tile([P, 2, F], BF16, tag="qfb", name="qf", bufs=1)
    kf = E.s2p.tile([P, 2, F], BF16, tag="kfb", name="kf", bufs=1)
    for mc in range(2):
        for dst, wT, bias, srcT in ((qf, wq, bq, q_src), (kf, wk, bk, kv_src)):
            ps = E.pp.tile([P, 512], F32, tag="mm", name="ps")
            for kc in range(2):
                nc.tensor.matmul(ps[:, :F], wT[:, kc, mc * P:(mc + 1) * P],
                                 srcT[:, kc, :], start=(kc == 0), stop=(kc == 1))
            nc.scalar.activation(dst[:, mc, :], ps[:, :F], AF.Identity,
                                 bias=bias[:, mc:mc + 1], scale=1.0)
            yield
    for b in range(PB):
        vtm = E.s2p.tile([P, 2, D], BF16, tag="vtmb", name="vtm", bufs=1)
        ps = E.pp.tile([P, 512], F32, tag="mm", name="ps")
        for tcn in range(2):
            for kc in range(2):
                nc.tensor.matmul(ps[:, tcn * D:(tcn + 1) * D],
                                 kv_src[:, kc, b * S + tcn * P: b * S + (tcn + 1) * P],
                                 wv[:, kc, :], start=(kc == 0), stop=(kc == 1))
        nc.scalar.copy(vtm[:, :, :].rearrange("p a d -> p (a d)"), ps[:, :])
        yield
        pse = None
        for h in range(H):
            hc, off = h // 2, (h % 2) * 64
            expT = E.s2p.tile([P, 2, S], BF16, tag="expT", name="expT", bufs=1)
            ps = E.pp.tile([P, 512], F32, tag="mm", name="ps")
            for kc in range(2):
                nc.tensor.matmul(ps[:, kc * S:(kc + 1) * S],
                                 kf[off:off + 64, hc, b * S + kc * P:b * S + (kc + 1) * P],
                                 qf[off:off + 64, hc, b * S:(b + 1) * S],
                                 start=True, stop=True)
            E.act(expT, ps, AF.Exp, scale=1.0 / np.sqrt(HD))
            if h % 2 == 0:
                pse = E.pn.tile([P, 512], F32, tag="th", name="pse")
            for kc in range(2):
                nc.tensor.matmul(pse[0:1, (h % 2) * S:(h % 2) * S + S],
                                 E.ones128, expT[:, kc, :],
                                 start=(kc == 0), stop=(kc == 1))
            if h % 2 == 1:
                E.act(se[0:1, h - 1:h + 1, b, :],
                      pse[0:1, :].rearrange("p (h s) -> p h s", h=2), AF.Ln)
            if h % 2 == 0:
                psav = E.pp.tile([P, 512], F32, tag="mm", name="psav")
            for kc in range(2):
                nc.tensor.matmul(psav[off:off + 64, :S],
                                 vtm[:, kc, h * 64:(h + 1) * 64],
                                 expT[:, kc, :], start=(kc == 0), stop=(kc == 1))
            if h % 2 == 1:
                nc.scalar.copy(ofm[:, hc, b * S:(b + 1) * S], psav[:, :S])
            yield
    E.act(se, se, AF.Exp, scale=-1.0)              # 1/sumexp, in place
    yield
    for h in range(H):
        dc, off = h // 2, (h % 2) * 64
        ps = E.pn.tile([P, 512], F32, tag="th", name="ps")
        nc.tensor.matmul(ps[0:64, :F], E.ones1x64,
                         se[0:1, h].rearrange("p b s -> p (b s)"),
                         start=True, stop=True)
        E.mul(ofm[off:off + 64, dc, :], ofm[off:off + 64, dc, :], ps[0:64, :F])
        yield
    ob[okey] = E.dense(ofm, wo, D, bias=bo, out_tag=out_tag)


def _g_mamba_prep_a(E, io, x, pre, l, flip, pr):
    """Silu-table phase: weights, in-proj xi, conv via host diag mats, z."""
    nc = E.nc
    d = pre
    inW = E.load_wT(io[pre + "inWT"][l], D, 2 * DI, "inW")
    cols = E.sb.tile([P, DIC, 3], F32, tag="mcols" + d, name="cols")
    cd = io[pre + "cols"][l]
    nc.sync.dma_start(out=cols, in_=bass.AP(
        tensor=cd.tensor, offset=cd.offset, ap=[[3, P], [P * 3, DIC], [1, 3]]))
    convD = E.sb.tile([P, DIC, DC, P], BF16, tag="convD", name="convD")
    nc.sync.dma_start(out=convD, in_=io[pre + "convD"][l])
    diagD = E.sb.tile([P, DIC, P], BF16, tag="diagD" + d, name="diagD")
    nc.sync.dma_start(out=diagD, in_=io[pre + "diagD"][l])
    xpw = E.load_wT(io[pre + "xpT"][l], DI, DTR + 2 * DS, "xpw" + d)
    dtw = E.sb.tile([2 * DS + DTR, DI], BF16, tag="dtw" + d, name="dtw")
    nc.sync.dma_start(out=dtw[2 * DS:, :], in_=io[pre + "dtWT"][l])
    ow = E.load_wT(io[pre + "outWT"][l], DI, D, "outW" + d)
    yield

    def inproj(c0, dst_tag, silu):
        dst = E.sb.tile([P, DIC, F], BF16, tag=dst_tag, name="xi")
        for c in range(DIC):
            ps = E.pp.tile([P, 512], F32, tag="mm", name="ps")
            for b in range(PB):
                for kc in range(2):
                    rhs = x[:, kc, b * S:(b + 1) * S]
                    if flip:
                        rhs = rev_view(rhs, 1, S)
                    nc.tensor.matmul(ps[:, b * S:(b + 1) * S],
                                     inW[:, kc, (c0 + c) * P:(c0 + c + 1) * P], rhs,
                                     start=(kc == 0), stop=(kc == 1))
            if silu:
                E.act(dst[:, c, :], ps, AF.Silu)
            else:
                nc.scalar.copy(dst[:, c, :], ps)
        return dst

    xi = inproj(0, "xi", False)
    yield
    xc = E.s2p.tile([P, DIC, F], BF16, tag="xc", name="xc")
    for c in range(DIC):
        ps = E.pp.tile([P, 512], F32, tag="mm", name="ps")
        for b in range(PB):
            nc.tensor.matmul(ps[:, b * S:(b + 1) * S], convD[:, c, DC - 1, :],
                             xi[:, c, b * S:(b + 1) * S], start=True, stop=False)
            for j in range(DC - 1):
                sh = DC - 1 - j
                nc.tensor.matmul(ps[:, b * S + sh:(b + 1) * S], convD[:, c, j, :],
                                 xi[:, c, b * S:(b + 1) * S - sh],
                                 start=False, stop=(j == DC - 2))
        E.act(xc[:, c, :], ps, AF.Silu, bias=cols[:, c, 0:1])
        yield
    z = inproj(DIC, "z" + d, True)
    yield
    pr.update(xc=xc, z=z, diagD=diagD, ow=ow, xpw=xpw, dtw=dtw, cols=cols)


def _g_mamba_prep_b(E, io, pr, pre, l, bcd):
    """NLE-table phase: xproj -> dbl (+DRAM bounce), dt softplus, dtu."""
    nc = E.nc
    d = pre
    xc, xpw, dtw, cols = pr["xc"], pr["xpw"], pr["dtw"], pr["cols"]
    dbl = E.sb.tile([DTR + 2 * DS, F], BF16, tag="dbl" + d, name="dbl")
    ps = E.pp.tile([P, 512], F32, tag="mm", name="ps")
    for kc in range(DIC):
        nc.tensor.matmul(ps[:DTR + 2 * DS, :F], xpw[:, kc, :], xc[:, kc, :],
                         start=(kc == 0), stop=(kc == DIC - 1))
    nc.scalar.copy(dbl, ps[:DTR + 2 * DS, :F])
    nc.sync.dma_start(out=bcd[:, :], in_=dbl[0:2 * DS, :])
    yield
    dt = E.sb.tile([P, DIC, F], BF16, tag="dt" + d, name="dt")
    for mc in range(DIC):
        ps = E.pp.tile([P, 512], F32, tag="mm", name="ps")
        nc.tensor.matmul(ps[:, :F], dtw[2 * DS:, mc * P:(mc + 1) * P],
                         dbl[2 * DS:2 * DS + DTR, :], start=True, stop=True)
        dtx = E.sb.tile([P, F], BF16, tag="dtx", name="dtx")
        E.act(dtx, ps[:, :F], AF.Exp, bias=cols[:, mc, 1:2])
        E.act(dt[:, mc, :], dtx, AF.Ln, bias=1.0)
        yield
    dtu = E.sb.tile([P, DIC, F], BF16, tag="dtu" + d, name="dtu")
    E.mul(dtu, dt, xc)
    nc.vector.memset(dt[:, :, 0:F:S], 1.0e30)
    pr.update(dt=dt, dtu=dtu, bcd=bcd)


def _mamba_scan(E, pr, avl, out_tag, bg, pump):
    """Per-state scan loop: dA on ACT (literal scalar scale), scans + h*C
    muls on DVE (bf16), dtu*B muls mostly on Pool, Sum_n C*h_n (+ D*u)
    accumulated in PSUM by PE identity matmuls; y*silu(z) reads the PSUM
    accumulators directly on DVE.  bg is pumped each iteration."""
    nc = E.nc
    dt, dtu, z = pr["dt"], pr["dtu"], pr["z"]
    yac = [E.pyac.tile([P, 512], F32, tag=f"yac{c}", name="yac") for c in range(DIC)]
    for c in range(DIC):
        nc.tensor.matmul(yac[c], pr["diagD"][:, c, :], pr["xc"][:, c, :],
                         start=True, stop=False, skip_group_check=True)
    dt2 = dt[:, :, :].rearrange("p c f -> p (c f)")
    bcd = pr["bcd"][:, :]
    for n in range(DS):
        bg.pump(pump)
        dA = E.s2p.tile([P, DIC * F], BF16, tag="dA", name="dA")
        E.act(dA, dt2, AF.Exp, scale=float(avl[n]))
        bc = E.s2p.tile([P, 2, F], BF16, tag="bc", name="bc", bufs=3)
        src = bass.AP(tensor=bcd.tensor, offset=bcd.offset + n * F,
                      ap=[[0, P], [DS * F, 2], [1, F]])
        nc.sync.dma_start(out=bc, in_=src)
        dBu = E.s2p.tile([P, DIC, F], BF16, tag="dBu", name="dBu", bufs=3)
        bview = bass.AP(tensor=bc.tensor, offset=bc.offset,
                        ap=[bc.ap[0], [0, DIC], [1, F]])
        # the dtu*B broadcast mul rides the otherwise-idle Pool engine for
        # most states (Multiply has a real gpsimd impl; the scan does not)
        meng = nc.vector if n in (0, 5, 10, 15) else nc.gpsimd
        meng.tensor_mul(dBu, dtu, bview)
        hn = E.s2p.tile([P, DIC, F], BF16, tag="hn", name="hn", bufs=3)
        nc.vector.tensor_tensor_scan(
            out=hn[:, :, :].rearrange("p c f -> p (c f)"),
            data0=dA[:, :],
            data1=dBu[:, :, :].rearrange("p c f -> p (c f)"),
            initial=0.0, op0=OP.mult, op1=OP.add)
        hnC = E.sb.tile([P, DIC, F], BF16, tag="hnC", name="hnC")
        cview = bass.AP(tensor=bc.tensor, offset=bc.offset + F,
                        ap=[bc.ap[0], [0, DIC], [1, F]])
        E.mul(hnC, hn, cview)
        last = (n == DS - 1)
        for c in range(DIC):
            nc.tensor.matmul(yac[c], E.identb, hnC[:, c, :],
                             start=False, stop=last,
                             skip_group_check=True)
    bg.pump(2)
    y = dtu            # dtu is dead after the last dBu; reuse its buffer
    for c in range(DIC):
        E.mul(y[:, c, :], z[:, c, :], yac[c])
    return E.dense(y, pr["ow"], D, out_pool=E.s2p, out_tag=out_tag)


# ------------------------------------------------------------------- program
def build_program(wshapes, av):
    nc = _Bacc()
    io = {}
    io["input"] = nc.declare_dram_parameter("input", [BC, S, D], F32, isOutput=False)
    for k, shp, dt in wshapes:
        io[k] = nc.declare_dram_parameter(k, list(shp), dt, isOutput=False)
    io["out"] = nc.declare_dram_parameter("out", [BC, S, D], F32, isOutput=True)
    for pss in range(NPASS):
        for l in range(NL):
            for pre in ("mf", "mb"):
                io[f"bcrows_{pss}_{l}_{pre}"] = nc.dram_tensor(
                    f"bcrows_{pss}_{l}_{pre}", [2 * DS, F], BF16)
    with tile.TileContext(nc) as tc:
        with ExitStack() as ctx:
            E = Emit(nc, tc, ctx)
            identb = E.sb.tile([P, P], BF16, tag="identb", name="identb")
            make_identity(nc, identb)
            E.identb = identb
            identf = E.sb.tile([P, P], F32, tag="identf", name="identf")
            make_identity(nc, identf)
            E.identf = identf
            E.ones128 = E.sb.tile([P, 1], BF16, tag="ones128", name="ones128")
            nc.vector.memset(E.ones128, 1.0)
            E.ones128f = E.sb.tile([P, 1], F32, tag="ones128f", name="ones128f")
            nc.vector.memset(E.ones128f, 1.0)
            E.ones1x64 = E.sb.tile([1, 64], BF16, tag="ones64", name="ones64")
            nc.vector.memset(E.ones1x64, 1.0)
            E.ones1xP = E.sb.tile([1, P], BF16, tag="ones1p", name="ones1p")
            nc.vector.memset(E.ones1xP, 1.0)
            E.onesF = E.sb.tile([1, 512], BF16, tag="onesF", name="onesF")
            nc.vector.memset(E.onesF, 1.0)
            E.eps = {}
            for ev in (1e-5, 1e-12):
                t = E.sb.tile([1, 1], F32, tag=f"eps{ev}", name="eps")
                nc.vector.memset(t, ev)
                E.eps[ev] = t
            # software-pipelined pass interleave: pass-1's FFT/wavelet/gate
            # stage and layer preps are emitted inside pass-0's scan windows
            # so the Pool engine (scans) never drains.
            bg = _BG()
            box = {}
            c00, c10, c01, c11 = {}, {}, {}, {}
            _run(_g_stage03(E, io, 0, box, "x1a"))
            _run(_g_layer_preps(E, io, 0, 0, lambda: box["x1a"], av, c00))
            bg.add(_chain(
                _g_stage03(E, io, 1, box, "x1b"),
                _g_layer_preps(E, io, 1, 0, lambda: box["x1b"], av, c10)))
            _emit_layer_scans(E, c00, av, bg)
            bg.drain()
            bg.add(_chain(
                _g_layer_post(E, c00, box, "x1a"),
                _g_layer_preps(E, io, 0, 1, lambda: box["x1a"], av, c01)))
            _emit_layer_scans(E, c10, av, bg)
            bg.drain()
            bg.add(_chain(
                _g_layer_post(E, c10, box, "x1b"),
                _g_layer_preps(E, io, 1, 1, lambda: box["x1b"], av, c11)))
            _emit_layer_scans(E, c01, av, bg)
            bg.drain()
            bg.add(_chain(
                _g_layer_post(E, c01, box, "x1a"),
                _g_glu(E, io, 0, lambda: box["x1a"])))
            _emit_layer_scans(E, c11, av, bg,
                              mid_add=_chain(_g_post_attn(E, c11, "mf", "af"),
                                             _g_post_lnt(E, c11)))
            bg.drain()
            _run(_g_layer_post(E, c11, box, "x1b"))
            _run(_g_glu(E, io, 1, lambda: box["x1b"]))
    nc.finalize()
    return nc


class _BG:
    def __init__(self):
        from collections import deque
        self.q = deque()

    def add(self, gen):
        self.q.append(gen)

    def pump(self, n=1):
        while n > 0 and self.q:
            try:
                next(self.q[0])
                n -= 1
            except StopIteration:
                self.q.popleft()

    def drain(self):
        while self.q:
            self.pump(64)


def _run(gen):
    for _ in gen:
        pass


def _chain(*gens):
    for g in gens:
        yield from g


def _g_stage03(E, io, pss, box, key):
    nc = E.nc
    # ---------------- stage 0: load x + cast + transpose to feature-major
    x_tm = E.sb.tile([P, PB * 2, D], BF16, tag="xtm", name="x_tm")
    for b in range(PB):
        for sc in range(2):
            xch = E.s2p.tile([P, D], F32, tag="xt32", name="xch")
            nc.sync.dma_start(out=xch,
                              in_=io["input"][pss * PB + b, sc * P:(sc + 1) * P, :])
            nc.vector.tensor_copy(x_tm[:, b * 2 + sc, :], xch)
    yield
    x_fm = E.sb.tile([P, 2, F], BF16, tag="xfm", name="x_fm")
    for b in range(PB):
        for sc in range(2):
            for dc in range(2):
                pst = E.pn.tile([P, P], BF16, tag="th", name="pst")
                nc.tensor.transpose(pst, x_tm[:, b * 2 + sc, dc * P:(dc + 1) * P],
                                    E.identb)
                nc.scalar.copy(x_fm[:, dc, b * S + sc * P: b * S + (sc + 1) * P], pst)
            yield

    # ---------------- stage 1: FFT path
    frT = E.load_wT(io["frT"], S, NF, "frT")
    fiT = E.load_wT(io["fiT"], S, NF, "fiT")
    fftWa = E.load_wT(io["fftWa"], 513, 2 * D, "fftWa")
    grT = E.load_wT(io["grT"], NF, S, "grT")
    giT = E.load_wT(io["giT"], NF, S, "giT")
    yield
    x_fft = E.sb.tile([P, 2, F], BF16, tag="xfft", name="x_fft")
    for b in range(PB):
        comb = E.s3p.tile([P, 4, NF], BF16, tag="t8", name="comb")
        for ri, mat in ((0, frT), (1, fiT)):
            for mc in range(2):
                ps = E.pp.tile([P, 512], F32, tag="mm", name="ps")
                for kc in range(2):
                    nc.tensor.matmul(ps[:, :NF], x_tm[:, b * 2 + kc, mc * P:(mc + 1) * P],
                                     mat[:, kc, :], start=(kc == 0), stop=(kc == 1))
                nc.scalar.copy(comb[:, ri * 2 + mc, :], ps[:, :NF])
                yield
        filt = E.s3p.tile([P, 2 * D], BF16, tag="t8", name="filt")
        filtN = E.sb.tile([1, 2 * D], BF16, tag="filtN", name="filtN")
        for mt, mp, f0 in ((filt, P, 0), (filtN, 1, P)):
            ps = E.pp.tile([P, 512], F32, tag="mm", name="ps")
            for kc in range(4):
                nc.tensor.matmul(ps[:mp, :], comb[:, kc, f0:f0 + mp], fftWa[:, kc, :],
                                 start=(kc == 0), stop=False)
            nc.tensor.matmul(ps[:mp, :], E.ones1xP[0:1, 0:mp], fftWa[0:1, 4, :],
                             start=False, stop=True)
            E.act(mt[0:mp, :] if mt is filtN else mt, ps[:mp, :], AF.Gelu)
            yield
        for mc in range(2):
            ps = E.pp.tile([P, 512], F32, tag="mm", name="ps")
            nc.tensor.matmul(ps[:, :S], filt[:, mc * P:(mc + 1) * P], grT[:, 0, :],
                             start=True, stop=False)
            nc.tensor.matmul(ps[:, :S], filtN[0:1, mc * P:(mc + 1) * P], grT[0:1, 1, :],
                             start=False, stop=False)
            nc.tensor.matmul(ps[:, :S], filt[:, D + mc * P:D + (mc + 1) * P], giT[:, 0, :],
                             start=False, stop=False)
            nc.tensor.matmul(ps[:, :S], filtN[0:1, D + mc * P:D + (mc + 1) * P],
                             giT[0:1, 1, :], start=False, stop=True)
            nc.scalar.copy(x_fft[:, mc, b * S:(b + 1) * S], ps[:, :S])
            yield

    # ---------------- stage 2: wavelet path
    tdT = E.load_wT(io["tdT"], S, L2, "tdT")
    iiT = E.sb.tile([L2, S], BF16, tag="iiT", name="iiT")
    nc.sync.dma_start(out=iiT, in_=io["iiT"][:, :])
    wl1T = [E.load_wT(io["wl1T"][k], D, D, t) for k, t in enumerate(("awq", "awk", "awv"))]
    wl2T = [E.load_wT(io["wl2T"][k], D, D, t) for k, t in enumerate(("awo", "wlo1", "wlo2"))]
    wl1b = E.load_col(io["wl1b"], D, "wl1b")
    wl2b = E.load_col(io["wl2b"], D, "wl2b")
    yield
    x_wl = E.sb.tile([P, 2, F], BF16, tag="xwl", name="x_wl")
    a_fm = E.sb.tile([P, 2, PB, L2], BF16, tag="afm", name="a_fm")
    for b in range(PB):
        for mc in range(2):
            ps = E.pp.tile([P, 512], F32, tag="mm", name="ps")
            for kc in range(2):
                nc.tensor.matmul(ps[:, :L2], x_tm[:, b * 2 + kc, mc * P:(mc + 1) * P],
                                 tdT[:, kc, :], start=(kc == 0), stop=(kc == 1))
            nc.scalar.copy(a_fm[:, mc, b, :], ps[:, :L2])
    yield

    def conv3(src, wT, bcol, actf, dst_tag):
        dst = E.s2p.tile([P, 2, PB, L2], BF16, tag=dst_tag, name="c3")
        for b in range(PB):
            for mc in range(2):
                ps = E.pp.tile([P, 512], F32, tag="mm", name="ps")
                for kc in range(2):
                    nc.tensor.matmul(ps[:, :L2], wT[1][:, kc, mc * P:(mc + 1) * P],
                                     src[:, kc, b, :], start=(kc == 0), stop=False)
                for kc in range(2):
                    nc.tensor.matmul(ps[:, 1:L2], wT[0][:, kc, mc * P:(mc + 1) * P],
                                     src[:, kc, b, 0:L2 - 1], start=False, stop=False)
                for kc in range(2):
                    nc.tensor.matmul(ps[:, 0:L2 - 1], wT[2][:, kc, mc * P:(mc + 1) * P],
                                     src[:, kc, b, 1:L2], start=False, stop=(kc == 1))
                E.act(dst[:, mc, b, :], ps[:, :L2], actf, bias=bcol[:, mc:mc + 1])
        return dst

    c1 = conv3(a_fm, wl1T, wl1b, AF.Gelu, "c1")  # s2p ring
    yield
    c2 = conv3(c1, wl2T, wl2b, AF.Identity, "afm")
    yield
    c2T = E.sb.tile([L2, 2, PB, P], BF16, tag="c2T", name="c2T")
    for b in range(PB):
        for mc in range(2):
            pst = E.pn.tile([P, P], BF16, tag="th", name="pst")
            nc.tensor.transpose(pst[0:L2, :], c2[:, mc, b, :], E.identb)
            nc.scalar.copy(c2T[:, mc, b, :], pst[0:L2, :])
    yield
    for b in range(PB):
        for mc in range(2):
            ps = E.pp.tile([P, 512], F32, tag="mm", name="ps")
            nc.tensor.matmul(ps[:, :S], c2T[:, mc, b, :], iiT, start=True, stop=True)
            nc.scalar.copy(x_wl[:, mc, b * S:(b + 1) * S], ps[:, :S])
    yield

    # ---------------- stage 3: cross-attention + gate + LN
    caWq = E.load_wT(io["caWqT"], D, D, "awq")
    caWk = E.load_wT(io["caWkT"], D, D, "awk")
    caWv = E.load_wT(io["caWvT"], D, D, "awv")
    caWo = E.load_wT(io["caWoT"], D, D, "awo")
    caBq = E.load_col(io["caBq"], D, "abq")
    caBk = E.load_col(io["caBk"], D, "abk")
    caBo = E.load_col(io["caBo"], D, "abo")
    ab = {}
    yield from _g_attention(E, x_fft, x_wl, caWq, caWk, caWv, caWo, caBq, caBk,
                            caBo, "t8", ab, "att")
    fused = E.s3p.tile([P, 2, F], BF16, tag="t8", name="fused")
    E.add(fused, ab["att"], x_fm)
    gateW = E.load_wT(io["gateWT"], 2 * D, 2 * D, "gateW")
    gateB = E.load_col(io["gateB"], 2 * D, "gateB")
    ga = E.s3p.tile([P, 2, F], BF16, tag="t8", name="ga")
    gb = E.s3p.tile([P, 2, F], BF16, tag="t8", name="gb")
    for mc in range(4):
        actf = AF.Identity if mc < 2 else AF.Sigmoid
        gdst = ga if mc < 2 else gb
        ps = E.pp.tile([P, 512], F32, tag="mm", name="ps")
        for kc in range(4):
            gsrc = fused if kc < 2 else x_fm
            nc.tensor.matmul(ps[:, :F], gateW[:, kc, mc * P:(mc + 1) * P],
                             gsrc[:, kc % 2, :], start=(kc == 0), stop=(kc == 3))
        E.act(gdst[:, mc % 2, :], ps[:, :F], actf, bias=gateB[:, mc:mc + 1])
        yield
    gated = ga
    E.mul(gated, ga, gb)
    flt = E.s2p.tile([1, 2, D], BF16, tag="lnFG", name="flt")
    nc.sync.dma_start(out=flt, in_=io["lnFG"][0])
    x1 = E.s2p.tile([P, 2, F], BF16, tag="x1", name="x1", bufs=3)
    yield from _g_layer_norm(E, gated, flt[0:1, 0, :], flt[0:1, 1, :], 1e-5, x1)
    box[key] = x1


_DIRS = (("mf", "af", False, "anf", "nf"),
         ("mb", "ab", True, "anb", "nb"))


def _g_layer_preps(E, io, pss, l, x1f, av, cd):
    x1 = x1f()
    prs = {}
    for (mp, ap_, flip, anG, nG) in _DIRS:
        prs[mp] = {}
        yield from _g_mamba_prep_a(E, io, x1, mp, l, flip, prs[mp])
    for (mp, ap_, flip, anG, nG) in _DIRS:
        bcd = io[f"bcrows_{pss}_{l}_{mp}"]
        yield from _g_mamba_prep_b(E, io, prs[mp], mp, l, bcd)
    cd.update(prs=prs, x1=x1, l=l, io=io, pss=pss)


def _emit_layer_scans(E, cd, av, bg, mid_add=None):
    cd["ms"] = {}
    for di, (mp, ap_, flip, anG, nG) in enumerate(_DIRS):
        cd["ms"][mp] = _mamba_scan(E, cd["prs"][mp], av[mp][cd["l"]],
                                   "ms" + mp, bg, pump=3 if di == 0 else 5)
        if di == 0 and mid_add is not None:
            bg.add(mid_add)


def _g_post_attn(E, cd, mp, ap_):
    nc = E.nc
    io, l = cd["io"], cd["l"]
    ab = {}
    wq = E.load_wT(io[ap_ + "WqT"][l], D, D, "awq" + mp)
    wk = E.load_wT(io[ap_ + "WkT"][l], D, D, "awk" + mp)
    wv = E.load_wT(io[ap_ + "WvT"][l], D, D, "awv" + mp)
    wo = E.load_wT(io[ap_ + "WoT"][l], D, D, "awo" + mp)
    abq = E.load_col(io[ap_ + "Bq"][l], D, "abq" + mp)
    abk = E.load_col(io[ap_ + "Bk"][l], D, "abk" + mp)
    abo = E.load_col(io[ap_ + "Bo"][l], D, "abo" + mp)
    ms = cd["ms"][mp]
    yield from _g_attention(E, ms, ms, wq, wk, wv, wo, abq, abk, abo,
                            "t8", ab, "att")
    E.add(ms, ms, ab["att"])
    cd.setdefault("s2d", {})[mp] = ms
    yield


def _g_post_lnt(E, cd):
    nc = E.nc
    if "lnt" in cd:
        return
    lnt = E.s2p.tile([1, 8, D], BF16, tag="lnAll", name="lnt")
    nc.sync.dma_start(out=lnt, in_=cd["io"]["lnAll"][cd["l"]])
    cd["lnt"] = lnt
    yield


def _ln_params(cd, name):
    nidx = {"anf": 0, "anb": 1, "nf": 2, "nb": 3}
    i = nidx[name] * 2
    lnt = cd["lnt"]
    return (lnt[0:1, i, :], lnt[0:1, i + 1, :])


def _g_post_lns(E, cd, mp, flip, anG, nG):
    x1 = cd["x1"]
    s3 = E.s3p.tile([P, 2, F], BF16, tag="t8", name="s3")
    (ang, anb_) = _ln_params(cd, anG)
    yield from _g_layer_norm(E, cd["s2d"][mp], ang, anb_, 1e-5, s3)
    s4 = E.s3p.tile([P, 2, F], BF16, tag="t8", name="s4")
    if flip:
        for kc in range(2):
            E.add(s4[:, kc, :].rearrange("p (b s) -> p b s", b=PB),
                  rev_view(s3[:, kc, :], PB, S),
                  x1[:, kc, :].rearrange("p (b s) -> p b s", b=PB))
    else:
        E.add(s4, s3, x1)
    yield
    s5 = E.s2p.tile([P, 2, F], BF16, tag="s5", name="s5")
    (ng, nb_) = _ln_params(cd, nG)
    yield from _g_layer_norm(E, s4, ng, nb_, 1e-5, s5)
    cd.setdefault("s5d", {})[mp] = s5


def _g_layer_post(E, cd, box, key):
    done = cd.get("s2d", {})
    for (mp, ap_, flip, anG, nG) in _DIRS:
        if mp not in done:
            yield from _g_post_attn(E, cd, mp, ap_)
    yield from _g_post_lnt(E, cd)
    s5d = cd.get("s5d", {})
    for (mp, ap_, flip, anG, nG) in _DIRS:
        if mp not in s5d:
            yield from _g_post_lns(E, cd, mp, flip, anG, nG)
    x1n = E.s2p.tile([P, 2, F], BF16, tag="x1", name="x1n", bufs=3)
    E.add(x1n, cd["s5d"]["mf"], cd["s5d"]["mb"])
    box[key] = x1n


def _g_glu(E, io, pss, x1f):
    nc = E.nc
    x1 = x1f()
    # ---------------- stage 5: GLU + final LN
    glu1W = E.load_wT(io["glu1WT"], D, 2 * D, "glu1W")
    glu1B = E.load_col(io["glu1B"], 2 * D, "glu1B")
    va = E.s3p.tile([P, 2, F], BF16, tag="t8", name="va")
    vb = E.s3p.tile([P, 2, F], BF16, tag="t8", name="vb")
    for mc in range(4):
        actf = AF.Identity if mc < 2 else AF.Sigmoid
        vdst = va if mc < 2 else vb
        ps = E.pp.tile([P, 512], F32, tag="mm", name="ps")
        for kc in range(2):
            nc.tensor.matmul(ps[:, :F], glu1W[:, kc, mc * P:(mc + 1) * P],
                             x1[:, kc, :], start=(kc == 0), stop=(kc == 1))
        E.act(vdst[:, mc % 2, :], ps[:, :F], actf, bias=glu1B[:, mc:mc + 1])
        yield
    gv = va
    E.mul(gv, va, vb)
    glu2W = E.load_wT(io["glu2WT"], D, D, "glu2W")
    glu2B = E.load_col(io["glu2B"], D, "glu2B")
    gvo = E.dense(gv, glu2W, D, bias=glu2B, out_tag="t8")
    yield
    res = E.sb.tile([P, 2, F], F32, tag="res", name="res")
    E.add(res, gvo, x1)
    glt = E.s2p.tile([1, 2, D], BF16, tag="lnFG", name="glt")
    nc.sync.dma_start(out=glt, in_=io["lnFG"][1])
    out_fm = E.sb.tile([P, 2, F], F32, tag="reso", name="out_fm")
    yield from _g_layer_norm(E, res, glt[0:1, 0, :], glt[0:1, 1, :], 1e-12, out_fm,
                             x_is_f32=True)

    # ---------------- stage 6: transpose + store
    for b in range(PB):
        for sc in range(2):
            ot = E.sb.tile([P, D], F32, tag="otile", name="ot")
            for dc in range(2):
                pst = E.pn.tile([P, P], F32, tag="th", name="pst")
                nc.tensor.transpose(pst, out_fm[:, dc, b * S + sc * P: b * S + (sc + 1) * P],
                                    E.identf)
                nc.scalar.copy(ot[:, dc * P:(dc + 1) * P], pst)
            nc.sync.dma_start(out=io["out"][pss * PB + b, sc * P:(sc + 1) * P, :], in_=ot)
            yield


# ------------------------------------------------------------------- driver
_CACHE = {}


def _get_program(w, av):
    wshapes = []
    for k, v in sorted(w.items()):
        dt = BF16 if v.dtype.itemsize == 2 else F32
        wshapes.append((k, tuple(v.shape), dt))
    avh = hashlib.sha256(
        b"".join(np.ascontiguousarray(av[p]).tobytes() for p in ("mf", "mb"))
    ).hexdigest()
    key = (tuple(wshapes), avh)
    if key not in _CACHE:
        _CACHE[key] = build_program(wshapes, av)
    return _CACHE[key]


def kernel(**inputs):
    from concourse.bass_utils import run_bass_kernel_spmd
    w = _prep_weights(inputs)
    av = _scan_consts(inputs)
    nc = _get_program(w, av)
    x = np.ascontiguousarray(np.asarray(inputs["input_tensor"], np.float32))
    in_maps = []
    for core in range(NCORES):
        m = {"input": np.ascontiguousarray(x[core * BC:(core + 1) * BC])}
        m.update(w)
        in_maps.append(m)
    res = run_bass_kernel_spmd(nc, in_maps, list(range(NCORES)))
    return np.concatenate([res.results[i]["out"] for i in range(NCORES)], axis=0)


# revision 48
# speedup vs baseline: 2.2987x; 1.0829x over previous
"""Trainium2 Bass kernel for nn_BiMaTrLayer (dual-path filter + bidirectional
Mamba/attention stack + GLU).  Data-parallel over 8 NeuronCores (4 samples per
core, processed as 2 passes of 2 samples).

v2: bf16 matmul datapath (weights host-cast, activations evacuated bf16),
selective-scan restructured: tensor_tensor_scan runs on the (otherwise idle)
Pool engine in n-pairs over channel-fused [128, 2*DIC*F] tiles, DVE keeps only
the dtu*B and h*C muls, and the sum over states accumulates in PSUM via PE
identity matmuls (D*u enters via a host-precomputed diagonal matmul).
dA = exp(a_n * dt) uses a literal scalar scale per state (A is d-independent
in this model; baked values are part of the program cache key).
"""

import sys
import hashlib
import numpy as np

sys.path.append("/opt/trn_rl_repo")

import concourse.bass as bass
from concourse import bacc


class _Bacc(bacc.Bacc):
    """Bacc with act-table steering: resolve Exp and Ln to the combined
    natural_log_exp_and_others set so softplus/LN chains don't ping-pong
    table loads (2.7us each)."""

    def insert_act_table_loads(self):
        import concourse.mybir as _mb
        from concourse.hw_specs import get_activation_tables
        from concourse import bacc as _bacc
        has_activation = any(
            isinstance(i, _mb.InstActivation)
            for b in self.main_func.blocks
            for i in b.instructions
        )
        if not has_activation:
            return
        tables = list(get_activation_tables(self.m.arch).items())
        AFT = _mb.ActivationFunctionType
        steer = {"exp_and_others": {AFT.Exp}, "exp_and_friends": {AFT.Exp},
                 "natural_log": {AFT.Ln}}
        tables = [(nm, fn - steer.get(nm, set())) for nm, fn in tables]
        _bacc._bass_rust.insert_act_table_loads(self, tables)

import concourse.mybir as mybir
import concourse.tile as tile
from concourse.masks import make_identity
from contextlib import ExitStack

AF = mybir.ActivationFunctionType
OP = mybir.AluOpType
F32 = mybir.dt.float32
BF16 = mybir.dt.bfloat16
P = 128

B, S, D = 32, 256, 256
NCORES = 8
BC = B // NCORES            # samples per core
PB = 2                      # samples per pass
NPASS = BC // PB
F = PB * S                  # 512: free dim (sample, time) per pass
DI, DS, DTR, NL, H, HD = 512, 16, 16, 2, 4, 64
DIC = DI // P
L2 = 69
NF = S // 2 + 1
DC = 4

DEC_LO = np.array([-0.010597401784997278, 0.032883011666982945,
                   0.030841381835986965, -0.18703481171888114,
                   -0.02798376941698385, 0.6308807679295904,
                   0.7148465705525415, 0.23037781330885523], np.float64)


def _bf16(a):
    import ml_dtypes
    return np.ascontiguousarray(np.asarray(a, np.float32).astype(ml_dtypes.bfloat16))


def _f32(a):
    return np.ascontiguousarray(np.asarray(a), np.float32)


# ----------------------------------------------------------------- host consts
def _dwt1_mat(L):
    out_full = L + 14 - 8 + 1
    idx = np.arange(1, out_full, 2)
    M = np.zeros((len(idx), L))
    for s in range(L):
        x = np.zeros(L)
        x[s] = 1.0
        y = np.correlate(np.pad(x, 7), DEC_LO[::-1], 'valid')
        M[:, s] = y[idx]
    return M


def _interp_mat(Lin, out_len):
    pos = (np.arange(out_len) + 0.5) * (Lin / out_len) - 0.5
    pos = np.clip(pos, 0.0, Lin - 1.0)
    lo = np.floor(pos).astype(int)
    hi = np.minimum(lo + 1, Lin - 1)
    t = pos - lo
    M = np.zeros((out_len, Lin))
    M[np.arange(out_len), lo] += 1.0 - t
    M[np.arange(out_len), hi] += t
    return M


def _fft_mats():
    s = np.arange(S)
    f = np.arange(NF)
    ang = 2 * np.pi * np.outer(f, s) / S
    Fr = np.cos(ang) / np.sqrt(S)
    Fi = -np.sin(ang) / np.sqrt(S)
    c = np.full(NF, 2.0)
    c[0] = 1.0
    c[-1] = 1.0
    angT = 2 * np.pi * np.outer(s, f) / S
    Gr = c * np.cos(angT) / np.sqrt(S)
    Gi = -c * np.sin(angT) / np.sqrt(S)
    Gi[:, 0] = 0.0
    Gi[:, -1] = 0.0
    return Fr, Fi, Gr, Gi


def _host_consts():
    Fr, Fi, Gr, Gi = _fft_mats()
    D1 = _dwt1_mat(S)
    D2 = _dwt1_mat(D1.shape[0])
    T = D2 @ D1
    I = _interp_mat(T.shape[0], S)
    return dict(frT=_bf16(Fr.T), fiT=_bf16(Fi.T), grT=_bf16(Gr.T),
                giT=_bf16(Gi.T), tdT=_bf16(T.T), iiT=_bf16(I.T))


def _prep_weights(inp):
    w = dict(_host_consts())
    w["fftWa"] = _bf16(np.concatenate([_f32(inp["fft_W"]).T,
                                       _f32(inp["fft_b"])[None, :]], 0))
    for nm in ("wl1", "wl2"):
        w[nm + "T"] = _bf16(_f32(inp[nm + "_W"]).transpose(2, 1, 0))
        w[nm + "b"] = _f32(np.asarray(inp[nm + "_b"])[:, None])
    qkv = _f32(inp["ca_Wqkv"])
    bqkv = _f32(inp["ca_bqkv"])
    wo = _f32(inp["ca_Wo"])
    w["caWqT"] = _bf16(qkv[0:D].T)
    w["caWkT"] = _bf16(qkv[D:2 * D].T)
    w["caWvT"] = _bf16(qkv[2 * D:].T)
    w["caWoT"] = _bf16(wo.T)
    w["caBq"] = _f32(bqkv[0:D][:, None])
    w["caBk"] = _f32(bqkv[D:2 * D][:, None])
    w["caBo"] = _f32((_f32(inp["ca_bo"]) + wo @ bqkv[2 * D:])[:, None])
    w["gateWT"] = _bf16(_f32(inp["gate_W"]).T)
    w["gateB"] = _f32(np.asarray(inp["gate_b"])[:, None])
    pidx = np.arange(P)
    for pre in ("mf", "mb"):
        w[pre + "inWT"] = _bf16(_f32(inp[pre + "_in_W"]).transpose(0, 2, 1))
        cw = _f32(inp[pre + "_conv_W"])          # [NL, DI, DC]
        cd = np.zeros((NL, P, DIC, DC, P), np.float32)
        cd[:, pidx, :, :, pidx] = cw.reshape(NL, DIC, P, DC).transpose(
            2, 0, 1, 3)                           # -> [P, NL, DIC, DC]
        w[pre + "convD"] = _bf16(cd)
        dv = _f32(inp[pre + "_D"])                # [NL, DI]
        dd = np.zeros((NL, P, DIC, P), np.float32)
        dd[:, pidx, :, pidx] = dv.reshape(NL, DIC, P).transpose(2, 0, 1)
        w[pre + "diagD"] = _bf16(dd)
        w[pre + "cols"] = _f32(np.stack([_f32(inp[pre + "_conv_b"]),
                                         _f32(inp[pre + "_dt_b"]),
                                         dv], -1))
        xp = _f32(inp[pre + "_xproj_W"]).transpose(0, 2, 1)   # [NL, DI, 48]
        perm = list(range(DTR, DTR + 2 * DS)) + list(range(DTR))  # [B;C;dt]
        w[pre + "xpT"] = _bf16(xp[:, :, perm])
        w[pre + "dtWT"] = _bf16(_f32(inp[pre + "_dt_W"]).transpose(0, 2, 1))
        w[pre + "outWT"] = _bf16(_f32(inp[pre + "_out_W"]).transpose(0, 2, 1))
    for pre in ("af", "ab"):
        qkv = _f32(inp[pre + "_Wqkv"])
        bqkv = _f32(inp[pre + "_bqkv"])
        wo = _f32(inp[pre + "_Wo"])
        w[pre + "WqT"] = _bf16(qkv[:, 0:D].transpose(0, 2, 1))
        w[pre + "WkT"] = _bf16(qkv[:, D:2 * D].transpose(0, 2, 1))
        w[pre + "WvT"] = _bf16(qkv[:, 2 * D:].transpose(0, 2, 1))
        w[pre + "WoT"] = _bf16(wo.transpose(0, 2, 1))
        w[pre + "Bq"] = _f32(bqkv[:, 0:D][:, :, None])
        w[pre + "Bk"] = _f32(bqkv[:, D:2 * D][:, :, None])
        w[pre + "Bo"] = _f32((_f32(inp[pre + "_bo"])
                              + np.einsum('lod,ld->lo', wo, bqkv[:, 2 * D:]))[:, :, None])
    fgrows = []
    for g, b in (("fl_ln_g", "fl_ln_b"), ("glu_ln_g", "glu_ln_b")):
        fgrows.append(np.stack([_f32(inp[g]), _f32(inp[b])], 0)[None])
    w["lnFG"] = _bf16(np.concatenate(fgrows, 0))         # [2, 2, D]
    rows = []
    for nm in ("anf", "anb", "nf", "nb"):
        rows += [_f32(inp[nm + "_g"])[:, None, :],
                 _f32(inp[nm + "_b"])[:, None, :]]
    w["lnAll"] = _bf16(np.concatenate(rows, 1))          # [NL, 8, D]
    w["glu1WT"] = _bf16(_f32(inp["glu1_W"]).T)
    w["glu1B"] = _f32(np.asarray(inp["glu1_b"])[:, None])
    w["glu2WT"] = _bf16(_f32(inp["glu2_W"]).T)
    w["glu2B"] = _f32(np.asarray(inp["glu2_b"])[:, None])
    return w


def _scan_consts(inp):
    """Per-(dir, layer, state) decay scales a_n = -exp(Alog); the model's Alog
    is d-independent, verified here; baked into the emitted program (cache-
    keyed on the values)."""
    av = {}
    for pre in ("mf", "mb"):
        al = _f32(inp[pre + "_Alog"])            # [NL, DI, DS]
        a = -np.exp(al.astype(np.float64))
        med = np.median(a, axis=1)               # [NL, DS]
        assert np.abs(a - med[:, None, :]).max() < 1e-5 * np.abs(med).max(), \
            "Alog is d-dependent; scalar-scale dA path invalid"
        av[pre] = med
    return av


# ----------------------------------------------------------------- emit helpers
class Emit:
    def __init__(self, nc, tc, ctx):
        self.nc, self.tc = nc, tc
        self.sb = ctx.enter_context(tc.tile_pool(name="sb", bufs=1))
        self.s2p = ctx.enter_context(tc.tile_pool(name="s2p", bufs=2))
        self.s3p = ctx.enter_context(tc.tile_pool(name="s3p", bufs=2))
        self.pp = ctx.enter_context(tc.tile_pool(name="pp", bufs=2, space="PSUM"))
        self.pn = ctx.enter_context(tc.tile_pool(name="pn", bufs=2, space="PSUM"))
        self.pyac = ctx.enter_context(tc.tile_pool(name="pyac", bufs=1, space="PSUM"))

    def load_wT(self, drh, K, M, tag):
        nc = self.nc
        if not isinstance(drh, bass.AP):
            drh = drh[:, :]
        kc_n = (K + P - 1) // P
        t = self.sb.tile([min(K, P), kc_n, M], BF16, tag=tag, name="wT")
        if K % P == 0:
            st = drh.ap[-1][0]
            src = bass.AP(tensor=drh.tensor, offset=drh.offset,
                          ap=[[M * st, P], [P * M * st, kc_n], [st, M]])
            nc.sync.dma_start(out=t, in_=src)
        else:
            for kc in range(kc_n):
                kp = min(P, K - kc * P)
                nc.sync.dma_start(out=t[:kp, kc, :], in_=drh[kc * P:kc * P + kp, :])
        return t

    def load_col(self, drh, M, tag):
        nc = self.nc
        if not isinstance(drh, bass.AP):
            drh = drh[:, :]
        mc_n = (M + P - 1) // P
        t = self.sb.tile([P, mc_n], F32, tag=tag, name="col")
        if M % P == 0:
            src = bass.AP(tensor=drh.tensor, offset=drh.offset,
                          ap=[[1, P], [P, mc_n]])
            nc.sync.dma_start(out=t, in_=src)
        else:
            for mc in range(mc_n):
                mp = min(P, M - mc * P)
                nc.sync.dma_start(out=t[:mp, mc:mc + 1],
                                  in_=drh[mc * P:mc * P + mp, :])
        return t

    def dense(self, x, wT, Mout, bias=None, act=None, out=None, out_pool=None,
              out_tag=None, Fw=None, out_dt=BF16):
        nc = self.nc
        Fw = Fw or F
        kc_n = x.shape[1]
        mc_n = (Mout + P - 1) // P
        if out is None:
            out = (out_pool or self.s3p).tile([P, mc_n, Fw], out_dt,
                                              tag=out_tag, name="dn")
        for mc in range(mc_n):
            mp = min(P, Mout - mc * P)
            ps = self.pp.tile([P, 512], F32, tag="mm", name="ps")
            for kc in range(kc_n):
                nc.tensor.matmul(ps[:mp, :Fw],
                                 wT[:, kc, mc * P:mc * P + mp],
                                 x[:, kc, 0:Fw],
                                 start=(kc == 0), stop=(kc == kc_n - 1))
            bap = bias[:mp, mc:mc + 1] if bias is not None else None
            if act is None and bias is None:
                nc.scalar.copy(out[:mp, mc, 0:Fw], ps[:mp, :Fw])
            else:
                nc.scalar.activation(out[:mp, mc, 0:Fw], ps[:mp, :Fw],
                                     act or AF.Identity,
                                     bias=bap if bap is not None else 0.0,
                                     scale=1.0)
        return out

    def add(self, out, a, b):
        self.nc.vector.tensor_add(out, a, b)

    def mul(self, out, a, b):
        self.nc.vector.tensor_mul(out, a, b)

    def act(self, out, in_, func, bias=0.0, scale=1.0):
        self.nc.scalar.activation(out=out, in_=in_, func=func, bias=bias, scale=scale)


def rev_view(ap2, n_blk, blk):
    st = ap2.ap[-1][0]
    off = ap2.offset + (blk - 1) * st
    if n_blk == 1:
        return bass.AP(tensor=ap2.tensor, offset=off, ap=[ap2.ap[0], [-st, blk]])
    return bass.AP(tensor=ap2.tensor, offset=off,
                   ap=[ap2.ap[0], [blk * st, n_blk], [-st, blk]])


def _g_layer_norm(E, x, gR, bR, eps, out, x_is_f32=False, tag=""):
    """x, out: [128, 2, F] feature-major (D=256 on partitions). gR/bR bf16
    rows [1, D].  Generator: yields at chunk boundaries."""
    nc = E.nc
    fw = F
    stat = E.sb.tile([1, 2, 512], F32, tag="lnstat" + tag, name="stat")
    A = stat[0:1, 0, :fw]          # m, later m*r
    Bv = stat[0:1, 1, :fw]         # q, later var, later r
    mrb = E.sb.tile([1, 2, 512], BF16, tag="lnthinb" + tag, name="mrb")
    xsq = E.sb.tile([P, 2, 512], BF16, tag="xsq", name="xsq")
    E.act(xsq, x, AF.Square)
    ones = E.ones128f if x_is_f32 else E.ones128
    for which, dst in ((0, A), (1, Bv)):
        ps = E.pn.tile([P, 512], F32, tag="th", name="ps")
        for kc in range(2):
            if which == 0:
                nc.tensor.matmul(ps[0:1, :fw], ones, x[:, kc, 0:fw],
                                 start=(kc == 0), stop=(kc == 1))
            else:
                nc.tensor.matmul(ps[0:1, :fw], E.ones128, xsq[:, kc, 0:fw],
                                 start=(kc == 0), stop=(kc == 1))
        nc.vector.tensor_scalar_mul(dst, ps[0:1, :fw], 1.0 / D)
        yield
    E.act(mrb[0:1, 0, :fw], A, AF.Square)          # m^2 (bf16 scratch)
    nc.vector.tensor_tensor(Bv, Bv, mrb[0:1, 0, :fw], OP.subtract)
    E.act(Bv, Bv, AF.Ln, bias=E.eps[eps][0:1, 0:1])
    E.act(Bv, Bv, AF.Exp, scale=-0.5)              # r (f32)
    nc.vector.tensor_copy(mrb[0:1, 0, :fw], Bv)    # r (bf16)
    E.mul(A, A, Bv)                                # m*r (f32, in place)
    E.act(mrb[0:1, 1, :fw], A, AF.Identity, scale=-1.0)   # -m*r (bf16)
    yield
    for mc in range(2):
        gRc = gR[0:1, mc * P:(mc + 1) * P]
        bRc = bR[0:1, mc * P:(mc + 1) * P]
        ps_s = E.pn.tile([P, 512], F32, tag="th", name="ps_s")
        nc.tensor.matmul(ps_s[:, :fw], gRc, mrb[0:1, 0, :fw], start=True, stop=True)
        ps_o = E.pn.tile([P, 512], F32, tag="th", name="ps_o")
        nc.tensor.matmul(ps_o[:, :fw], bRc, E.onesF[0:1, :fw],
                         start=True, stop=False)
        nc.tensor.matmul(ps_o[:, :fw], gRc, mrb[0:1, 1, :fw], start=False, stop=True)
        tmp = E.s2p.tile([P, 512], BF16, tag="lntmp", name="tmp", bufs=1)
        E.mul(tmp[:, :fw], x[:, mc, 0:fw], ps_s[:, :fw])
        E.add(out[:, mc, 0:fw], tmp[:, :fw], ps_o[:, :fw])
        yield


def _g_attention(E, q_src, kv_src, wq, wk, wv, wo, bq, bk, bo, out_tag, ob, okey):
    """MHA over PB samples; q_src/kv_src [128, 2, F] fm bf16.  Generator;
    result tile into ob[okey]."""
    nc = E.nc
    ofm = E.s3p.tile([P, 2, F], BF16, tag="t8", name="ofm")
    se = E.sb.tile([1, H, PB, S], BF16, tag="thin8", name="se")
    qf = E.s2p.tile([P, 2, F], BF16, tag="qfb", name="qf", bufs=1)
    kf = E.s2p.tile([P, 2, F], BF16, tag="kfb", name="kf", bufs=1)
    for mc in range(2):
        for dst, wT, bias, srcT in ((qf, wq, bq, q_src), (kf, wk, bk, kv_src)):
            ps = E.pp.tile([P, 512], F32, tag="mm", name="ps")
            for kc in range(2):
                nc.tensor.matmul(ps[:, :F], wT[:, kc, mc * P:(mc + 1) * P],
                                 srcT[:, kc, :], start=(kc == 0), stop=(kc == 1))
            nc.scalar.activation(dst[:, mc, :], ps[:, :F], AF.Identity,
                                 bias=bias[:, mc:mc + 1], scale=1.0)
            yield
    for b in range(PB):
        vtm = E.s2p.tile([P, 2, D], BF16, tag="vtmb", name="vtm", bufs=1)
        ps = E.pp.tile([P, 512], F32, tag="mm", name="ps")
        for tcn in range(2):
            for kc in range(2):
                nc.tensor.matmul(ps[:, tcn * D:(tcn + 1) * D],
                                 kv_src[:, kc, b * S + tcn * P: b * S + (tcn + 1) * P],
                                 wv[:, kc, :], start=(kc == 0), stop=(kc == 1))
        nc.scalar.copy(vtm[:, :, :].rearrange("p a d -> p (a d)"), ps[:, :])
        yield
        pse = None
        for h in range(H):
            hc, off = h // 2, (h % 2) * 64
            expT = E.s2p.tile([P, 2, S], BF16, tag="expT", name="expT", bufs=1)
            ps = E.pp.tile([P, 512], F32, tag="mm", name="ps")
            for kc in range(2):
                nc.tensor.matmul(ps[:, kc * S:(kc + 1) * S],
                                 kf[off:off + 64, hc, b * S + kc * P:b * S + (kc + 1) * P],
                                 qf[off:off + 64, hc, b * S:(b + 1) * S],
                                 start=True, stop=True)
            E.act(expT, ps, AF.Exp, scale=1.0 / np.sqrt(HD))
            if h % 2 == 0:
                pse = E.pn.tile([P, 512], F32, tag="th", name="pse")
            for kc in range(2):
                nc.tensor.matmul(pse[0:1, (h % 2) * S:(h % 2) * S + S],
                                 E.ones128, expT[:, kc, :],
                                 start=(kc == 0), stop=(kc == 1))
            if h % 2 == 1:
                E.act(se[0:1, h - 1:h + 1, b, :],
                      pse[0:1, :].rearrange("p (h s) -> p h s", h=2), AF.Ln)
            if h % 2 == 0:
                psav = E.pp.tile([P, 512], F32, tag="mm", name="psav")
            for kc in range(2):
                nc.tensor.matmul(psav[off:off + 64, :S],
                                 vtm[:, kc, h * 64:(h + 1) * 64],
                                 expT[:, kc, :], start=(kc == 0), stop=(kc == 1))
            if h % 2 == 1:
                nc.scalar.copy(ofm[:, hc, b * S:(b + 1) * S], psav[:, :S])
            yield
    E.act(se, se, AF.Exp, scale=-1.0)              # 1/sumexp, in place
    yield
    for h in range(H):
        dc, off = h // 2, (h % 2) * 64
        ps = E.pn.tile([P, 512], F32, tag="th", name="ps")
        nc.tensor.matmul(ps[0:64, :F], E.ones1x64,
                         se[0:1, h].rearrange("p b s -> p (b s)"),
                         start=True, stop=True)
        E.mul(ofm[off:off + 64, dc, :], ofm[off:off + 64, dc, :], ps[0:64, :F])
        yield
    ob[okey] = E.dense(ofm, wo, D, bias=bo, out_tag=out_tag)


def _g_mamba_prep_a(E, io, x, pre, l, flip, pr):
    """Silu-table phase: weights, in-proj xi, conv via host diag mats, z."""
    nc = E.nc
    d = pre
    inW = E.load_wT(io[pre + "inWT"][l], D, 2 * DI, "inW")
    cols = E.sb.tile([P, DIC, 3], F32, tag="mcols" + d, name="cols")
    cd = io[pre + "cols"][l]
    nc.sync.dma_start(out=cols, in_=bass.AP(
        tensor=cd.tensor, offset=cd.offset, ap=[[3, P], [P * 3, DIC], [1, 3]]))
    convD = E.sb.tile([P, DIC, DC, P], BF16, tag="convD", name="convD")
    nc.sync.dma_start(out=convD, in_=io[pre + "convD"][l])
    diagD = E.sb.tile([P, DIC, P], BF16, tag="diagD" + d, name="diagD")
    nc.sync.dma_start(out=diagD, in_=io[pre + "diagD"][l])
    xpw = E.load_wT(io[pre + "xpT"][l], DI, DTR + 2 * DS, "xpw" + d)
    dtw = E.sb.tile([2 * DS + DTR, DI], BF16, tag="dtw" + d, name="dtw")
    nc.sync.dma_start(out=dtw[2 * DS:, :], in_=io[pre + "dtWT"][l])
    ow = E.load_wT(io[pre + "outWT"][l], DI, D, "outW" + d)
    yield

    def inproj(c0, dst_tag, silu):
        dst = E.sb.tile([P, DIC, F], BF16, tag=dst_tag, name="xi")
        for c in range(DIC):
            ps = E.pp.tile([P, 512], F32, tag="mm", name="ps")
            for b in range(PB):
                for kc in range(2):
                    rhs = x[:, kc, b * S:(b + 1) * S]
                    if flip:
                        rhs = rev_view(rhs, 1, S)
                    nc.tensor.matmul(ps[:, b * S:(b + 1) * S],
                                     inW[:, kc, (c0 + c) * P:(c0 + c + 1) * P], rhs,
                                     start=(kc == 0), stop=(kc == 1))
            if silu:
                E.act(dst[:, c, :], ps, AF.Silu)
            else:
                nc.scalar.copy(dst[:, c, :], ps)
        return dst

    xi = inproj(0, "xi", False)
    yield
    xc = E.s2p.tile([P, DIC, F], BF16, tag="xc", name="xc")
    for c in range(DIC):
        ps = E.pp.tile([P, 512], F32, tag="mm", name="ps")
        for b in range(PB):
            nc.tensor.matmul(ps[:, b * S:(b + 1) * S], convD[:, c, DC - 1, :],
                             xi[:, c, b * S:(b + 1) * S], start=True, stop=False)
            for j in range(DC - 1):
                sh = DC - 1 - j
                nc.tensor.matmul(ps[:, b * S + sh:(b + 1) * S], convD[:, c, j, :],
                                 xi[:, c, b * S:(b + 1) * S - sh],
                                 start=False, stop=(j == DC - 2))
        E.act(xc[:, c, :], ps, AF.Silu, bias=cols[:, c, 0:1])
        yield
    z = inproj(DIC, "z" + d, True)
    yield
    pr.update(xc=xc, z=z, diagD=diagD, ow=ow, xpw=xpw, dtw=dtw, cols=cols)


def _g_mamba_prep_b(E, io, pr, pre, l, bcd):
    """NLE-table phase: xproj -> dbl (+DRAM bounce), dt softplus, dtu."""
    nc = E.nc
    d = pre
    xc, xpw, dtw, cols = pr["xc"], pr["xpw"], pr["dtw"], pr["cols"]
    dbl = E.sb.tile([DTR + 2 * DS, F], BF16, tag="dbl" + d, name="dbl")
    ps = E.pp.tile([P, 512], F32, tag="mm", name="ps")
    for kc in range(DIC):
        nc.tensor.matmul(ps[:DTR + 2 * DS, :F], xpw[:, kc, :], xc[:, kc, :],
                         start=(kc == 0), stop=(kc == DIC - 1))
    nc.scalar.copy(dbl, ps[:DTR + 2 * DS, :F])
    nc.sync.dma_start(out=bcd[:, :], in_=dbl[0:2 * DS, :])
    yield
    dt = E.sb.tile([P, DIC, F], BF16, tag="dt" + d, name="dt")
    for mc in range(DIC):
        ps = E.pp.tile([P, 512], F32, tag="mm", name="ps")
        nc.tensor.matmul(ps[:, :F], dtw[2 * DS:, mc * P:(mc + 1) * P],
                         dbl[2 * DS:2 * DS + DTR, :], start=True, stop=True)
        dtx = E.sb.tile([P, F], BF16, tag="dtx", name="dtx")
        E.act(dtx, ps[:, :F], AF.Exp, bias=cols[:, mc, 1:2])
        E.act(dt[:, mc, :], dtx, AF.Ln, bias=1.0)
        yield
    dtu = E.sb.tile([P, DIC, F], BF16, tag="dtu" + d, name="dtu")
    E.mul(dtu, dt, xc)
    nc.vector.memset(dt[:, :, 0:F:S], 1.0e30)
    pr.update(dt=dt, dtu=dtu, bcd=bcd)


def _mamba_scan(E, pr, avl, out_tag, bg, pump):
    """Per-state scan loop: dA on ACT (literal scalar scale), scans + h*C
    muls on DVE (bf16), dtu*B muls mostly on Pool, Sum_n C*h_n (+ D*u)
    accumulated in PSUM by PE identity matmuls; y*silu(z) reads the PSUM
    accumulators directly on DVE.  bg is pumped each iteration."""
    nc = E.nc
    dt, dtu, z = pr["dt"], pr["dtu"], pr["z"]
    yac = [E.pyac.tile([P, 512], F32, tag=f"yac{c}", name="yac") for c in range(DIC)]
    for c in range(DIC):
        nc.tensor.matmul(yac[c], pr["diagD"][:, c, :], pr["xc"][:, c, :],
                         start=True, stop=False, skip_group_check=True)
    dt2 = dt[:, :, :].rearrange("p c f -> p (c f)")
    bcd = pr["bcd"][:, :]
    for n in range(DS):
        bg.pump(pump)
        dA = E.s2p.tile([P, DIC * F], BF16, tag="dA", name="dA")
        E.act(dA, dt2, AF.Exp, scale=float(avl[n]))
        bc = E.s2p.tile([P, 2, F], BF16, tag="bc", name="bc", bufs=3)
        src = bass.AP(tensor=bcd.tensor, offset=bcd.offset + n * F,
                      ap=[[0, P], [DS * F, 2], [1, F]])
        nc.sync.dma_start(out=bc, in_=src)
        dBu = E.s2p.tile([P, DIC, F], BF16, tag="dBu", name="dBu", bufs=3)
        bview = bass.AP(tensor=bc.tensor, offset=bc.offset,
                        ap=[bc.ap[0], [0, DIC], [1, F]])
        # the dtu*B broadcast mul rides the otherwise-idle Pool engine for
        # most states (Multiply has a real gpsimd impl; the scan does not)
        meng = nc.vector if n in (0, 4, 8, 12) else nc.gpsimd
        meng.tensor_mul(dBu, dtu, bview)
        hn = E.s2p.tile([P, DIC, F], BF16, tag="hn", name="hn", bufs=3)
        nc.vector.tensor_tensor_scan(
            out=hn[:, :, :].rearrange("p c f -> p (c f)"),
            data0=dA[:, :],
            data1=dBu[:, :, :].rearrange("p c f -> p (c f)"),
            initial=0.0, op0=OP.mult, op1=OP.add)
        hnC = E.sb.tile([P, DIC, F], BF16, tag="hnC", name="hnC")
        cview = bass.AP(tensor=bc.tensor, offset=bc.offset + F,
                        ap=[bc.ap[0], [0, DIC], [1, F]])
        E.mul(hnC, hn, cview)
        last = (n == DS - 1)
        for c in range(DIC):
            nc.tensor.matmul(yac[c], E.identb, hnC[:, c, :],
                             start=False, stop=last,
                             skip_group_check=True)
    bg.pump(2)
    y = dtu            # dtu is dead after the last dBu; reuse its buffer
    for c in range(DIC):
        E.mul(y[:, c, :], z[:, c, :], yac[c])
    return E.dense(y, pr["ow"], D, out_pool=E.s2p, out_tag=out_tag)


# ------------------------------------------------------------------- program
def build_program(wshapes, av):
    nc = _Bacc()
    io = {}
    io["input"] = nc.declare_dram_parameter("input", [BC, S, D], F32, isOutput=False)
    for k, shp, dt in wshapes:
        io[k] = nc.declare_dram_parameter(k, list(shp), dt, isOutput=False)
    io["out"] = nc.declare_dram_parameter("out", [BC, S, D], F32, isOutput=True)
    for pss in range(NPASS):
        for l in range(NL):
            for pre in ("mf", "mb"):
                io[f"bcrows_{pss}_{l}_{pre}"] = nc.dram_tensor(
                    f"bcrows_{pss}_{l}_{pre}", [2 * DS, F], BF16)
    with tile.TileContext(nc) as tc:
        with ExitStack() as ctx:
            E = Emit(nc, tc, ctx)
            identb = E.sb.tile([P, P], BF16, tag="identb", name="identb")
            make_identity(nc, identb)
            E.identb = identb
            identf = E.sb.tile([P, P], F32, tag="identf", name="identf")
            make_identity(nc, identf)
            E.identf = identf
            E.ones128 = E.sb.tile([P, 1], BF16, tag="ones128", name="ones128")
            nc.vector.memset(E.ones128, 1.0)
            E.ones128f = E.sb.tile([P, 1], F32, tag="ones128f", name="ones128f")
            nc.vector.memset(E.ones128f, 1.0)
            E.ones1x64 = E.sb.tile([1, 64], BF16, tag="ones64", name="ones64")
            nc.vector.memset(E.ones1x64, 1.0)
            E.ones1xP = E.sb.tile([1, P], BF16, tag="ones1p", name="ones1p")
            nc.vector.memset(E.ones1xP, 1.0)
            E.onesF = E.sb.tile([1, 512], BF16, tag="onesF", name="onesF")
            nc.vector.memset(E.onesF, 1.0)
            E.eps = {}
            for ev in (1e-5, 1e-12):
                t = E.sb.tile([1, 1], F32, tag=f"eps{ev}", name="eps")
                nc.vector.memset(t, ev)
                E.eps[ev] = t
            # software-pipelined pass interleave: pass-1's FFT/wavelet/gate
            # stage and layer preps are emitted inside pass-0's scan windows
            # so the Pool engine (scans) never drains.
            bg = _BG()
            box = {}
            c00, c10, c01, c11 = {}, {}, {}, {}
            _run(_g_stage03(E, io, 0, box, "x1a"))
            _run(_g_layer_preps(E, io, 0, 0, lambda: box["x1a"], av, c00))
            bg.add(_chain(
                _g_stage03(E, io, 1, box, "x1b"),
                _g_layer_preps(E, io, 1, 0, lambda: box["x1b"], av, c10)))
            _emit_layer_scans(E, c00, av, bg)
            bg.drain()
            bg.add(_chain(
                _g_layer_post(E, c00, box, "x1a"),
                _g_layer_preps(E, io, 0, 1, lambda: box["x1a"], av, c01)))
            _emit_layer_scans(E, c10, av, bg)
            bg.drain()
            bg.add(_chain(
                _g_layer_post(E, c10, box, "x1b"),
                _g_layer_preps(E, io, 1, 1, lambda: box["x1b"], av, c11)))
            _emit_layer_scans(E, c01, av, bg)
            bg.drain()
            bg.add(_chain(
                _g_layer_post(E, c01, box, "x1a"),
                _g_glu(E, io, 0, lambda: box["x1a"])))
            _emit_layer_scans(E, c11, av, bg,
                              mid_add=_chain(_g_post_attn(E, c11, "mf", "af"),
                                             _g_post_lnt(E, c11)))
            bg.drain()
            _run(_g_layer_post(E, c11, box, "x1b"))
            _run(_g_glu(E, io, 1, lambda: box["x1b"]))
    nc.finalize()
    return nc


class _BG:
    def __init__(self):
        from collections import deque
        self.q = deque()

    def add(self, gen):
        self.q.append(gen)

    def pump(self, n=1):
        while n > 0 and self.q:
            try:
                next(self.q[0])
                n -= 1
            except StopIteration:
                self.q.popleft()

    def drain(self):
        while self.q:
            self.pump(64)


def _run(gen):
    for _ in gen:
        pass


def _chain(*gens):
    for g in gens:
        yield from g


def _g_stage03(E, io, pss, box, key):
    nc = E.nc
    # ---------------- stage 0: load x + cast + transpose to feature-major
    x_tm = E.sb.tile([P, PB * 2, D], BF16, tag="xtm", name="x_tm")
    for b in range(PB):
        for sc in range(2):
            xch = E.s2p.tile([P, D], F32, tag="xt32", name="xch")
            nc.sync.dma_start(out=xch,
                              in_=io["input"][pss * PB + b, sc * P:(sc + 1) * P, :])
            nc.vector.tensor_copy(x_tm[:, b * 2 + sc, :], xch)
    yield
    x_fm = E.sb.tile([P, 2, F], BF16, tag="xfm", name="x_fm")
    for b in range(PB):
        for sc in range(2):
            for dc in range(2):
                pst = E.pn.tile([P, P], BF16, tag="th", name="pst")
                nc.tensor.transpose(pst, x_tm[:, b * 2 + sc, dc * P:(dc + 1) * P],
                                    E.identb)
                nc.scalar.copy(x_fm[:, dc, b * S + sc * P: b * S + (sc + 1) * P], pst)
            yield

    # ---------------- stage 1: FFT path
    frT = E.load_wT(io["frT"], S, NF, "frT")
    fiT = E.load_wT(io["fiT"], S, NF, "fiT")
    fftWa = E.load_wT(io["fftWa"], 513, 2 * D, "fftWa")
    grT = E.load_wT(io["grT"], NF, S, "grT")
    giT = E.load_wT(io["giT"], NF, S, "giT")
    yield
    x_fft = E.sb.tile([P, 2, F], BF16, tag="xfft", name="x_fft")
    for b in range(PB):
        comb = E.s3p.tile([P, 4, NF], BF16, tag="t8", name="comb")
        for ri, mat in ((0, frT), (1, fiT)):
            for mc in range(2):
                ps = E.pp.tile([P, 512], F32, tag="mm", name="ps")
                for kc in range(2):
                    nc.tensor.matmul(ps[:, :NF], x_tm[:, b * 2 + kc, mc * P:(mc + 1) * P],
                                     mat[:, kc, :], start=(kc == 0), stop=(kc == 1))
                nc.scalar.copy(comb[:, ri * 2 + mc, :], ps[:, :NF])
                yield
        filt = E.s3p.tile([P, 2 * D], BF16, tag="t8", name="filt")
        filtN = E.sb.tile([1, 2 * D], BF16, tag="filtN", name="filtN")
        for mt, mp, f0 in ((filt, P, 0), (filtN, 1, P)):
            ps = E.pp.tile([P, 512], F32, tag="mm", name="ps")
            for kc in range(4):
                nc.tensor.matmul(ps[:mp, :], comb[:, kc, f0:f0 + mp], fftWa[:, kc, :],
                                 start=(kc == 0), stop=False)
            nc.tensor.matmul(ps[:mp, :], E.ones1xP[0:1, 0:mp], fftWa[0:1, 4, :],
                             start=False, stop=True)
            E.act(mt[0:mp, :] if mt is filtN else mt, ps[:mp, :], AF.Gelu)
            yield
        for mc in range(2):
            ps = E.pp.tile([P, 512], F32, tag="mm", name="ps")
            nc.tensor.matmul(ps[:, :S], filt[:, mc * P:(mc + 1) * P], grT[:, 0, :],
                             start=True, stop=False)
            nc.tensor.matmul(ps[:, :S], filtN[0:1, mc * P:(mc + 1) * P], grT[0:1, 1, :],
                             start=False, stop=False)
            nc.tensor.matmul(ps[:, :S], filt[:, D + mc * P:D + (mc + 1) * P], giT[:, 0, :],
                             start=False, stop=False)
            nc.tensor.matmul(ps[:, :S], filtN[0:1, D + mc * P:D + (mc + 1) * P],
                             giT[0:1, 1, :], start=False, stop=True)
            nc.scalar.copy(x_fft[:, mc, b * S:(b + 1) * S], ps[:, :S])
            yield

    # ---------------- stage 2: wavelet path
    tdT = E.load_wT(io["tdT"], S, L2, "tdT")
    iiT = E.sb.tile([L2, S], BF16, tag="iiT", name="iiT")
    nc.sync.dma_start(out=iiT, in_=io["iiT"][:, :])
    wl1T = [E.load_wT(io["wl1T"][k], D, D, t) for k, t in enumerate(("awq", "awk", "awv"))]
    wl2T = [E.load_wT(io["wl2T"][k], D, D, t) for k, t in enumerate(("awo", "wlo1", "wlo2"))]
    wl1b = E.load_col(io["wl1b"], D, "wl1b")
    wl2b = E.load_col(io["wl2b"], D, "wl2b")
    yield
    x_wl = E.sb.tile([P, 2, F], BF16, tag="xwl", name="x_wl")
    a_fm = E.sb.tile([P, 2, PB, L2], BF16, tag="afm", name="a_fm")
    for b in range(PB):
        for mc in range(2):
            ps = E.pp.tile([P, 512], F32, tag="mm", name="ps")
            for kc in range(2):
                nc.tensor.matmul(ps[:, :L2], x_tm[:, b * 2 + kc, mc * P:(mc + 1) * P],
                                 tdT[:, kc, :], start=(kc == 0), stop=(kc == 1))
            nc.scalar.copy(a_fm[:, mc, b, :], ps[:, :L2])
    yield

    def conv3(src, wT, bcol, actf, dst_tag):
        dst = E.s2p.tile([P, 2, PB, L2], BF16, tag=dst_tag, name="c3")
        for b in range(PB):
            for mc in range(2):
                ps = E.pp.tile([P, 512], F32, tag="mm", name="ps")
                for kc in range(2):
                    nc.tensor.matmul(ps[:, :L2], wT[1][:, kc, mc * P:(mc + 1) * P],
                                     src[:, kc, b, :], start=(kc == 0), stop=False)
                for kc in range(2):
                    nc.tensor.matmul(ps[:, 1:L2], wT[0][:, kc, mc * P:(mc + 1) * P],
                                     src[:, kc, b, 0:L2 - 1], start=False, stop=False)
                for kc in range(2):
                    nc.tensor.matmul(ps[:, 0:L2 - 1], wT[2][:, kc, mc * P:(mc + 1) * P],
                                     src[:, kc, b, 1:L2], start=False, stop=(kc == 1))
                E.act(dst[:, mc, b, :], ps[:, :L2], actf, bias=bcol[:, mc:mc + 1])
        return dst

    c1 = conv3(a_fm, wl1T, wl1b, AF.Gelu, "c1")  # s2p ring
    yield
    c2 = conv3(c1, wl2T, wl2b, AF.Identity, "afm")
    yield
    c2T = E.sb.tile([L2, 2, PB, P], BF16, tag="c2T", name="c2T")
    for b in range(PB):
        for mc in range(2):
            pst = E.pn.tile([P, P], BF16, tag="th", name="pst")
            nc.tensor.transpose(pst[0:L2, :], c2[:, mc, b, :], E.identb)
            nc.scalar.copy(c2T[:, mc, b, :], pst[0:L2, :])
    yield
    for b in range(PB):
        for mc in range(2):
            ps = E.pp.tile([P, 512], F32, tag="mm", name="ps")
            nc.tensor.matmul(ps[:, :S], c2T[:, mc, b, :], iiT, start=True, stop=True)
            nc.scalar.copy(x_wl[:, mc, b * S:(b + 1) * S], ps[:, :S])
    yield

    # ---------------- stage 3: cross-attention + gate + LN
    caWq = E.load_wT(io["caWqT"], D, D, "awq")
    caWk = E.load_wT(io["caWkT"], D, D, "awk")
    caWv = E.load_wT(io["caWvT"], D, D, "awv")
    caWo = E.load_wT(io["caWoT"], D, D, "awo")
    caBq = E.load_col(io["caBq"], D, "abq")
    caBk = E.load_col(io["caBk"], D, "abk")
    caBo = E.load_col(io["caBo"], D, "abo")
    ab = {}
    yield from _g_attention(E, x_fft, x_wl, caWq, caWk, caWv, caWo, caBq, caBk,
                            caBo, "t8", ab, "att")
    fused = E.s3p.tile([P, 2, F], BF16, tag="t8", name="fused")
    E.add(fused, ab["att"], x_fm)
    gateW = E.load_wT(io["gateWT"], 2 * D, 2 * D, "gateW")
    gateB = E.load_col(io["gateB"], 2 * D, "gateB")
    ga = E.s3p.tile([P, 2, F], BF16, tag="t8", name="ga")
    gb = E.s3p.tile([P, 2, F], BF16, tag="t8", name="gb")
    for mc in range(4):
        actf = AF.Identity if mc < 2 else AF.Sigmoid
        gdst = ga if mc < 2 else gb
        ps = E.pp.tile([P, 512], F32, tag="mm", name="ps")
        for kc in range(4):
            gsrc = fused if kc < 2 else x_fm
            nc.tensor.matmul(ps[:, :F], gateW[:, kc, mc * P:(mc + 1) * P],
                             gsrc[:, kc % 2, :], start=(kc == 0), stop=(kc == 3))
        E.act(gdst[:, mc % 2, :], ps[:, :F], actf, bias=gateB[:, mc:mc + 1])
        yield
    gated = ga
    E.mul(gated, ga, gb)
    flt = E.s2p.tile([1, 2, D], BF16, tag="lnFG", name="flt")
    nc.sync.dma_start(out=flt, in_=io["lnFG"][0])
    x1 = E.s2p.tile([P, 2, F], BF16, tag="x1", name="x1", bufs=3)
    yield from _g_layer_norm(E, gated, flt[0:1, 0, :], flt[0:1, 1, :], 1e-5, x1)
    box[key] = x1


_DIRS = (("mf", "af", False, "anf", "nf"),
         ("mb", "ab", True, "anb", "nb"))


def _g_layer_preps(E, io, pss, l, x1f, av, cd):
    x1 = x1f()
    prs = {}
    for (mp, ap_, flip, anG, nG) in _DIRS:
        prs[mp] = {}
        yield from _g_mamba_prep_a(E, io, x1, mp, l, flip, prs[mp])
    for (mp, ap_, flip, anG, nG) in _DIRS:
        bcd = io[f"bcrows_{pss}_{l}_{mp}"]
        yield from _g_mamba_prep_b(E, io, prs[mp], mp, l, bcd)
    cd.update(prs=prs, x1=x1, l=l, io=io, pss=pss)


def _emit_layer_scans(E, cd, av, bg, mid_add=None):
    cd["ms"] = {}
    for di, (mp, ap_, flip, anG, nG) in enumerate(_DIRS):
        cd["ms"][mp] = _mamba_scan(E, cd["prs"][mp], av[mp][cd["l"]],
                                   "ms" + mp, bg, pump=3 if di == 0 else 5)
        if di == 0 and mid_add is not None:
            bg.add(mid_add)


def _g_post_attn(E, cd, mp, ap_):
    nc = E.nc
    io, l = cd["io"], cd["l"]
    ab = {}
    wq = E.load_wT(io[ap_ + "WqT"][l], D, D, "awq" + mp)
    wk = E.load_wT(io[ap_ + "WkT"][l], D, D, "awk" + mp)
    wv = E.load_wT(io[ap_ + "WvT"][l], D, D, "awv" + mp)
    wo = E.load_wT(io[ap_ + "WoT"][l], D, D, "awo" + mp)
    abq = E.load_col(io[ap_ + "Bq"][l], D, "abq" + mp)
    abk = E.load_col(io[ap_ + "Bk"][l], D, "abk" + mp)
    abo = E.load_col(io[ap_ + "Bo"][l], D, "abo" + mp)
    ms = cd["ms"][mp]
    yield from _g_attention(E, ms, ms, wq, wk, wv, wo, abq, abk, abo,
                            "t8", ab, "att")
    E.add(ms, ms, ab["att"])
    cd.setdefault("s2d", {})[mp] = ms
    yield


def _g_post_lnt(E, cd):
    nc = E.nc
    if "lnt" in cd:
        return
    lnt = E.s2p.tile([1, 8, D], BF16, tag="lnAll", name="lnt")
    nc.sync.dma_start(out=lnt, in_=cd["io"]["lnAll"][cd["l"]])
    cd["lnt"] = lnt
    yield


def _ln_params(cd, name):
    nidx = {"anf": 0, "anb": 1, "nf": 2, "nb": 3}
    i = nidx[name] * 2
    lnt = cd["lnt"]
    return (lnt[0:1, i, :], lnt[0:1, i + 1, :])


def _g_post_lns(E, cd, mp, flip, anG, nG):
    x1 = cd["x1"]
    s3 = E.s3p.tile([P, 2, F], BF16, tag="t8", name="s3")
    (ang, anb_) = _ln_params(cd, anG)
    yield from _g_layer_norm(E, cd["s2d"][mp], ang, anb_, 1e-5, s3)
    s4 = E.s3p.tile([P, 2, F], BF16, tag="t8", name="s4")
    if flip:
        for kc in range(2):
            E.add(s4[:, kc, :].rearrange("p (b s) -> p b s", b=PB),
                  rev_view(s3[:, kc, :], PB, S),
                  x1[:, kc, :].rearrange("p (b s) -> p b s", b=PB))
    else:
        E.add(s4, s3, x1)
    yield
    s5 = E.s2p.tile([P, 2, F], BF16, tag="s5", name="s5")
    (ng, nb_) = _ln_params(cd, nG)
    yield from _g_layer_norm(E, s4, ng, nb_, 1e-5, s5)
    cd.setdefault("s5d", {})[mp] = s5


def _g_layer_post(E, cd, box, key):
    done = cd.get("s2d", {})
    for (mp, ap_, flip, anG, nG) in _DIRS:
        if mp not in done:
            yield from _g_post_attn(E, cd, mp, ap_)
    yield from _g_post_lnt(E, cd)
    s5d = cd.get("s5d", {})
    for (mp, ap_, flip, anG, nG) in _DIRS:
        if mp not in s5d:
            yield from _g_post_lns(E, cd, mp, flip, anG, nG)
    x1n = E.s2p.tile([P, 2, F], BF16, tag="x1", name="x1n", bufs=3)
    E.add(x1n, cd["s5d"]["mf"], cd["s5d"]["mb"])
    box[key] = x1n


def _g_glu(E, io, pss, x1f):
    nc = E.nc
    x1 = x1f()
    # ---------------- stage 5: GLU + final LN
    glu1W = E.load_wT(io["glu1WT"], D, 2 * D, "glu1W")
    glu1B = E.load_col(io["glu1B"], 2 * D, "glu1B")
    va = E.s3p.tile([P, 2, F], BF16, tag="t8", name="va")
    vb = E.s3p.tile([P, 2, F], BF16, tag="t8", name="vb")
    for mc in range(4):
        actf = AF.Identity if mc < 2 else AF.Sigmoid
        vdst = va if mc < 2 else vb
        ps = E.pp.tile([P, 512], F32, tag="mm", name="ps")
        for kc in range(2):
            nc.tensor.matmul(ps[:, :F], glu1W[:, kc, mc * P:(mc + 1) * P],
                             x1[:, kc, :], start=(kc == 0), stop=(kc == 1))
        E.act(vdst[:, mc % 2, :], ps[:, :F], actf, bias=glu1B[:, mc:mc + 1])
        yield
    gv = va
    E.mul(gv, va, vb)
    glu2W = E.load_wT(io["glu2WT"], D, D, "glu2W")
    glu2B = E.load_col(io["glu2B"], D, "glu2B")
    gvo = E.dense(gv, glu2W, D, bias=glu2B, out_tag="t8")
    yield
    res = E.sb.tile([P, 2, F], F32, tag="res", name="res")
    E.add(res, gvo, x1)
    glt = E.s2p.tile([1, 2, D], BF16, tag="lnFG", name="glt")
    nc.sync.dma_start(out=glt, in_=io["lnFG"][1])
    out_fm = E.sb.tile([P, 2, F], F32, tag="reso", name="out_fm")
    yield from _g_layer_norm(E, res, glt[0:1, 0, :], glt[0:1, 1, :], 1e-12, out_fm,
                             x_is_f32=True)

    # ---------------- stage 6: transpose + store
    for b in range(PB):
        for sc in range(2):
            ot = E.sb.tile([P, D], F32, tag="otile", name="ot")
            for dc in range(2):
                pst = E.pn.tile([P, P], F32, tag="th", name="pst")
                nc.tensor.transpose(pst, out_fm[:, dc, b * S + sc * P: b * S + (sc + 1) * P],
                                    E.identf)
                nc.scalar.copy(ot[:, dc * P:(dc + 1) * P], pst)
            nc.sync.dma_start(out=io["out"][pss * PB + b, sc * P:(sc + 1) * P, :], in_=ot)
            yield


# ------------------------------------------------------------------- driver
_CACHE = {}


def _get_program(w, av):
    wshapes = []
    for k, v in sorted(w.items()):
        dt = BF16 if v.dtype.itemsize == 2 else F32
        wshapes.append((k, tuple(v.shape), dt))
    avh = hashlib.sha256(
        b"".join(np.ascontiguousarray(av[p]).tobytes() for p in ("mf", "mb"))
    ).hexdigest()
    key = (tuple(wshapes), avh)
    if key not in _CACHE:
        _CACHE[key] = build_program(wshapes, av)
    return _CACHE[key]


def kernel(**inputs):
    from concourse.bass_utils import run_bass_kernel_spmd
    w = _prep_weights(inputs)
    av = _scan_consts(inputs)
    nc = _get_program(w, av)
    x = np.ascontiguousarray(np.asarray(inputs["input_tensor"], np.float32))
    in_maps = []
    for core in range(NCORES):
        m = {"input": np.ascontiguousarray(x[core * BC:(core + 1) * BC])}
        m.update(w)
        in_maps.append(m)
    res = run_bass_kernel_spmd(nc, in_maps, list(range(NCORES)))
    return np.concatenate([res.results[i]["out"] for i in range(NCORES)], axis=0)
